# revision 1
# baseline (speedup 1.0000x reference)
"""GPT2 causal attention (B=2, T=2048, C=1024, H=16) on 8 TRN2 NeuronCores.

Sharding: core g = (batch b = g//4, head-group hg = g%4 of 4 heads).
Tensor-parallel over heads (column-split W_attn, row-split W_proj) x
data-parallel over batch. Each core computes a full [T, C] partial of the
output projection for its 4 heads; host sums the 4 partials per batch and
adds b_proj. No collectives.

Per-core kernel (bf16 matmuls, fp32 PSUM):
  qT/kT in [d, T] layout, V in [T, d] natural layout with a ones-column
  per head (so attention*V also produces the softmax row-sums). Scores are
  computed transposed, S^T[tk, tq] = kT_tile^T @ qT, exp'd without
  max-subtraction (scores ~ N(0,1)), causal tiles only, diagonal 128x128
  blocks masked with a host tri mask (left-of-diagonal junk memset to 0).
  Yu^T[d, tq] = V_aug^T @ expS^T accumulates over tk in PSUM. Row-sum
  reciprocals go through a DRAM bounce to land 128-lane for DVE recip,
  then broadcast back; yT is normalized in place and fed to the output
  projection as the stationary operand.
"""

import numpy as np
import ml_dtypes

BF16 = ml_dtypes.bfloat16

B, T, C, H, D = 2, 2048, 1024, 16, 64
HL = 4          # heads per core
DL = HL * D     # 256 local head dims
N_CORES = 8
NT = T // 128   # 16 tk tiles
NJ = T // 512   # 4 tq groups
SCALE = 1.0 / np.sqrt(D)

_CACHE = {}


def _build_program():
    import concourse.tile as tile
    from concourse import bacc
    import concourse.mybir as mybir

    f32 = mybir.dt.float32
    bf16 = mybir.dt.bfloat16
    Exp = mybir.ActivationFunctionType.Exp

    nc = bacc.Bacc("TRN2", target_bir_lowering=False, debug=False)

    # ---- DRAM I/O (host pre-sharded and pre-packed to SBUF layout) ----
    xT_d = nc.dram_tensor("xTp", [128, 8 * T], bf16, kind="ExternalInput").ap()
    wq_d = nc.dram_tensor("wqp", [128, 8 * DL], bf16, kind="ExternalInput").ap()
    wk_d = nc.dram_tensor("wkp", [128, 8 * DL], bf16, kind="ExternalInput").ap()
    wv_d = nc.dram_tensor("wvp", [128, 8 * DL], bf16, kind="ExternalInput").ap()
    wp_d = nc.dram_tensor("wpp", [128, 2 * C], bf16, kind="ExternalInput").ap()
    bq_d = nc.dram_tensor("bq", [128, 2], f32, kind="ExternalInput").ap()
    bk_d = nc.dram_tensor("bk", [128, 2], f32, kind="ExternalInput").ap()
    bvr_d = nc.dram_tensor("bvr", [128, DL], f32, kind="ExternalInput").ap()
    tri_d = nc.dram_tensor("tri", [128, 128], bf16, kind="ExternalInput").ap()
    out_d = nc.dram_tensor("out", [T, C], f32, kind="ExternalOutput").ap()
    s_dram = nc.dram_tensor("s_scratch", [HL * T], f32).ap()
    r_dram = nc.dram_tensor("r_scratch", [HL * T], bf16).ap()

    with tile.TileContext(nc) as tc:
        with (
            tc.tile_pool(name="const", bufs=1) as cpool,
            tc.tile_pool(name="exp", bufs=8) as epool,
            tc.tile_pool(name="rep", bufs=6) as rpool,
            tc.tile_pool(name="small", bufs=6) as spool,
            tc.tile_pool(name="ostage", bufs=6) as opool,
            tc.tile_pool(name="psbig", bufs=3, space="PSUM") as pbig,
            tc.tile_pool(name="psyu", bufs=2, space="PSUM") as pyu,
        ):
            # ---- persistent SBUF ----
            xT = cpool.tile([128, 8 * T], bf16, tag="xT")       # c-chunk c at [:, c*T:]
            wq = cpool.tile([128, 8 * DL], bf16, tag="wq")
            wk = cpool.tile([128, 8 * DL], bf16, tag="wk")
            wv = cpool.tile([128, 8 * DL], bf16, tag="wv")
            wp = cpool.tile([128, 2 * C], bf16, tag="wp")       # d-chunk dc at [:, dc*C:]
            bq = cpool.tile([128, 2], f32, tag="bq")
            bk = cpool.tile([128, 2], f32, tag="bk")
            bvr = cpool.tile([128, DL], f32, tag="bvr")
            tri = cpool.tile([128, 128], bf16, tag="tri")
            qT = cpool.tile([128, 2 * T], bf16, tag="qT")       # head h: [64*(h%2):, (h//2)*T + t]
            kT = cpool.tile([128, 2 * T], bf16, tag="kT")
            yT = cpool.tile([128, 2 * T], bf16, tag="yT")
            V = cpool.tile([128, NT * (HL * 65)], bf16, tag="V")  # t-tile tt, head h at [:, tt*260 + 65*h : +65]

            # ---- load inputs (few big DMAs, ordered so PE starts earliest) ----
            nc.sync.dma_start(out=wq[:, :], in_=wq_d[:, :])
            nc.sync.dma_start(out=bq[:, :], in_=bq_d[:, :])
            for c in range(8):  # per-chunk so the first QKV matmuls start early
                nc.sync.dma_start(out=xT[:, c * T:(c + 1) * T],
                                  in_=xT_d[:, c * T:(c + 1) * T])
            nc.sync.dma_start(out=wk[:, :], in_=wk_d[:, :])
            nc.sync.dma_start(out=bk[:, :], in_=bk_d[:, :])
            nc.sync.dma_start(out=wv[:, :], in_=wv_d[:, :])
            nc.sync.dma_start(out=bvr[:, :], in_=bvr_d[:, :])
            nc.sync.dma_start(out=tri[:, :], in_=tri_d[:, :])
            nc.sync.dma_start(out=wp[:, :], in_=wp_d[:, :])

            # ---- QKV projections ----
            for (w_sb, b_sb, dst) in ((wq, bq, qT), (wk, bk, kT)):
                for dc in range(2):
                    for ts in range(4):
                        ps = pbig.tile([128, 512], f32, tag="big")
                        for c in range(8):
                            nc.tensor.matmul(
                                ps[:, :],
                                w_sb[:, c * DL + dc * 128: c * DL + (dc + 1) * 128],
                                xT[:, c * T + ts * 512: c * T + (ts + 1) * 512],
                                start=(c == 0), stop=(c == 7),
                            )
                        nc.vector.tensor_scalar_add(
                            dst[:, dc * T + ts * 512: dc * T + (ts + 1) * 512],
                            ps[:, :], b_sb[:, dc:dc + 1],
                        )

            # V natural [t, d]: stationary xT chunk, moving W_v chunk.
            nc.vector.memset(V[:, :], 1.0)  # ones-columns; data cols overwritten
            for tt in range(NT):
                ps = pbig.tile([128, DL], f32, tag="big")
                for c in range(8):
                    nc.tensor.matmul(
                        ps[:, :],
                        xT[:, c * T + tt * 128: c * T + (tt + 1) * 128],
                        wv[:, c * DL:(c + 1) * DL],
                        start=(c == 0), stop=(c == 7),
                    )
                vdst = V[:, tt * (HL * 65): (tt + 1) * (HL * 65)].rearrange(
                    "p (h e) -> p h e", h=HL)[:, :, 0:64]
                nc.vector.tensor_add(
                    vdst,
                    ps[:, :].rearrange("p (h e) -> p h e", h=HL),
                    bvr[:, :].rearrange("p (h e) -> p h e", h=HL),
                )

            # ---- attention, head-pair interleaved, j-outer ----
            # Even/odd head score matmuls (K=64) land back-to-back with
            # tile_position rows (0,0)/(64,0), so they run concurrently in
            # the PE array's two row-group halves.
            s_view = s_dram.rearrange("(h c p) -> p h c", h=HL, c=16)
            r_view = r_dram.rearrange("(h c p) -> p h c", h=HL, c=16)
            for hp in range(2):
                fb = hp * T               # free-dim base for this head pair
                for j in range(NJ):
                    yu = [
                        pyu.tile([65, 512], f32, tag="yu", name=f"yu_{hp}_{j}_{half}")
                        for half in (0, 1)
                    ]
                    ni = 4 * j + 4        # causal: tk tiles 0..4j+3
                    for i in range(ni):
                        sc = pbig.tile([128, 1024], f32, tag="big",
                                       name=f"sc_{hp}_{j}_{i}")
                        for half in (0, 1):
                            po = 64 * half
                            nc.tensor.matmul(
                                sc[:, half * 512:(half + 1) * 512],
                                kT[po:po + 64, fb + i * 128: fb + (i + 1) * 128],
                                qT[po:po + 64, fb + j * 512: fb + (j + 1) * 512],
                                start=True, stop=True,
                            )
                        d0 = max(128 * (i - 4 * j), 0)  # diag offset in slice
                        et = epool.tile([128, 1024], bf16, tag="exp",
                                        name=f"et_{hp}_{j}_{i}")
                        et2 = et[:, :].rearrange("p (g q) -> p g q", g=2)
                        sc2 = sc[:, :].rearrange("p (g q) -> p g q", g=2)
                        nc.scalar.activation(
                            et2[:, :, d0:512], sc2[:, :, d0:512],
                            Exp, scale=float(SCALE),
                        )
                        if d0 > 0:
                            nc.vector.memset(et2[:, :, 0:d0], 0.0)
                        if i >= 4 * j:  # diagonal band: mask both halves
                            for half in (0, 1):
                                sl = slice(half * 512 + d0, half * 512 + d0 + 128)
                                nc.vector.tensor_mul(et[:, sl], et[:, sl], tri[:, :])
                        for half in (0, 1):
                            h = 2 * hp + half
                            nc.tensor.matmul(
                                yu[half][:, :],
                                V[:, i * (HL * 65) + 65 * h: i * (HL * 65) + 65 * h + 65],
                                et[:, half * 512:(half + 1) * 512],
                                start=(i == 0), stop=(i == ni - 1),
                            )
                    # Evict yu through a [65,512] fp32 stage: row 64 is the
                    # softmax denominator, rows 0-63 the unnormalized yT.
                    # DVE lanes are physical, so odd heads (po=64) cross
                    # partitions via a casting gpsimd DMA instead of DVE.
                    for half in (0, 1):
                        h = 2 * hp + half
                        stg = spool.tile([65, 512], f32, tag="stg",
                                         name=f"stg_{h}_{j}")
                        nc.vector.tensor_copy(stg[:, :], yu[half][:, :])
                        if half == 0:
                            nc.vector.tensor_copy(
                                yT[0:64, fb + j * 512: fb + (j + 1) * 512],
                                stg[0:64, :],
                            )
                        else:
                            nc.gpsimd.dma_start(
                                out=yT[64:128, fb + j * 512: fb + (j + 1) * 512],
                                in_=stg[0:64, :],
                            )
                        nc.sync.dma_start(
                            out=s_dram[h * T + j * 512: h * T + (j + 1) * 512],
                            in_=stg[64:65, :],
                        )
                    # per-(pair,j) reciprocal dance (overlaps later j's):
                    # DRAM-bounce both heads' [512] sums into [128,8] for
                    # 128-lane recip, broadcast back, normalize yT in place.
                    sT = spool.tile([128, 8], f32, tag="sT",
                                    name=f"sT_{hp}_{j}")
                    for half in (0, 1):
                        nc.sync.dma_start(
                            out=sT[:, half * 4:(half + 1) * 4],
                            in_=s_view[:, 2 * hp + half, 4 * j:4 * j + 4],
                        )
                    rT = spool.tile([128, 8], f32, tag="rT",
                                    name=f"rT_{hp}_{j}")
                    nc.vector.reciprocal(rT[:, :], sT[:, :])
                    rTb = spool.tile([128, 8], bf16, tag="rTb",
                                     name=f"rTb_{hp}_{j}")
                    nc.vector.tensor_copy(rTb[:, :], rT[:, :])
                    for half in (0, 1):
                        nc.sync.dma_start(
                            out=r_view[:, 2 * hp + half, 4 * j:4 * j + 4],
                            in_=rTb[:, half * 4:(half + 1) * 4],
                        )
                    for half in (0, 1):
                        h = 2 * hp + half
                        po = 64 * half
                        rep = rpool.tile([128, 512], bf16, tag="rep",
                                         name=f"rep_{h}_{j}")
                        nc.sync.dma_start(
                            out=rep[po:po + 64, :],
                            in_=r_dram[h * T + j * 512:
                                       h * T + (j + 1) * 512].partition_broadcast(64),
                        )
                        nc.vector.tensor_mul(
                            yT[po:po + 64, fb + j * 512: fb + (j + 1) * 512],
                            yT[po:po + 64, fb + j * 512: fb + (j + 1) * 512],
                            rep[po:po + 64, :],
                        )

            # ---- output projection: out[t, c] = sum_d yT[d, t] * wp[d, c] ----
            for tt in range(NT):
                for cc in range(2):
                    pp = pbig.tile([128, 512], f32, tag="big")
                    for dc in range(2):
                        nc.tensor.matmul(
                            pp[:, :],
                            yT[:, dc * T + tt * 128: dc * T + (tt + 1) * 128],
                            wp[:, dc * C + cc * 512: dc * C + (cc + 1) * 512],
                            start=(dc == 0), stop=(dc == 1),
                        )
                    ot = opool.tile([128, 512], f32, tag="ot")
                    if (tt + cc) % 2 == 0:
                        nc.scalar.copy(ot[:, :], pp[:, :])
                    else:
                        nc.vector.tensor_copy(ot[:, :], pp[:, :])
                    nc.sync.dma_start(
                        out=out_d[tt * 128:(tt + 1) * 128, cc * 512:(cc + 1) * 512],
                        in_=ot[:, :],
                    )

    nc.compile()
    return nc


def get_program():
    if "nc" not in _CACHE:
        _CACHE["nc"] = _build_program()
    return _CACHE["nc"]


def _pack_cmajor(a):
    """[C_rows, N] -> [128, (C_rows/128)*N] with chunk c at [:, c*N:(c+1)*N]."""
    rows, n = a.shape
    return np.ascontiguousarray(
        a.reshape(rows // 128, 128, n).transpose(1, 0, 2).reshape(128, -1))


def make_in_maps(x, W_attn, b_attn, W_proj):
    """Host-side sharding: per-core input dict."""
    x = np.asarray(x, np.float32)
    W_attn = np.asarray(W_attn, np.float32)
    b_attn = np.asarray(b_attn, np.float32)
    W_proj = np.asarray(W_proj, np.float32)

    tk = np.arange(128)[:, None]
    tq = np.arange(128)[None, :]
    tri = (tq >= tk).astype(BF16)

    xT_b = [_pack_cmajor(x[b].T.astype(BF16)) for b in range(B)]

    in_maps = []
    for g in range(N_CORES):
        b, hg = divmod(g, 4)
        cs = slice(hg * DL, (hg + 1) * DL)
        wq = _pack_cmajor(W_attn[:, 0 * C:1 * C][:, cs].astype(BF16))
        wk = _pack_cmajor(W_attn[:, 1 * C:2 * C][:, cs].astype(BF16))
        wv = _pack_cmajor(W_attn[:, 2 * C:3 * C][:, cs].astype(BF16))
        wp = _pack_cmajor(W_proj[cs, :].astype(BF16))
        bq = np.ascontiguousarray(b_attn[0 * C:1 * C][cs].reshape(2, 128).T)
        bk = np.ascontiguousarray(b_attn[1 * C:2 * C][cs].reshape(2, 128).T)
        bvr = np.ascontiguousarray(np.tile(b_attn[2 * C:3 * C][cs][None, :], (128, 1)))
        in_maps.append({
            "xTp": xT_b[b],
            "wqp": wq, "wkp": wk, "wvp": wv, "wpp": wp,
            "bq": bq.astype(np.float32), "bk": bk.astype(np.float32),
            "bvr": bvr.astype(np.float32),
            "tri": tri,
        })
    return in_maps


def assemble_output(results, b_proj):
    """results: per-core dicts with 'out' [T, C] partials."""
    b_proj = np.asarray(b_proj, np.float32)
    out = np.zeros((B, T, C), np.float32)
    for g in range(N_CORES):
        out[g // 4] += np.asarray(results[g]["out"], np.float32)
    out += b_proj[None, None, :]
    return out


def kernel(x, W_attn, b_attn, W_proj, b_proj):
    from concourse.bass_utils import run_bass_kernel_spmd

    nc = get_program()
    in_maps = make_in_maps(x, W_attn, b_attn, W_proj)
    res = run_bass_kernel_spmd(nc, in_maps, list(range(N_CORES)))
    return assemble_output(res.results, b_proj)



# revision 11
# speedup vs baseline: 1.1684x; 1.1684x over previous
"""GPT2 causal attention (B=2, T=2048, C=1024, H=16) on 8 TRN2 NeuronCores.

Sharding: core g = (batch b = g//4, head-group hg = g%4 of 4 heads).
Tensor-parallel over heads x data-parallel over batch. Each core emits a
full [T, C] bf16 partial of the output projection for its 4 heads; host
sums the 4 partials per batch and adds b_proj. No collectives.

Per-core kernel:
  QKV projections run in compensated fp8 (e4m3) DoubleRow matmuls:
  x = x8 + xr and W = W8 + Wr host-split (W pre-scaled by 64), with
  x@W ~= x8@W8 + x8@Wr + xr@W8 (error ~ xr@Wr = O(eps^2), below bf16).
  Each DoubleRow matmul contracts 256 rows (a pair of 128-row k-tiles).

  Scores per head pair, transposed: S^T[tk, tq] = kT^T @ qT in bf16,
  causally trimmed to [d0:512] at 128-col granularity (both the matmuls
  and the exp), exp'd on ACT without max subtraction, diagonal 128-blocks
  masked with a host tri mask on DVE.

  attention*V runs in the natural orientation: per 128-row tq tile,
  yu[tq, (h, 65)] = et_chunk^T @ V_aug accumulates over tk tiles in PSUM;
  V_aug carries a ones-column per head so column 64 of each head group is
  the softmax row-sum, landing on the free dim. Normalization is then
  per-partition: gpsimd normalize_recip (attn library) divides by the sum
  and writes bf16. y is transposed back to [d, t] with XBAR DMA
  transposes for the bf16 output projection.
"""

import numpy as np
import ml_dtypes

BF16 = ml_dtypes.bfloat16
F8 = ml_dtypes.float8_e4m3

B, T, C, H, D = 2, 2048, 1024, 16, 64
HL = 4          # heads per core
DL = HL * D     # 256 local head dims
N_CORES = 8
NT = T // 128   # 16 t tiles
NJ = T // 512   # 4 tq groups
SCALE = 1.0 / np.sqrt(D)
WSC = 64.0      # host pre-scale on W_attn for fp8 range
VSTR = HL * 65  # V tile col stride (4 heads x (64 d + ones col))
YOFF = (0, 130, 260, 512)  # yu subtile col offsets (none crosses a bank)

_CACHE = {}


def _build_program():
    import concourse.tile as tile
    from concourse import bacc
    from concourse import library_config
    import concourse.mybir as mybir

    f32 = mybir.dt.float32
    bf16 = mybir.dt.bfloat16
    fp8 = mybir.dt.float8e4
    Exp = mybir.ActivationFunctionType.Exp
    DR = mybir.MatmulPerfMode.DoubleRow

    nc = bacc.Bacc("TRN2", target_bir_lowering=False, debug=False)

    x8_d = nc.dram_tensor("x8p", [128, 8 * T], fp8, kind="ExternalInput").ap()
    xr_d = nc.dram_tensor("xrp", [128, 8 * T], fp8, kind="ExternalInput").ap()
    w8_d = {k: nc.dram_tensor(f"w8{k}", [128, 8 * DL], fp8,
                              kind="ExternalInput").ap() for k in "qkv"}
    wr_d = {k: nc.dram_tensor(f"wr{k}", [128, 8 * DL], fp8,
                              kind="ExternalInput").ap() for k in "qkv"}
    wp_d = nc.dram_tensor("wpp", [128, 2 * C], bf16, kind="ExternalInput").ap()
    bq_d = nc.dram_tensor("bq", [128, 2], f32, kind="ExternalInput").ap()
    bk_d = nc.dram_tensor("bk", [128, 2], f32, kind="ExternalInput").ap()
    bvr_d = nc.dram_tensor("bvr", [128, DL], f32, kind="ExternalInput").ap()
    tri_d = nc.dram_tensor("tri", [128, 128], bf16, kind="ExternalInput").ap()
    out_d = nc.dram_tensor("out", [T, C], bf16, kind="ExternalOutput").ap()
    import os
    dbg = os.environ.get("K_DEBUG") == "1"
    if dbg:
        dbg_d = {n: nc.dram_tensor(f"dbg_{n}", [128, 2 * T], bf16,
                                   kind="ExternalOutput").ap()
                 for n in ("qT", "kT", "yT")}
        dbgv_d = nc.dram_tensor("dbg_V", [128, NT * VSTR], bf16,
                                kind="ExternalOutput").ap()
        dbgyn_d = nc.dram_tensor("dbg_yn", [128, 2 * T], bf16,
                                 kind="ExternalOutput").ap()
        dbgstg_d = nc.dram_tensor("dbg_stg", [128, 32 * 130], f32,
                                  kind="ExternalOutput").ap()

    with tile.TileContext(nc) as tc:
        with (
            tc.tile_pool(name="const", bufs=1) as cpool,
            tc.tile_pool(name="exp", bufs=6) as epool,
            tc.tile_pool(name="ystg", bufs=4) as spool,
            tc.tile_pool(name="ynat", bufs=4) as npool,
            tc.tile_pool(name="ostage", bufs=4) as opool,
            tc.tile_pool(name="pssc", bufs=2, space="PSUM") as psc,
            tc.tile_pool(name="psqkv", bufs=2, space="PSUM") as pqkv,
            tc.tile_pool(name="psyu", bufs=1, space="PSUM") as pyu,
        ):
            # ---- persistent SBUF ----
            x8 = cpool.tile([128, 8 * T], fp8, tag="x8")     # chunk c at c*T
            xr = cpool.tile([128, 8 * T], fp8, tag="xr")
            w8 = {k: cpool.tile([128, 8 * DL], fp8, tag=f"w8{k}", name=f"w8{k}")
                  for k in "qkv"}
            wr = {k: cpool.tile([128, 8 * DL], fp8, tag=f"wr{k}", name=f"wr{k}")
                  for k in "qkv"}
            wp = cpool.tile([128, 2 * C], bf16, tag="wp")    # d-chunk dc at dc*C
            bq = cpool.tile([128, 2], f32, tag="bq")
            bk = cpool.tile([128, 2], f32, tag="bk")
            bvr = cpool.tile([128, DL], f32, tag="bvr")
            tri = cpool.tile([128, 128], bf16, tag="tri")
            qT = cpool.tile([128, 2 * T], bf16, tag="qT")    # head h: rows 64*(h%2), cols (h//2)*T+t
            kT = cpool.tile([128, 2 * T], bf16, tag="kT")
            yT = cpool.tile([128, 2 * T], bf16, tag="yT")    # d-chunk dc at dc*T
            V = cpool.tile([128, NT * VSTR], bf16, tag="V")  # tile tt, head h at tt*VSTR+65h

            nc.gpsimd.load_library(library_config.attn)

            # ---- input DMAs, ordered for earliest PE start ----
            nc.sync.dma_start(out=w8["q"][:, :], in_=w8_d["q"][:, :])
            nc.sync.dma_start(out=bq[:, :], in_=bq_d[:, :])
            for c in range(4):  # x8 in DoubleRow 2-chunk pairs
                nc.sync.dma_start(out=x8[:, 2 * c * T:(2 * c + 2) * T],
                                  in_=x8_d[:, 2 * c * T:(2 * c + 2) * T])
            nc.sync.dma_start(out=w8["k"][:, :], in_=w8_d["k"][:, :])
            nc.sync.dma_start(out=bk[:, :], in_=bk_d[:, :])
            nc.sync.dma_start(out=w8["v"][:, :], in_=w8_d["v"][:, :])
            nc.sync.dma_start(out=bvr[:, :], in_=bvr_d[:, :])
            for k in "qkv":
                nc.sync.dma_start(out=wr[k][:, :], in_=wr_d[k][:, :])
            for c in range(4):
                nc.sync.dma_start(out=xr[:, 2 * c * T:(2 * c + 2) * T],
                                  in_=xr_d[:, 2 * c * T:(2 * c + 2) * T])
            nc.sync.dma_start(out=tri[:, :], in_=tri_d[:, :])
            nc.sync.dma_start(out=wp[:, :], in_=wp_d[:, :])

            x8c = x8[:, :].rearrange("p (c t) -> p c t", c=8)
            xrc = xr[:, :].rearrange("p (c t) -> p c t", c=8)
            w8c = {k: w8[k][:, :].rearrange("p (c m) -> p c m", c=8) for k in "qkv"}
            wrc = {k: wr[k][:, :].rearrange("p (c m) -> p c m", c=8) for k in "qkv"}
            TERMS = [(x8c, w8c), (x8c, wrc), (xrc, w8c)]

            def qk_tile(key, dc, ts, pool):
                """One [128d, 512t] q/k projection tile + bias eviction."""
                b_sb, dst = (bq, qT) if key == "q" else (bk, kT)
                ps = pool.tile([128, 512], f32, tag=pool._qkv_tag,
                               name=f"p{key}_{dc}_{ts}")
                msl = slice(dc * 128, (dc + 1) * 128)
                nsl = slice(ts * 512, (ts + 1) * 512)
                for term, (xa, wd) in enumerate(TERMS):
                    for c in range(4):
                        nc.tensor.matmul(
                            ps[:, 0:512],
                            wd[key][:, 2 * c:2 * c + 2, msl],
                            xa[:, 2 * c:2 * c + 2, nsl],
                            start=(term == 0 and c == 0),
                            stop=(term == 2 and c == 3),
                            perf_mode=DR,
                        )
                nc.vector.tensor_scalar_add(
                    dst[:, dc * T + ts * 512: dc * T + (ts + 1) * 512],
                    ps[:, 0:512], b_sb[:, dc:dc + 1])

            def v_tile(tt, pool):
                """One [128t, 256d] V tile (natural layout) + bias eviction."""
                ps = pool.tile([128, 512], f32, tag=pool._qkv_tag,
                               name=f"pv_{tt}")
                tsl = slice(tt * 128, (tt + 1) * 128)
                for term, (xa, wd) in enumerate(TERMS):
                    for c in range(4):
                        nc.tensor.matmul(
                            ps[:, 0:DL],
                            xa[:, 2 * c:2 * c + 2, tsl],
                            wd["v"][:, 2 * c:2 * c + 2, :],
                            start=(term == 0 and c == 0),
                            stop=(term == 2 and c == 3),
                            perf_mode=DR,
                        )
                vdst = V[:, tt * VSTR:(tt + 1) * VSTR].rearrange(
                    "p (h e) -> p h e", h=HL)[:, :, 0:64]
                nc.vector.tensor_add(
                    vdst, ps[:, 0:DL].rearrange("p (h e) -> p h e", h=HL),
                    bvr[:, :].rearrange("p (h e) -> p h e", h=HL))

            psc._qkv_tag = "sc"
            pqkv._qkv_tag = "qkv"

            # ---- startup QKV: q/k dc0 + V0..3 on the (idle) score ring ----
            nc.vector.memset(V[:, :], 1.0)  # ones cols; data overwritten
            for ts in range(4):
                qk_tile("q", 0, ts, psc)
            for ts in range(4):
                qk_tile("k", 0, ts, psc)
            for tt in range(4):
                v_tile(tt, psc)

            # filler thunks: spread through attention on the pqkv ring.
            # V tiles first — each must be EMITTED before the attention
            # group that consumes it (program order is the dataflow).
            filler = []
            for tt in range(4, 16):
                filler.append(lambda tt=tt: v_tile(tt, pqkv))
            for ts in range(4):
                filler.append(lambda ts=ts: qk_tile("q", 1, ts, pqkv))
                filler.append(lambda ts=ts: qk_tile("k", 1, ts, pqkv))

            pending = []   # (ready_step, thunk) deferred PE work
            step = [0]

            def tick():
                step[0] += 1
                if filler:
                    filler.pop(0)()
                if pending and pending[0][0] <= step[0]:
                    pending.pop(0)[1]()

            def proj_tile(tt, last):
                """Output projection for one 128-row t tile: [128, 1024]."""
                ot = opool.tile([128, 1024], bf16, tag="ot", name=f"ot_{tt}")
                for cc in range(2):
                    pp = pqkv.tile([128, 512], f32, tag="qkv",
                                   name=f"po_{tt}_{cc}")
                    for dc in range(2):
                        nc.tensor.matmul(
                            pp[:, :],
                            yT[:, dc * T + tt * 128: dc * T + (tt + 1) * 128],
                            wp[:, dc * C + cc * 512: dc * C + (cc + 1) * 512],
                            start=(dc == 0), stop=(dc == 1),
                        )
                    nc.vector.tensor_copy(ot[:, cc * 512:(cc + 1) * 512],
                                          pp[:, :])
                nc.sync.dma_start(
                    out=out_d[tt * 128:(tt + 1) * 128, :], in_=ot[:, :])

            def finish_tile(hp, j, tl, yu, last_group):
                """After tile tl's diagonal yu: stage, normalize, transpose,
                and (in the hp1 phase) queue the output projection."""
                tt = 4 * j + tl
                stg = spool.tile([128, 130], f32, tag="stg",
                                 name=f"stg_{hp}_{tt}")
                nc.vector.tensor_copy(stg[:, :],
                                      yu[:, YOFF[tl]:YOFF[tl] + 130])
                if dbg:
                    nc.sync.dma_start(
                        out=dbgstg_d[:, (hp * NT + tt) * 130:
                                     (hp * NT + tt + 1) * 130],
                        in_=stg[:, :])
                yn = npool.tile([128, 128], bf16, tag="yn",
                                name=f"yn_{hp}_{tt}")
                for lh in range(2):
                    nc.gpsimd.normalize_recip(
                        yn[:, lh * 64:(lh + 1) * 64],
                        stg[:, lh * 65: lh * 65 + 64],
                        stg[:, lh * 65 + 64: lh * 65 + 65],
                    )
                if dbg:
                    nc.sync.dma_start(
                        out=dbgyn_d[:, hp * T + tt * 128:
                                    hp * T + (tt + 1) * 128],
                        in_=yn[:, :])
                nc.sync.dma_start(
                    out=yT[:, hp * T + tt * 128: hp * T + (tt + 1) * 128],
                    in_=yn[:, :], transpose=True)
                if hp == 1:
                    if last_group and tl == 3:
                        proj_tile(tt, True)   # tail: emit immediately
                    else:
                        pending.append((step[0] + 3,
                                        lambda tt=tt: proj_tile(tt, False)))

            def attn_group(hp, j, last_group):
                ni = 4 * j + 4
                fb = hp * T
                yu = pyu.tile([128, 642], f32, tag="yu", name=f"yu_{hp}_{j}")
                prev = None  # deferred yu batch (software pipeline depth 1)
                for i in range(ni):
                    d0 = max(128 * (i - 4 * j), 0)
                    sc = psc.tile([128, 1024], f32, tag="sc",
                                  name=f"sc_{hp}_{j}_{i}")
                    for half in (0, 1):
                        po = 64 * half
                        nc.tensor.matmul(
                            sc[:, half * 512 + d0:(half + 1) * 512],
                            kT[po:po + 64, fb + i * 128: fb + (i + 1) * 128],
                            qT[po:po + 64, fb + j * 512 + d0: fb + (j + 1) * 512],
                            start=True, stop=True,
                        )
                    et = epool.tile([128, 1024], bf16, tag="exp",
                                    name=f"et_{hp}_{j}_{i}")
                    nc.scalar.activation(
                        et[:, :].rearrange("p (g q) -> p g q", g=2)[:, :, d0:512],
                        sc[:, :].rearrange("p (g q) -> p g q", g=2)[:, :, d0:512],
                        Exp, scale=float(SCALE / (WSC * WSC)),
                    )
                    if i >= 4 * j:
                        for half in (0, 1):
                            sl = slice(half * 512 + d0, half * 512 + d0 + 128)
                            nc.vector.tensor_mul(et[:, sl], et[:, sl], tri[:, :])
                    if prev is not None:
                        prev()
                    def yu_batch(i=i, d0=d0, et=et):
                        # start=True clears has_written for the WHOLE PSUM
                        # bank, so only the first matmul into each bank of
                        # the yu tile may carry it; later subtiles first-
                        # write on cleared bits (overwrite) with start=False.
                        for tl in range(4):
                            if 128 * tl < d0:
                                continue
                            for half in (0, 1):
                                h = 2 * hp + half
                                nc.tensor.matmul(
                                    yu[:, YOFF[tl] + 65 * half:
                                       YOFF[tl] + 65 * half + 65],
                                    et[:, half * 512 + tl * 128:
                                       half * 512 + (tl + 1) * 128],
                                    V[:, i * VSTR + 65 * h:
                                      i * VSTR + 65 * h + 65],
                                    start=(i == 0 and half == 0
                                           and tl in (0, 3)),
                                    stop=(i == 4 * j + tl),
                                    skip_group_check=True,
                                )
                        if i >= 4 * j:
                            finish_tile(hp, j, i - 4 * j, yu, last_group)
                    prev = yu_batch
                    tick()
                prev()
                tick()

            for hp in range(2):
                for j in range(NJ):
                    attn_group(hp, j, hp == 1 and j == NJ - 1)
            while filler:
                filler.pop(0)()
            while pending:
                pending.pop(0)[1]()
            if dbg:
                for n, sb in (("qT", qT), ("kT", kT), ("yT", yT)):
                    nc.sync.dma_start(out=dbg_d[n][:, :], in_=sb[:, :])
                nc.sync.dma_start(out=dbgv_d[:, :], in_=V[:, :])

    nc.compile()
    return nc


def get_program():
    if "nc" not in _CACHE:
        _CACHE["nc"] = _build_program()
    return _CACHE["nc"]


def _pack_cmajor(a):
    """[C_rows, N] -> [128, (C_rows/128)*N] with chunk c at [:, c*N:(c+1)*N]."""
    rows, n = a.shape
    return np.ascontiguousarray(
        a.reshape(rows // 128, 128, n).transpose(1, 0, 2).reshape(128, -1))


def make_in_maps(x, W_attn, b_attn, W_proj):
    """Host-side sharding: per-core input dict."""
    x = np.asarray(x, np.float32)
    W_attn = np.asarray(W_attn, np.float32) * WSC
    b_attn = np.asarray(b_attn, np.float32) * WSC
    W_proj = np.asarray(W_proj, np.float32) / WSC

    tri = (np.arange(128)[None, :] >= np.arange(128)[:, None]).astype(BF16)

    x8_b, xr_b = [], []
    for b in range(B):
        xt = x[b].T.astype(np.float32)                    # [C, T]
        x8 = xt.astype(F8)
        xres = (xt - x8.astype(np.float32)).astype(F8)
        x8_b.append(_pack_cmajor(x8))
        xr_b.append(_pack_cmajor(xres))

    in_maps = []
    for g in range(N_CORES):
        b, hg = divmod(g, 4)
        cs = slice(hg * DL, (hg + 1) * DL)
        m = {"x8p": x8_b[b], "xrp": xr_b[b], "tri": tri}
        for ki, key in enumerate("qkv"):
            Wk = W_attn[:, ki * C:(ki + 1) * C][:, cs]
            W8 = Wk.astype(F8)
            Wr = (Wk - W8.astype(np.float32)).astype(F8)
            m[f"w8{key}"] = _pack_cmajor(W8)
            m[f"wr{key}"] = _pack_cmajor(Wr)
        m["wpp"] = _pack_cmajor(W_proj[cs, :].astype(BF16))
        m["bq"] = np.ascontiguousarray(
            b_attn[0 * C:1 * C][cs].reshape(2, 128).T).astype(np.float32)
        m["bk"] = np.ascontiguousarray(
            b_attn[1 * C:2 * C][cs].reshape(2, 128).T).astype(np.float32)
        m["bvr"] = np.ascontiguousarray(
            np.tile(b_attn[2 * C:3 * C][cs][None, :], (128, 1))).astype(np.float32)
        in_maps.append(m)
    return in_maps


def assemble_output(results, b_proj):
    """results: per-core dicts with 'out' [T, C] bf16 partials."""
    b_proj = np.asarray(b_proj, np.float32)
    out = np.zeros((B, T, C), np.float32)
    for g in range(N_CORES):
        out[g // 4] += np.asarray(results[g]["out"], np.float32)
    out += b_proj[None, None, :]
    return out


def kernel(x, W_attn, b_attn, W_proj, b_proj):
    from concourse.bass_utils import run_bass_kernel_spmd

    nc = get_program()
    in_maps = make_in_maps(x, W_attn, b_attn, W_proj)
    res = run_bass_kernel_spmd(nc, in_maps, list(range(N_CORES)))
    return assemble_output(res.results, b_proj)


# revision 22
# speedup vs baseline: 1.2423x; 1.0633x over previous
"""GPT2 causal attention (B=2, T=2048, C=1024, H=16) on 8 TRN2 NeuronCores.

Sharding: core g = (batch b = g//4, head-group hg = g%4 of 4 heads).
Tensor-parallel over heads x data-parallel over batch. Each core emits a
full [T, C] bf16 partial of the output projection for its 4 heads; host
sums the 4 partials per batch and adds b_proj. No collectives.

Per-core kernel:
  QKV projections run in compensated fp8 (e4m3) DoubleRow matmuls:
  x = x8 + xr and W = W8 + Wr host-split (W pre-scaled by 64), with
  x@W ~= x8@W8 + x8@Wr + xr@W8 (error ~ xr@Wr = O(eps^2), below bf16).
  Each DoubleRow matmul contracts 256 rows (a pair of 128-row k-tiles).

  Scores per head pair, transposed: S^T[tk, tq] = kT^T @ qT in bf16,
  causally trimmed to [d0:512] at 128-col granularity (both the matmuls
  and the exp), exp'd on ACT without max subtraction, diagonal 128-blocks
  masked with a host tri mask on DVE.

  attention*V runs in the natural orientation: per 128-row tq tile,
  yu[tq, (h, 65)] = et_chunk^T @ V_aug accumulates over tk tiles in PSUM;
  V_aug carries a ones-column per head so column 64 of each head group is
  the softmax row-sum, landing on the free dim. Normalization is then
  per-partition: gpsimd normalize_recip (attn library) divides by the sum
  and writes bf16. y is transposed back to [d, t] with XBAR DMA
  transposes for the bf16 output projection.
"""

import numpy as np
import ml_dtypes

BF16 = ml_dtypes.bfloat16
F8 = ml_dtypes.float8_e4m3

B, T, C, H, D = 2, 2048, 1024, 16, 64
HL = 4          # heads per core
DL = HL * D     # 256 local head dims
N_CORES = 8
NT = T // 128   # 16 t tiles
NJ = T // 512   # 4 tq groups
SCALE = 1.0 / np.sqrt(D)
WSC = 64.0      # host pre-scale on W_attn for fp8 range
VSTR = HL * 65  # V tile col stride (4 heads x (64 d + ones col))
YOFF = (0, 130, 260, 512)  # yu subtile col offsets (none crosses a bank)

_CACHE = {}


def _build_program():
    import concourse.tile as tile
    from concourse import bacc
    from concourse import library_config
    import concourse.mybir as mybir

    f32 = mybir.dt.float32
    bf16 = mybir.dt.bfloat16
    fp8 = mybir.dt.float8e4
    Exp = mybir.ActivationFunctionType.Exp
    DR = mybir.MatmulPerfMode.DoubleRow

    nc = bacc.Bacc("TRN2", target_bir_lowering=False, debug=False)

    x8_d = nc.dram_tensor("x8p", [128, 8 * T], fp8, kind="ExternalInput").ap()
    xr_d = nc.dram_tensor("xrp", [128, 8 * T], fp8, kind="ExternalInput").ap()
    w8_d = {k: nc.dram_tensor(f"w8{k}", [128, 8 * DL], fp8,
                              kind="ExternalInput").ap() for k in "qkv"}
    wr_d = {k: nc.dram_tensor(f"wr{k}", [128, 8 * DL], fp8,
                              kind="ExternalInput").ap() for k in "qkv"}
    wp_d = nc.dram_tensor("wpp", [128, 2 * C], bf16, kind="ExternalInput").ap()
    bq_d = nc.dram_tensor("bq", [128, 2], f32, kind="ExternalInput").ap()
    bk_d = nc.dram_tensor("bk", [128, 2], f32, kind="ExternalInput").ap()
    bvr_d = nc.dram_tensor("bvr", [128, DL], f32, kind="ExternalInput").ap()
    tri_d = nc.dram_tensor("tri", [128, 128], bf16, kind="ExternalInput").ap()
    out_d = nc.dram_tensor("out", [T, C], bf16, kind="ExternalOutput").ap()
    import os
    dbg = os.environ.get("K_DEBUG") == "1"
    if dbg:
        dbg_d = {n: nc.dram_tensor(f"dbg_{n}", [128, 2 * T], bf16,
                                   kind="ExternalOutput").ap()
                 for n in ("qT", "kT", "yT")}
        dbgv_d = nc.dram_tensor("dbg_V", [128, NT * VSTR], bf16,
                                kind="ExternalOutput").ap()
        dbgyn_d = nc.dram_tensor("dbg_yn", [128, 2 * T], bf16,
                                 kind="ExternalOutput").ap()
        dbgstg_d = nc.dram_tensor("dbg_stg", [128, 32 * 130], f32,
                                  kind="ExternalOutput").ap()

    with tile.TileContext(nc) as tc:
        with (
            tc.tile_pool(name="const", bufs=1) as cpool,
            tc.tile_pool(name="exp", bufs=6) as epool,
            tc.tile_pool(name="ystg", bufs=4) as spool,
            tc.tile_pool(name="ynat", bufs=4) as npool,
            tc.tile_pool(name="ostage", bufs=4) as opool,
            tc.tile_pool(name="pssc", bufs=2, space="PSUM") as psc,
            tc.tile_pool(name="psqkv", bufs=2, space="PSUM") as pqkv,
            tc.tile_pool(name="psyu", bufs=1, space="PSUM") as pyu,
        ):
            # ---- persistent SBUF ----
            x8 = cpool.tile([128, 8 * T], fp8, tag="x8")     # chunk c at c*T
            xr = cpool.tile([128, 8 * T], fp8, tag="xr")
            w8 = {k: cpool.tile([128, 8 * DL], fp8, tag=f"w8{k}", name=f"w8{k}")
                  for k in "qkv"}
            wr = {k: cpool.tile([128, 8 * DL], fp8, tag=f"wr{k}", name=f"wr{k}")
                  for k in "qkv"}
            wp = cpool.tile([128, 2 * C], bf16, tag="wp")    # d-chunk dc at dc*C
            bq = cpool.tile([128, 2], f32, tag="bq")
            bk = cpool.tile([128, 2], f32, tag="bk")
            bvr = cpool.tile([128, DL], f32, tag="bvr")
            tri = cpool.tile([128, 128], bf16, tag="tri")
            qT = cpool.tile([128, 2 * T], bf16, tag="qT")    # head h: rows 64*(h%2), cols (h//2)*T+t
            kT = cpool.tile([128, 2 * T], bf16, tag="kT")
            yT = cpool.tile([128, 2 * T], bf16, tag="yT")    # d-chunk dc at dc*T
            V = cpool.tile([128, NT * VSTR], bf16, tag="V")  # tile tt, head h at tt*VSTR+65h

            nc.gpsimd.load_library(library_config.attn)

            # ---- input DMAs, ordered for earliest PE start ----
            nc.sync.dma_start(out=w8["q"][:, :], in_=w8_d["q"][:, :])
            nc.sync.dma_start(out=x8[:, 0:2 * T], in_=x8_d[:, 0:2 * T])
            nc.sync.dma_start(out=w8["k"][:, :], in_=w8_d["k"][:, :])
            for c in range(1, 4):  # x8 in DoubleRow 2-chunk pairs
                nc.sync.dma_start(out=x8[:, 2 * c * T:(2 * c + 2) * T],
                                  in_=x8_d[:, 2 * c * T:(2 * c + 2) * T])
            nc.sync.dma_start(out=wr["q"][:, :], in_=wr_d["q"][:, :])
            nc.sync.dma_start(out=wr["k"][:, :], in_=wr_d["k"][:, :])
            nc.sync.dma_start(out=bq[:, :], in_=bq_d[:, :])
            nc.sync.dma_start(out=bk[:, :], in_=bk_d[:, :])
            for c in range(4):
                nc.sync.dma_start(out=xr[:, 2 * c * T:(2 * c + 2) * T],
                                  in_=xr_d[:, 2 * c * T:(2 * c + 2) * T])
            nc.sync.dma_start(out=w8["v"][:, :], in_=w8_d["v"][:, :])
            nc.sync.dma_start(out=bvr[:, :], in_=bvr_d[:, :])
            nc.sync.dma_start(out=wr["v"][:, :], in_=wr_d["v"][:, :])
            nc.sync.dma_start(out=tri[:, :], in_=tri_d[:, :])
            nc.sync.dma_start(out=wp[:, :], in_=wp_d[:, :])

            x8c = x8[:, :].rearrange("p (c t) -> p c t", c=8)
            xrc = xr[:, :].rearrange("p (c t) -> p c t", c=8)
            w8c = {k: w8[k][:, :].rearrange("p (c m) -> p c m", c=8) for k in "qkv"}
            wrc = {k: wr[k][:, :].rearrange("p (c m) -> p c m", c=8) for k in "qkv"}
            TERMS = [(x8c, w8c), (x8c, wrc), (xrc, w8c)]

            def qk_tile(key, dc, ts, pool):
                """One [128d, 512t] q/k projection tile + bias eviction."""
                b_sb, dst = (bq, qT) if key == "q" else (bk, kT)
                ps = pool.tile([128, 512], f32, tag=pool._qkv_tag,
                               name=f"p{key}_{dc}_{ts}")
                msl = slice(dc * 128, (dc + 1) * 128)
                nsl = slice(ts * 512, (ts + 1) * 512)
                for term, (xa, wd) in enumerate(TERMS):
                    for c in range(4):
                        nc.tensor.matmul(
                            ps[:, 0:512],
                            wd[key][:, 2 * c:2 * c + 2, msl],
                            xa[:, 2 * c:2 * c + 2, nsl],
                            start=(term == 0 and c == 0),
                            stop=(term == 2 and c == 3),
                            perf_mode=DR,
                        )
                nc.vector.tensor_scalar_add(
                    dst[:, dc * T + ts * 512: dc * T + (ts + 1) * 512],
                    ps[:, 0:512], b_sb[:, dc:dc + 1])

            def v_tile(tt, pool):
                """One [128t, 256d] V tile (natural layout) + bias eviction."""
                ps = pool.tile([128, 512], f32, tag=pool._qkv_tag,
                               name=f"pv_{tt}")
                tsl = slice(tt * 128, (tt + 1) * 128)
                for term, (xa, wd) in enumerate(TERMS):
                    for c in range(4):
                        nc.tensor.matmul(
                            ps[:, 0:DL],
                            xa[:, 2 * c:2 * c + 2, tsl],
                            wd["v"][:, 2 * c:2 * c + 2, :],
                            start=(term == 0 and c == 0),
                            stop=(term == 2 and c == 3),
                            perf_mode=DR,
                        )
                vdst = V[:, tt * VSTR:(tt + 1) * VSTR].rearrange(
                    "p (h e) -> p h e", h=HL)[:, :, 0:64]
                nc.vector.tensor_add(
                    vdst, ps[:, 0:DL].rearrange("p (h e) -> p h e", h=HL),
                    bvr[:, :].rearrange("p (h e) -> p h e", h=HL))

            psc._qkv_tag = "sc"
            pqkv._qkv_tag = "qkv"

            # ---- startup QKV: only what group (hp0, j0) needs ----
            nc.vector.memset(V[:, :], 1.0)  # ones cols; data overwritten
            qk_tile("q", 0, 0, psc)
            qk_tile("k", 0, 0, psc)
            for tt in range(4):
                v_tile(tt, psc)

            # filler thunks: spread through attention on the pqkv ring, in
            # deadline order — each must be EMITTED before the attention
            # group that consumes it (program order is the dataflow).
            filler = []
            for ts in range(1, 4):
                filler.append(lambda ts=ts: qk_tile("k", 0, ts, pqkv))
                filler.append(lambda ts=ts: qk_tile("q", 0, ts, pqkv))
                filler.append(lambda ts=ts: v_tile(4 * ts, pqkv))
                filler.append(lambda ts=ts: v_tile(4 * ts + 1, pqkv))
                filler.append(lambda ts=ts: v_tile(4 * ts + 2, pqkv))
                filler.append(lambda ts=ts: v_tile(4 * ts + 3, pqkv))
            for ts in range(4):
                filler.append(lambda ts=ts: qk_tile("k", 1, ts, pqkv))
                filler.append(lambda ts=ts: qk_tile("q", 1, ts, pqkv))

            pending = []   # (ready_step, thunk) deferred PE work
            step = [0]
            NSTEPS = 80    # total attention i steps; clamp deferrals

            def tick():
                step[0] += 1
                if filler:
                    filler.pop(0)()
                while pending and pending[0][0] <= step[0]:
                    pending.pop(0)[1]()

            def proj_tile(tt, last):
                """Output projection for one 128-row t tile: [128, 1024]."""
                ot = opool.tile([128, 1024], bf16, tag="ot", name=f"ot_{tt}")
                for cc in range(2):
                    pp = pqkv.tile([128, 512], f32, tag="qkv",
                                   name=f"po_{tt}_{cc}")
                    for dc in range(2):
                        nc.tensor.matmul(
                            pp[:, :],
                            yT[:, dc * T + tt * 128: dc * T + (tt + 1) * 128],
                            wp[:, dc * C + cc * 512: dc * C + (cc + 1) * 512],
                            start=(dc == 0), stop=(dc == 1),
                        )
                    nc.vector.tensor_copy(ot[:, cc * 512:(cc + 1) * 512],
                                          pp[:, :])

                def out_dma(tt=tt, ot=ot):
                    nc.sync.dma_start(
                        out=out_d[tt * 128:(tt + 1) * 128, :], in_=ot[:, :])
                if last:
                    out_dma()
                else:
                    pending.append((min(step[0] + 2, NSTEPS - 1), out_dma))

            def finish_tile(hp, j, tl, yu, last_group):
                """After tile tl's diagonal yu: stage, normalize, transpose,
                and (in the hp1 phase) queue the output projection."""
                tt = 4 * j + tl
                stg = spool.tile([128, 130], f32, tag="stg",
                                 name=f"stg_{hp}_{tt}")
                nc.vector.tensor_copy(stg[:, :],
                                      yu[:, YOFF[tl]:YOFF[tl] + 130])
                if dbg:
                    nc.sync.dma_start(
                        out=dbgstg_d[:, (hp * NT + tt) * 130:
                                     (hp * NT + tt + 1) * 130],
                        in_=stg[:, :])
                yn = npool.tile([128, 128], bf16, tag="yn",
                                name=f"yn_{hp}_{tt}")
                for lh in range(2):
                    nc.gpsimd.normalize_recip(
                        yn[:, lh * 64:(lh + 1) * 64],
                        stg[:, lh * 65: lh * 65 + 64],
                        stg[:, lh * 65 + 64: lh * 65 + 65],
                    )
                if dbg:
                    nc.sync.dma_start(
                        out=dbgyn_d[:, hp * T + tt * 128:
                                    hp * T + (tt + 1) * 128],
                        in_=yn[:, :])

                def transp(hp=hp, tt=tt, yn=yn):
                    nc.sync.dma_start(
                        out=yT[:, hp * T + tt * 128: hp * T + (tt + 1) * 128],
                        in_=yn[:, :], transpose=True)
                tail = last_group and tl == 3
                if tail:
                    transp()
                    proj_tile(tt, True)
                else:
                    pending.append((min(step[0] + 1, NSTEPS - 2), transp))
                    if hp == 1:
                        pending.append((min(step[0] + 4, NSTEPS - 1),
                                        lambda tt=tt: proj_tile(tt, False)))

            yu_tiles = {}

            def make_step(hp, j, i, last_group):
                """Returns (sc_thunk, yu_thunk) for one i step."""
                ni = 4 * j + 4
                fb = hp * T
                d0 = max(128 * (i - 4 * j), 0)
                box = {}

                def sc_emit():
                    if i == 0:
                        yu_tiles[(hp, j)] = pyu.tile(
                            [128, 642], f32, tag="yu", name=f"yu_{hp}_{j}")
                    sc = psc.tile([128, 1024], f32, tag="sc",
                                  name=f"sc_{hp}_{j}_{i}")
                    for half in (0, 1):
                        po = 64 * half
                        nc.tensor.matmul(
                            sc[:, half * 512 + d0:(half + 1) * 512],
                            kT[po:po + 64, fb + i * 128: fb + (i + 1) * 128],
                            qT[po:po + 64,
                               fb + j * 512 + d0: fb + (j + 1) * 512],
                            start=True, stop=True,
                        )
                    et = epool.tile([128, 1024], bf16, tag="exp",
                                    name=f"et_{hp}_{j}_{i}")
                    nc.scalar.activation(
                        et[:, :].rearrange("p (g q) -> p g q", g=2)[:, :, d0:512],
                        sc[:, :].rearrange("p (g q) -> p g q", g=2)[:, :, d0:512],
                        Exp, scale=float(SCALE / (WSC * WSC)),
                    )
                    if i >= 4 * j:
                        for half in (0, 1):
                            sl = slice(half * 512 + d0, half * 512 + d0 + 128)
                            nc.vector.tensor_mul(et[:, sl], et[:, sl],
                                                 tri[:, :])
                    box["et"] = et

                def yu_emit():
                    # start=True clears has_written for the WHOLE PSUM bank,
                    # so only the first matmul into each bank of the yu tile
                    # may carry it; later subtiles first-write on cleared
                    # bits (overwrite) with start=False.
                    yu, et = yu_tiles[(hp, j)], box["et"]
                    for tl in range(4):
                        if 128 * tl < d0:
                            continue
                        for half in (0, 1):
                            h = 2 * hp + half
                            nc.tensor.matmul(
                                yu[:, YOFF[tl] + 65 * half:
                                   YOFF[tl] + 65 * half + 65],
                                et[:, half * 512 + tl * 128:
                                   half * 512 + (tl + 1) * 128],
                                V[:, i * VSTR + 65 * h:
                                  i * VSTR + 65 * h + 65],
                                start=(i == 0 and half == 0 and tl in (0, 3)),
                                stop=(i == 4 * j + tl),
                                skip_group_check=True,
                            )
                    if i >= 4 * j:
                        finish_tile(hp, j, i - 4 * j, yu, last_group)

                return sc_emit, yu_emit

            steps = []
            for hp in range(2):
                for j in range(NJ):
                    for i in range(4 * j + 4):
                        steps.append(make_step(hp, j, i,
                                               hp == 1 and j == NJ - 1))
            # flat software pipeline: sc(i+1) is emitted before yu(i), incl.
            # across group boundaries, so the PE never heads-of-line on exp
            prev = None
            for sc_emit, yu_emit in steps:
                sc_emit()
                if prev is not None:
                    prev()
                prev = yu_emit
                tick()
            prev()
            tick()
            while filler:
                filler.pop(0)()
            while pending:
                pending.pop(0)[1]()
            if dbg:
                for n, sb in (("qT", qT), ("kT", kT), ("yT", yT)):
                    nc.sync.dma_start(out=dbg_d[n][:, :], in_=sb[:, :])
                nc.sync.dma_start(out=dbgv_d[:, :], in_=V[:, :])

    nc.compile()
    return nc


def get_program():
    if "nc" not in _CACHE:
        _CACHE["nc"] = _build_program()
    return _CACHE["nc"]


def _pack_cmajor(a):
    """[C_rows, N] -> [128, (C_rows/128)*N] with chunk c at [:, c*N:(c+1)*N]."""
    rows, n = a.shape
    return np.ascontiguousarray(
        a.reshape(rows // 128, 128, n).transpose(1, 0, 2).reshape(128, -1))


def make_in_maps(x, W_attn, b_attn, W_proj):
    """Host-side sharding: per-core input dict."""
    x = np.asarray(x, np.float32)
    W_attn = np.asarray(W_attn, np.float32) * WSC
    b_attn = np.asarray(b_attn, np.float32) * WSC
    W_proj = np.asarray(W_proj, np.float32) / WSC

    tri = (np.arange(128)[None, :] >= np.arange(128)[:, None]).astype(BF16)

    x8_b, xr_b = [], []
    for b in range(B):
        xt = x[b].T.astype(np.float32)                    # [C, T]
        x8 = xt.astype(F8)
        xres = (xt - x8.astype(np.float32)).astype(F8)
        x8_b.append(_pack_cmajor(x8))
        xr_b.append(_pack_cmajor(xres))

    in_maps = []
    for g in range(N_CORES):
        b, hg = divmod(g, 4)
        cs = slice(hg * DL, (hg + 1) * DL)
        m = {"x8p": x8_b[b], "xrp": xr_b[b], "tri": tri}
        for ki, key in enumerate("qkv"):
            Wk = W_attn[:, ki * C:(ki + 1) * C][:, cs]
            W8 = Wk.astype(F8)
            Wr = (Wk - W8.astype(np.float32)).astype(F8)
            m[f"w8{key}"] = _pack_cmajor(W8)
            m[f"wr{key}"] = _pack_cmajor(Wr)
        m["wpp"] = _pack_cmajor(W_proj[cs, :].astype(BF16))
        m["bq"] = np.ascontiguousarray(
            b_attn[0 * C:1 * C][cs].reshape(2, 128).T).astype(np.float32)
        m["bk"] = np.ascontiguousarray(
            b_attn[1 * C:2 * C][cs].reshape(2, 128).T).astype(np.float32)
        m["bvr"] = np.ascontiguousarray(
            np.tile(b_attn[2 * C:3 * C][cs][None, :], (128, 1))).astype(np.float32)
        in_maps.append(m)
    return in_maps


def assemble_output(results, b_proj):
    """results: per-core dicts with 'out' [T, C] bf16 partials."""
    b_proj = np.asarray(b_proj, np.float32)
    out = np.zeros((B, T, C), np.float32)
    for g in range(N_CORES):
        out[g // 4] += np.asarray(results[g]["out"], np.float32)
    out += b_proj[None, None, :]
    return out


def kernel(x, W_attn, b_attn, W_proj, b_proj):
    from concourse.bass_utils import run_bass_kernel_spmd

    nc = get_program()
    in_maps = make_in_maps(x, W_attn, b_attn, W_proj)
    res = run_bass_kernel_spmd(nc, in_maps, list(range(N_CORES)))
    return assemble_output(res.results, b_proj)


# revision 26
# speedup vs baseline: 1.2730x; 1.0246x over previous
"""GPT2 causal attention (B=2, T=2048, C=1024, H=16) on 8 TRN2 NeuronCores.

Sharding: core g = (batch b = g//4, head-group hg = g%4 of 4 heads).
Tensor-parallel over heads x data-parallel over batch. Each core emits a
full [T, C] bf16 partial of the output projection for its 4 heads; host
sums the 4 partials per batch and adds b_proj. No collectives.

Per-core kernel:
  QKV projections run in compensated fp8 (e4m3) DoubleRow matmuls:
  x = x8 + xr and W = W8 + Wr host-split (W pre-scaled by 64), with
  x@W ~= x8@W8 + x8@Wr + xr@W8 (error ~ xr@Wr = O(eps^2), below bf16).
  Each DoubleRow matmul contracts 256 rows (a pair of 128-row k-tiles).

  Scores per head pair, transposed: S^T[tk, tq] = kT^T @ qT in bf16,
  causally trimmed to [d0:512] at 128-col granularity (both the matmuls
  and the exp), exp'd on ACT without max subtraction, diagonal 128-blocks
  masked with a host tri mask on DVE.

  attention*V runs in the natural orientation: per 128-row tq tile,
  yu[tq, (h, 65)] = et_chunk^T @ V_aug accumulates over tk tiles in PSUM;
  V_aug carries a ones-column per head so column 64 of each head group is
  the softmax row-sum, landing on the free dim. Normalization is then
  per-partition: gpsimd normalize_recip (attn library) divides by the sum
  and writes bf16. y is transposed back to [d, t] with XBAR DMA
  transposes for the bf16 output projection.
"""

import numpy as np
import ml_dtypes

BF16 = ml_dtypes.bfloat16
F8 = ml_dtypes.float8_e4m3

B, T, C, H, D = 2, 2048, 1024, 16, 64
HL = 4          # heads per core
DL = HL * D     # 256 local head dims
N_CORES = 8
NT = T // 128   # 16 t tiles
NJ = T // 512   # 4 tq groups
SCALE = 1.0 / np.sqrt(D)
WSC = 64.0      # host pre-scale on W_attn for fp8 range
VSTR = HL * 65  # V tile col stride (4 heads x (64 d + ones col))
YOFF = (0, 130, 260, 512)  # yu subtile col offsets (none crosses a bank)

_CACHE = {}


def _build_program():
    import concourse.tile as tile
    from concourse import bacc
    from concourse import library_config
    import concourse.mybir as mybir

    f32 = mybir.dt.float32
    bf16 = mybir.dt.bfloat16
    fp8 = mybir.dt.float8e4
    Exp = mybir.ActivationFunctionType.Exp
    DR = mybir.MatmulPerfMode.DoubleRow

    nc = bacc.Bacc("TRN2", target_bir_lowering=False, debug=False)

    x8_d = nc.dram_tensor("x8p", [128, 8 * T], fp8, kind="ExternalInput").ap()
    xr_d = nc.dram_tensor("xrp", [128, 8 * T], fp8, kind="ExternalInput").ap()
    w8_d = {k: nc.dram_tensor(f"w8{k}", [128, 8 * DL], fp8,
                              kind="ExternalInput").ap() for k in "qkv"}
    wr_d = {k: nc.dram_tensor(f"wr{k}", [128, 8 * DL], fp8,
                              kind="ExternalInput").ap() for k in "qkv"}
    wp_d = nc.dram_tensor("wpp", [128, 2 * C], bf16, kind="ExternalInput").ap()
    bq_d = nc.dram_tensor("bq", [128, 2], f32, kind="ExternalInput").ap()
    bk_d = nc.dram_tensor("bk", [128, 2], f32, kind="ExternalInput").ap()
    bvr_d = nc.dram_tensor("bvr", [128, DL], f32, kind="ExternalInput").ap()
    mlo_d = nc.dram_tensor("mlo", [128, 128], bf16, kind="ExternalInput").ap()
    nei_d = nc.dram_tensor("nei", [128, 128], bf16, kind="ExternalInput").ap()
    out_d = nc.dram_tensor("out", [T, C], bf16, kind="ExternalOutput").ap()
    import os
    dbg = os.environ.get("K_DEBUG") == "1"
    if dbg:
        dbg_d = {n: nc.dram_tensor(f"dbg_{n}", [128, 2 * T], bf16,
                                   kind="ExternalOutput").ap()
                 for n in ("qT", "kT", "yT")}
        dbgv_d = nc.dram_tensor("dbg_V", [128, NT * VSTR], bf16,
                                kind="ExternalOutput").ap()
        dbgyn_d = nc.dram_tensor("dbg_yn", [128, 2 * T], bf16,
                                 kind="ExternalOutput").ap()
        dbgstg_d = nc.dram_tensor("dbg_stg", [128, 32 * 130], f32,
                                  kind="ExternalOutput").ap()

    with tile.TileContext(nc) as tc:
        with (
            tc.tile_pool(name="const", bufs=1) as cpool,
            tc.tile_pool(name="exp", bufs=6) as epool,
            tc.tile_pool(name="ystg", bufs=4) as spool,
            tc.tile_pool(name="ynat", bufs=4) as npool,
            tc.tile_pool(name="ostage", bufs=4) as opool,
            tc.tile_pool(name="pssc", bufs=2, space="PSUM") as psc,
            tc.tile_pool(name="psqkv", bufs=2, space="PSUM") as pqkv,
            tc.tile_pool(name="psyu", bufs=1, space="PSUM") as pyu,
        ):
            # ---- persistent SBUF ----
            x8 = cpool.tile([128, 8 * T], fp8, tag="x8")     # chunk c at c*T
            xr = cpool.tile([128, 8 * T], fp8, tag="xr")
            w8 = {k: cpool.tile([128, 8 * DL], fp8, tag=f"w8{k}", name=f"w8{k}")
                  for k in "qkv"}
            wr = {k: cpool.tile([128, 8 * DL], fp8, tag=f"wr{k}", name=f"wr{k}")
                  for k in "qkv"}
            wp = cpool.tile([128, 2 * C], bf16, tag="wp")    # d-chunk dc at dc*C
            bq = cpool.tile([128, 2], f32, tag="bq")
            bk = cpool.tile([128, 2], f32, tag="bk")
            bvr = cpool.tile([128, DL], f32, tag="bvr")
            mlo = cpool.tile([128, 128], bf16, tag="mlo")
            nei = cpool.tile([128, 128], bf16, tag="nei")
            qT = cpool.tile([128, 2 * T], bf16, tag="qT")    # head h: rows 64*(h%2), cols (h//2)*T+t
            kT = cpool.tile([128, 2 * T], bf16, tag="kT")
            yT = cpool.tile([128, 2 * T], bf16, tag="yT")    # d-chunk dc at dc*T
            V = cpool.tile([128, NT * VSTR], bf16, tag="V")  # tile tt, head h at tt*VSTR+65h

            nc.gpsimd.load_library(library_config.attn)

            # ---- input DMAs, ordered for earliest PE start ----
            nc.sync.dma_start(out=bq[:, :], in_=bq_d[:, :])
            nc.sync.dma_start(out=bk[:, :], in_=bk_d[:, :])
            nc.sync.dma_start(out=w8["q"][:, :], in_=w8_d["q"][:, :])
            nc.sync.dma_start(out=x8[:, 0:2 * T], in_=x8_d[:, 0:2 * T])
            nc.sync.dma_start(out=w8["k"][:, :], in_=w8_d["k"][:, :])
            for c in range(1, 4):  # x8 in DoubleRow 2-chunk pairs
                nc.sync.dma_start(out=x8[:, 2 * c * T:(2 * c + 2) * T],
                                  in_=x8_d[:, 2 * c * T:(2 * c + 2) * T])
            for c in range(4):
                nc.sync.dma_start(out=xr[:, 2 * c * T:(2 * c + 2) * T],
                                  in_=xr_d[:, 2 * c * T:(2 * c + 2) * T])
            nc.sync.dma_start(out=wr["q"][:, :], in_=wr_d["q"][:, :])
            nc.sync.dma_start(out=wr["k"][:, :], in_=wr_d["k"][:, :])
            nc.sync.dma_start(out=w8["v"][:, :], in_=w8_d["v"][:, :])
            nc.sync.dma_start(out=bvr[:, :], in_=bvr_d[:, :])
            nc.sync.dma_start(out=wr["v"][:, :], in_=wr_d["v"][:, :])
            nc.sync.dma_start(out=mlo[:, :], in_=mlo_d[:, :])
            nc.sync.dma_start(out=nei[:, :], in_=nei_d[:, :])
            nc.sync.dma_start(out=wp[:, :], in_=wp_d[:, :])

            warm = epool.tile([128, 2], bf16, tag="exp", name="warm")
            nc.scalar.activation(warm[:, :], bq[:, :],
                                 Exp, scale=1e-6)

            x8c = x8[:, :].rearrange("p (c t) -> p c t", c=8)
            xrc = xr[:, :].rearrange("p (c t) -> p c t", c=8)
            w8c = {k: w8[k][:, :].rearrange("p (c m) -> p c m", c=8) for k in "qkv"}
            wrc = {k: wr[k][:, :].rearrange("p (c m) -> p c m", c=8) for k in "qkv"}
            TERMS = [(x8c, w8c), (xrc, w8c), (x8c, wrc)]

            def qk_tile(key, dc, ts, pool):
                """One [128d, 512t] q/k projection tile + bias eviction."""
                b_sb, dst = (bq, qT) if key == "q" else (bk, kT)
                ps = pool.tile([128, 512], f32, tag=pool._qkv_tag,
                               name=f"p{key}_{dc}_{ts}")
                msl = slice(dc * 128, (dc + 1) * 128)
                nsl = slice(ts * 512, (ts + 1) * 512)
                for term, (xa, wd) in enumerate(TERMS):
                    for c in range(4):
                        nc.tensor.matmul(
                            ps[:, 0:512],
                            wd[key][:, 2 * c:2 * c + 2, msl],
                            xa[:, 2 * c:2 * c + 2, nsl],
                            start=(term == 0 and c == 0),
                            stop=(term == 2 and c == 3),
                            perf_mode=DR,
                        )
                nc.vector.tensor_scalar_add(
                    dst[:, dc * T + ts * 512: dc * T + (ts + 1) * 512],
                    ps[:, 0:512], b_sb[:, dc:dc + 1])

            def v_tile(tt, pool):
                """One [128t, 256d] V tile (natural layout) + bias eviction."""
                ps = pool.tile([128, 512], f32, tag=pool._qkv_tag,
                               name=f"pv_{tt}")
                tsl = slice(tt * 128, (tt + 1) * 128)
                for term, (xa, wd) in enumerate(TERMS):
                    for c in range(4):
                        nc.tensor.matmul(
                            ps[:, 0:DL],
                            xa[:, 2 * c:2 * c + 2, tsl],
                            wd["v"][:, 2 * c:2 * c + 2, :],
                            start=(term == 0 and c == 0),
                            stop=(term == 2 and c == 3),
                            perf_mode=DR,
                        )
                vdst = V[:, tt * VSTR:(tt + 1) * VSTR].rearrange(
                    "p (h e) -> p h e", h=HL)[:, :, 0:64]
                nc.vector.tensor_add(
                    vdst, ps[:, 0:DL].rearrange("p (h e) -> p h e", h=HL),
                    bvr[:, :].rearrange("p (h e) -> p h e", h=HL))

            psc._qkv_tag = "sc"
            pqkv._qkv_tag = "qkv"

            # ---- startup QKV: only what group (hp0, j0) needs ----
            nc.vector.memset(V[:, :], 1.0)  # ones cols; data overwritten
            qk_tile("q", 0, 0, psc)
            qk_tile("k", 0, 0, psc)
            v_tile(0, psc)
            v_tile(1, psc)

            # filler thunks: spread through attention on the pqkv ring, in
            # deadline order — each must be EMITTED before the attention
            # group that consumes it (program order is the dataflow).
            # (deadline_step, thunk): emitted no later than deadline, so
            # every attention read sees its producer earlier in program order
            filler = [
                (1, lambda: v_tile(2, pqkv)),
                (2, lambda: v_tile(3, pqkv)),
                (3, lambda: qk_tile("k", 0, 1, pqkv)),
                (3, lambda: qk_tile("q", 0, 1, pqkv)),
                (5, lambda: v_tile(4, pqkv)),
                (6, lambda: v_tile(5, pqkv)),
                (7, lambda: v_tile(6, pqkv)),
                (8, lambda: v_tile(7, pqkv)),
                (10, lambda: qk_tile("k", 0, 2, pqkv)),
                (10, lambda: qk_tile("q", 0, 2, pqkv)),
                (13, lambda: v_tile(8, pqkv)),
                (15, lambda: v_tile(9, pqkv)),
                (17, lambda: v_tile(10, pqkv)),
                (19, lambda: v_tile(11, pqkv)),
                (21, lambda: qk_tile("k", 0, 3, pqkv)),
                (21, lambda: qk_tile("q", 0, 3, pqkv)),
                (24, lambda: v_tile(12, pqkv)),
                (27, lambda: v_tile(13, pqkv)),
                (30, lambda: v_tile(14, pqkv)),
                (33, lambda: v_tile(15, pqkv)),
                (36, lambda: qk_tile("k", 1, 0, pqkv)),
                (37, lambda: qk_tile("q", 1, 0, pqkv)),
                (38, lambda: qk_tile("k", 1, 1, pqkv)),
                (39, lambda: qk_tile("q", 1, 1, pqkv)),
                (40, lambda: qk_tile("k", 1, 2, pqkv)),
                (41, lambda: qk_tile("q", 1, 2, pqkv)),
                (42, lambda: qk_tile("k", 1, 3, pqkv)),
                (43, lambda: qk_tile("q", 1, 3, pqkv)),
            ]

            pending = []   # (ready_step, thunk) deferred PE work
            step = [0]
            NSTEPS = 80    # total attention i steps; clamp deferrals

            def tick():
                step[0] += 1
                while filler and filler[0][0] <= step[0]:
                    filler.pop(0)[1]()
                while pending and pending[0][0] <= step[0]:
                    pending.pop(0)[1]()

            def proj_tile(tt, last):
                """Output projection for one 128-row t tile: [128, 1024]."""
                ot = opool.tile([128, 1024], bf16, tag="ot", name=f"ot_{tt}")
                for cc in range(2):
                    pp = pqkv.tile([128, 512], f32, tag="qkv",
                                   name=f"po_{tt}_{cc}")
                    for dc in range(2):
                        nc.tensor.matmul(
                            pp[:, :],
                            yT[:, dc * T + tt * 128: dc * T + (tt + 1) * 128],
                            wp[:, dc * C + cc * 512: dc * C + (cc + 1) * 512],
                            start=(dc == 0), stop=(dc == 1),
                        )
                    nc.vector.tensor_copy(ot[:, cc * 512:(cc + 1) * 512],
                                          pp[:, :])
                    if last:
                        nc.sync.dma_start(
                            out=out_d[tt * 128:(tt + 1) * 128,
                                      cc * 512:(cc + 1) * 512],
                            in_=ot[:, cc * 512:(cc + 1) * 512])

                def out_dma(tt=tt, ot=ot):
                    nc.sync.dma_start(
                        out=out_d[tt * 128:(tt + 1) * 128, :], in_=ot[:, :])
                if last:
                    pass  # halves DMA'd inline below
                else:
                    pending.append((min(step[0] + 2, NSTEPS - 1), out_dma))

            def finish_tile(hp, j, tl, yu, last_group):
                """After tile tl's diagonal yu: stage, normalize, transpose,
                and (in the hp1 phase) queue the output projection."""
                tt = 4 * j + tl
                stg = spool.tile([128, 130], f32, tag="stg",
                                 name=f"stg_{hp}_{tt}")
                nc.vector.tensor_copy(stg[:, :],
                                      yu[:, YOFF[tl]:YOFF[tl] + 130])
                if dbg:
                    nc.sync.dma_start(
                        out=dbgstg_d[:, (hp * NT + tt) * 130:
                                     (hp * NT + tt + 1) * 130],
                        in_=stg[:, :])
                yn = npool.tile([128, 128], bf16, tag="yn",
                                name=f"yn_{hp}_{tt}")
                for lh in range(2):
                    nc.gpsimd.normalize_recip(
                        yn[:, lh * 64:(lh + 1) * 64],
                        stg[:, lh * 65: lh * 65 + 64],
                        stg[:, lh * 65 + 64: lh * 65 + 65],
                    )
                if dbg:
                    nc.sync.dma_start(
                        out=dbgyn_d[:, hp * T + tt * 128:
                                    hp * T + (tt + 1) * 128],
                        in_=yn[:, :])

                def transp(hp=hp, tt=tt, yn=yn):
                    nc.sync.dma_start(
                        out=yT[:, hp * T + tt * 128: hp * T + (tt + 1) * 128],
                        in_=yn[:, :], transpose=True)
                tail = last_group and tl == 3
                if tail:
                    transp()
                    proj_tile(tt, True)
                else:
                    dt_, dp_ = (1, 2) if last_group else (1, 4)
                    pending.append((min(step[0] + dt_, NSTEPS - 2), transp))
                    if hp == 1:
                        pending.append((min(step[0] + dp_, NSTEPS - 1),
                                        lambda tt=tt: proj_tile(tt, False)))

            yu_tiles = {}

            def make_step(hp, j, i, last_group):
                """Returns (sc_thunk, yu_thunk) for one i step."""
                ni = 4 * j + 4
                fb = hp * T
                d0 = max(128 * (i - 4 * j), 0)
                box = {}

                def sc_emit():
                    if i == 0:
                        yu_tiles[(hp, j)] = pyu.tile(
                            [128, 642], f32, tag="yu", name=f"yu_{hp}_{j}")
                    sc = psc.tile([128, 1024], f32, tag="sc",
                                  name=f"sc_{hp}_{j}_{i}")
                    for half in (0, 1):
                        po = 64 * half
                        nc.tensor.matmul(
                            sc[:, half * 512 + d0:(half + 1) * 512],
                            kT[po:po + 64, fb + i * 128: fb + (i + 1) * 128],
                            qT[po:po + 64,
                               fb + j * 512 + d0: fb + (j + 1) * 512],
                            start=True, stop=True,
                        )
                    if i >= 4 * j:
                        # mask the diagonal 128-block by accumulating
                        # -BIG*[tq<tk] into the scores (exp then gives 0)
                        for half in (0, 1):
                            nc.tensor.matmul(
                                sc[:, half * 512 + d0:half * 512 + d0 + 128],
                                mlo[:, :], nei[:, :],
                                start=False, stop=True,
                                skip_group_check=True,
                            )
                    et = epool.tile([128, 1024], bf16, tag="exp",
                                    name=f"et_{hp}_{j}_{i}")
                    nc.scalar.activation(
                        et[:, :].rearrange("p (g q) -> p g q", g=2)[:, :, d0:512],
                        sc[:, :].rearrange("p (g q) -> p g q", g=2)[:, :, d0:512],
                        Exp, scale=float(SCALE / (WSC * WSC)),
                    )
                    box["et"] = et

                def yu_emit():
                    # start=True clears has_written for the WHOLE PSUM bank,
                    # so only the first matmul into each bank of the yu tile
                    # may carry it; later subtiles first-write on cleared
                    # bits (overwrite) with start=False.
                    yu, et = yu_tiles[(hp, j)], box["et"]
                    for tl in range(4):
                        if 128 * tl < d0:
                            continue
                        for half in (0, 1):
                            h = 2 * hp + half
                            nc.tensor.matmul(
                                yu[:, YOFF[tl] + 65 * half:
                                   YOFF[tl] + 65 * half + 65],
                                et[:, half * 512 + tl * 128:
                                   half * 512 + (tl + 1) * 128],
                                V[:, i * VSTR + 65 * h:
                                  i * VSTR + 65 * h + 65],
                                start=(i == 0 and half == 0 and tl in (0, 3)),
                                stop=(i == 4 * j + tl),
                                skip_group_check=True,
                            )
                    if i >= 4 * j:
                        finish_tile(hp, j, i - 4 * j, yu, last_group)

                return sc_emit, yu_emit

            steps = []
            for hp in range(2):
                for j in range(NJ):
                    for i in range(4 * j + 4):
                        steps.append(make_step(hp, j, i,
                                               hp == 1 and j == NJ - 1))
            # flat software pipeline: sc(i+1) is emitted before yu(i), incl.
            # across group boundaries, so the PE never heads-of-line on exp
            prev = None
            for sc_emit, yu_emit in steps:
                sc_emit()
                if prev is not None:
                    prev()
                prev = yu_emit
                tick()
            prev()
            tick()
            while filler:
                filler.pop(0)[1]()
            while pending:
                pending.pop(0)[1]()
            if dbg:
                for n, sb in (("qT", qT), ("kT", kT), ("yT", yT)):
                    nc.sync.dma_start(out=dbg_d[n][:, :], in_=sb[:, :])
                nc.sync.dma_start(out=dbgv_d[:, :], in_=V[:, :])

    nc.compile()
    return nc


def get_program():
    if "nc" not in _CACHE:
        _CACHE["nc"] = _build_program()
    return _CACHE["nc"]


def _pack_cmajor(a):
    """[C_rows, N] -> [128, (C_rows/128)*N] with chunk c at [:, c*N:(c+1)*N]."""
    rows, n = a.shape
    return np.ascontiguousarray(
        a.reshape(rows // 128, 128, n).transpose(1, 0, 2).reshape(128, -1))


def make_in_maps(x, W_attn, b_attn, W_proj):
    """Host-side sharding: per-core input dict."""
    x = np.asarray(x, np.float32)
    W_attn = np.asarray(W_attn, np.float32) * WSC
    b_attn = np.asarray(b_attn, np.float32) * WSC
    W_proj = np.asarray(W_proj, np.float32) / WSC

    mlo = (np.arange(128)[:, None] < np.arange(128)[None, :]).astype(BF16)
    nei = (-3.0e6 * np.eye(128)).astype(BF16)

    x8_b, xr_b = [], []
    for b in range(B):
        xt = x[b].T.astype(np.float32)                    # [C, T]
        x8 = xt.astype(F8)
        xres = (xt - x8.astype(np.float32)).astype(F8)
        x8_b.append(_pack_cmajor(x8))
        xr_b.append(_pack_cmajor(xres))

    in_maps = []
    for g in range(N_CORES):
        b, hg = divmod(g, 4)
        cs = slice(hg * DL, (hg + 1) * DL)
        m = {"x8p": x8_b[b], "xrp": xr_b[b], "mlo": mlo, "nei": nei}
        for ki, key in enumerate("qkv"):
            Wk = W_attn[:, ki * C:(ki + 1) * C][:, cs]
            W8 = Wk.astype(F8)
            Wr = (Wk - W8.astype(np.float32)).astype(F8)
            m[f"w8{key}"] = _pack_cmajor(W8)
            m[f"wr{key}"] = _pack_cmajor(Wr)
        m["wpp"] = _pack_cmajor(W_proj[cs, :].astype(BF16))
        m["bq"] = np.ascontiguousarray(
            b_attn[0 * C:1 * C][cs].reshape(2, 128).T).astype(np.float32)
        m["bk"] = np.ascontiguousarray(
            b_attn[1 * C:2 * C][cs].reshape(2, 128).T).astype(np.float32)
        m["bvr"] = np.ascontiguousarray(
            np.tile(b_attn[2 * C:3 * C][cs][None, :], (128, 1))).astype(np.float32)
        in_maps.append(m)
    return in_maps


def assemble_output(results, b_proj):
    """results: per-core dicts with 'out' [T, C] bf16 partials."""
    b_proj = np.asarray(b_proj, np.float32)
    out = np.zeros((B, T, C), np.float32)
    for g in range(N_CORES):
        out[g // 4] += np.asarray(results[g]["out"], np.float32)
    out += b_proj[None, None, :]
    return out


def kernel(x, W_attn, b_attn, W_proj, b_proj):
    from concourse.bass_utils import run_bass_kernel_spmd

    nc = get_program()
    in_maps = make_in_maps(x, W_attn, b_attn, W_proj)
    res = run_bass_kernel_spmd(nc, in_maps, list(range(N_CORES)))
    return assemble_output(res.results, b_proj)


# revision 40
# speedup vs baseline: 1.2940x; 1.0165x over previous
"""GPT2 causal attention (B=2, T=2048, C=1024, H=16) on 8 TRN2 NeuronCores.

Sharding: core g = (batch b = g//4, head-group hg = g%4 of 4 heads).
Tensor-parallel over heads x data-parallel over batch. Each core emits a
full [T, C] bf16 partial of the output projection for its 4 heads; host
sums the 4 partials per batch and adds b_proj. No collectives.

Per-core kernel:
  QKV projections run in compensated fp8 (e4m3) DoubleRow matmuls:
  x = x8 + xr and W = W8 + Wr host-split (W pre-scaled by 64), with
  x@W ~= x8@W8 + x8@Wr + xr@W8 (error ~ xr@Wr = O(eps^2), below bf16).
  Each DoubleRow matmul contracts 256 rows (a pair of 128-row k-tiles).

  Scores per head pair, transposed: S^T[tk, tq] = kT^T @ qT in bf16,
  causally trimmed to [d0:512] at 128-col granularity (both the matmuls
  and the exp), exp'd on ACT without max subtraction, diagonal 128-blocks
  masked with a host tri mask on DVE.

  attention*V runs in the natural orientation: per 128-row tq tile,
  yu[tq, (h, 65)] = et_chunk^T @ V_aug accumulates over tk tiles in PSUM;
  V_aug carries a ones-column per head so column 64 of each head group is
  the softmax row-sum, landing on the free dim. Normalization is then
  per-partition: gpsimd normalize_recip (attn library) divides by the sum
  and writes bf16. y is transposed back to [d, t] with XBAR DMA
  transposes for the bf16 output projection.
"""

import numpy as np
import ml_dtypes

BF16 = ml_dtypes.bfloat16
F8 = ml_dtypes.float8_e4m3

B, T, C, H, D = 2, 2048, 1024, 16, 64
HL = 4          # heads per core
DL = HL * D     # 256 local head dims
N_CORES = 8
NT = T // 128   # 16 t tiles
NJ = T // 512   # 4 tq groups
SCALE = 1.0 / np.sqrt(D)
WSC = 64.0      # host pre-scale on W_attn for fp8 range
VSTR = HL * 65  # V tile col stride (4 heads x (64 d + ones col))
YOFF = (0, 130, 260, 512)  # yu subtile col offsets (none crosses a bank)

_CACHE = {}


def _build_program():
    import concourse.tile as tile
    from concourse import bacc
    from concourse import library_config
    import concourse.mybir as mybir

    f32 = mybir.dt.float32
    bf16 = mybir.dt.bfloat16
    fp8 = mybir.dt.float8e4
    Exp = mybir.ActivationFunctionType.Exp
    DR = mybir.MatmulPerfMode.DoubleRow

    nc = bacc.Bacc("TRN2", target_bir_lowering=False, debug=False)

    x8_d = nc.dram_tensor("x8p", [128, 8 * T], fp8, kind="ExternalInput").ap()
    xr_d = nc.dram_tensor("xrp", [128, 8 * T], fp8, kind="ExternalInput").ap()
    w8_d = {k: nc.dram_tensor(f"w8{k}", [128, 8 * DL], fp8,
                              kind="ExternalInput").ap() for k in "qkv"}
    wr_d = {k: nc.dram_tensor(f"wr{k}", [128, 8 * DL], fp8,
                              kind="ExternalInput").ap() for k in "qkv"}
    wp_d = nc.dram_tensor("wpp", [128, 2 * C], bf16, kind="ExternalInput").ap()
    bq_d = nc.dram_tensor("bq", [128, 2], f32, kind="ExternalInput").ap()
    bk_d = nc.dram_tensor("bk", [128, 2], f32, kind="ExternalInput").ap()
    bvr_d = nc.dram_tensor("bvr", [128, DL], f32, kind="ExternalInput").ap()
    mlo_d = nc.dram_tensor("mlo", [128, 128], bf16, kind="ExternalInput").ap()
    nei_d = nc.dram_tensor("nei", [128, 128], bf16, kind="ExternalInput").ap()
    eye_d = nc.dram_tensor("eye", [128, 128], bf16, kind="ExternalInput").ap()
    out_d = nc.dram_tensor("out", [T, C], bf16, kind="ExternalOutput").ap()
    import os
    dbg = os.environ.get("K_DEBUG") == "1"
    if dbg:
        dbg_d = {n: nc.dram_tensor(f"dbg_{n}", [128, 2 * T], bf16,
                                   kind="ExternalOutput").ap()
                 for n in ("qT", "kT", "yT")}
        dbgv_d = nc.dram_tensor("dbg_V", [128, NT * VSTR], bf16,
                                kind="ExternalOutput").ap()
        dbgyn_d = nc.dram_tensor("dbg_yn", [128, 2 * T], bf16,
                                 kind="ExternalOutput").ap()
        dbgstg_d = nc.dram_tensor("dbg_stg", [128, 32 * 130], f32,
                                  kind="ExternalOutput").ap()

    with tile.TileContext(nc) as tc:
        with (
            tc.tile_pool(name="const", bufs=1) as cpool,
            tc.tile_pool(name="exp", bufs=6) as epool,
            tc.tile_pool(name="ystg", bufs=4) as spool,
            tc.tile_pool(name="ynat", bufs=4) as npool,
            tc.tile_pool(name="ostage", bufs=4) as opool,
            tc.tile_pool(name="pssc", bufs=2, space="PSUM") as psc,
            tc.tile_pool(name="psqkv", bufs=2, space="PSUM") as pqkv,
            tc.tile_pool(name="psyu", bufs=1, space="PSUM") as pyu,
        ):
            # ---- persistent SBUF ----
            x8 = cpool.tile([128, 8 * T], fp8, tag="x8")     # chunk c at c*T
            xr = cpool.tile([128, 8 * T], fp8, tag="xr")
            w8 = {k: cpool.tile([128, 8 * DL], fp8, tag=f"w8{k}", name=f"w8{k}")
                  for k in "qkv"}
            wr = {k: cpool.tile([128, 8 * DL], fp8, tag=f"wr{k}", name=f"wr{k}")
                  for k in "qkv"}
            wp = cpool.tile([128, 2 * C], bf16, tag="wp")    # d-chunk dc at dc*C
            bq = cpool.tile([128, 2], f32, tag="bq")
            bk = cpool.tile([128, 2], f32, tag="bk")
            bvr = cpool.tile([128, DL], f32, tag="bvr")
            mlo = cpool.tile([128, 128], bf16, tag="mlo")
            nei = cpool.tile([128, 128], bf16, tag="nei")
            eye = cpool.tile([128, 128], bf16, tag="eye")
            qT = cpool.tile([128, 2 * T], bf16, tag="qT")    # head h: rows 64*(h%2), cols (h//2)*T+t
            kT = cpool.tile([128, 2 * T], bf16, tag="kT")
            yT = cpool.tile([128, 2 * T], bf16, tag="yT")    # d-chunk dc at dc*T
            V = cpool.tile([128, NT * VSTR], bf16, tag="V")  # tile tt, head h at tt*VSTR+65h

            nc.gpsimd.load_library(library_config.attn)

            # ---- input DMAs, ordered for earliest PE start ----
            nc.sync.dma_start(out=bq[:, :], in_=bq_d[:, :])
            nc.sync.dma_start(out=bk[:, :], in_=bk_d[:, :])
            nc.sync.dma_start(out=w8["q"][:, :], in_=w8_d["q"][:, :])
            nc.sync.dma_start(out=x8[:, 0:2 * T], in_=x8_d[:, 0:2 * T])
            nc.sync.dma_start(out=w8["k"][:, :], in_=w8_d["k"][:, :])
            for c in range(1, 4):  # x8 in DoubleRow 2-chunk pairs
                nc.sync.dma_start(out=x8[:, 2 * c * T:(2 * c + 2) * T],
                                  in_=x8_d[:, 2 * c * T:(2 * c + 2) * T])
            for c in range(4):
                nc.sync.dma_start(out=xr[:, 2 * c * T:(2 * c + 2) * T],
                                  in_=xr_d[:, 2 * c * T:(2 * c + 2) * T])
            nc.sync.dma_start(out=wr["q"][:, :], in_=wr_d["q"][:, :])
            nc.sync.dma_start(out=wr["k"][:, :], in_=wr_d["k"][:, :])
            nc.sync.dma_start(out=w8["v"][:, :], in_=w8_d["v"][:, :])
            nc.sync.dma_start(out=bvr[:, :], in_=bvr_d[:, :])
            nc.sync.dma_start(out=wr["v"][:, :], in_=wr_d["v"][:, :])
            nc.sync.dma_start(out=mlo[:, :], in_=mlo_d[:, :])
            nc.sync.dma_start(out=nei[:, :], in_=nei_d[:, :])
            nc.sync.dma_start(out=eye[:, :], in_=eye_d[:, :])
            nc.sync.dma_start(out=wp[:, :], in_=wp_d[:, :])

            warm = epool.tile([128, 2], bf16, tag="exp", name="warm")
            nc.scalar.activation(warm[:, :], bq[:, :],
                                 Exp, scale=1e-6)

            x8c = x8[:, :].rearrange("p (c t) -> p c t", c=8)
            xrc = xr[:, :].rearrange("p (c t) -> p c t", c=8)
            w8c = {k: w8[k][:, :].rearrange("p (c m) -> p c m", c=8) for k in "qkv"}
            wrc = {k: wr[k][:, :].rearrange("p (c m) -> p c m", c=8) for k in "qkv"}
            TERMS = [(x8c, w8c), (xrc, w8c), (x8c, wrc)]

            def qk_tile(key, dc, ts, pool):
                """One [128d, 512t] q/k projection tile + bias eviction."""
                b_sb, dst = (bq, qT) if key == "q" else (bk, kT)
                ps = pool.tile([128, 512], f32, tag=pool._qkv_tag,
                               name=f"p{key}_{dc}_{ts}")
                msl = slice(dc * 128, (dc + 1) * 128)
                nsl = slice(ts * 512, (ts + 1) * 512)
                for term, (xa, wd) in enumerate(TERMS):
                    for c in range(4):
                        nc.tensor.matmul(
                            ps[:, 0:512],
                            wd[key][:, 2 * c:2 * c + 2, msl],
                            xa[:, 2 * c:2 * c + 2, nsl],
                            start=(term == 0 and c == 0),
                            stop=(term == 2 and c == 3),
                            perf_mode=DR,
                        )
                nc.vector.tensor_scalar_add(
                    dst[:, dc * T + ts * 512: dc * T + (ts + 1) * 512],
                    ps[:, 0:512], b_sb[:, dc:dc + 1])

            def v_tile(tt, pool):
                """One [128t, 256d] V tile (natural layout) + bias eviction."""
                ps = pool.tile([128, 512], f32, tag=pool._qkv_tag,
                               name=f"pv_{tt}")
                tsl = slice(tt * 128, (tt + 1) * 128)
                for term, (xa, wd) in enumerate(TERMS):
                    for c in range(4):
                        nc.tensor.matmul(
                            ps[:, 0:DL],
                            xa[:, 2 * c:2 * c + 2, tsl],
                            wd["v"][:, 2 * c:2 * c + 2, :],
                            start=(term == 0 and c == 0),
                            stop=(term == 2 and c == 3),
                            perf_mode=DR,
                        )
                vdst = V[:, tt * VSTR:(tt + 1) * VSTR].rearrange(
                    "p (h e) -> p h e", h=HL)[:, :, 0:64]
                nc.vector.tensor_add(
                    vdst, ps[:, 0:DL].rearrange("p (h e) -> p h e", h=HL),
                    bvr[:, :].rearrange("p (h e) -> p h e", h=HL))

            psc._qkv_tag = "sc"
            pqkv._qkv_tag = "qkv"

            # ---- startup QKV: only what group (hp0, j0) needs ----
            nc.vector.memset(V[:, :], 1.0)  # ones cols; data overwritten
            # q0/k0 startup tiles, chunk-major interleaved so neither
            # blocks the other's ready matmuls in the PE FIFO
            ps_q = psc.tile([128, 512], f32, tag="sc", name="pq_0_0")
            ps_k = psc.tile([128, 512], f32, tag="sc", name="pk_0_0")
            for term, (xa, wd) in enumerate(TERMS):
                for c in range(4):
                    for key, ps in (("q", ps_q), ("k", ps_k)):
                        nc.tensor.matmul(
                            ps[:, 0:512],
                            wd[key][:, 2 * c:2 * c + 2, 0:128],
                            xa[:, 2 * c:2 * c + 2, 0:512],
                            start=(term == 0 and c == 0),
                            stop=(term == 2 and c == 3),
                            perf_mode=DR,
                        )
            nc.vector.tensor_scalar_add(qT[:, 0:512], ps_q[:, 0:512],
                                        bq[:, 0:1])
            nc.vector.tensor_scalar_add(kT[:, 0:512], ps_k[:, 0:512],
                                        bk[:, 0:1])
            v_tile(0, psc)
            v_tile(1, psc)

            # filler thunks: spread through attention on the pqkv ring, in
            # deadline order — each must be EMITTED before the attention
            # group that consumes it (program order is the dataflow).
            # (deadline_step, thunk): emitted no later than deadline, so
            # every attention read sees its producer earlier in program order
            filler = [
                (1, lambda: v_tile(2, pqkv)),
                (2, lambda: v_tile(3, pqkv)),
                (3, lambda: qk_tile("k", 0, 1, pqkv)),
                (3, lambda: qk_tile("q", 0, 1, pqkv)),
                (5, lambda: v_tile(4, pqkv)),
                (6, lambda: v_tile(5, pqkv)),
                (7, lambda: v_tile(6, pqkv)),
                (8, lambda: v_tile(7, pqkv)),
                (10, lambda: qk_tile("k", 0, 2, pqkv)),
                (10, lambda: qk_tile("q", 0, 2, pqkv)),
                (13, lambda: v_tile(8, pqkv)),
                (15, lambda: v_tile(9, pqkv)),
                (17, lambda: v_tile(10, pqkv)),
                (19, lambda: v_tile(11, pqkv)),
                (21, lambda: qk_tile("k", 0, 3, pqkv)),
                (21, lambda: qk_tile("q", 0, 3, pqkv)),
                (24, lambda: v_tile(12, pqkv)),
                (27, lambda: v_tile(13, pqkv)),
                (30, lambda: v_tile(14, pqkv)),
                (33, lambda: v_tile(15, pqkv)),
                (36, lambda: qk_tile("k", 1, 0, pqkv)),
                (37, lambda: qk_tile("q", 1, 0, pqkv)),
                (38, lambda: qk_tile("k", 1, 1, pqkv)),
                (39, lambda: qk_tile("q", 1, 1, pqkv)),
                (40, lambda: qk_tile("k", 1, 2, pqkv)),
                (41, lambda: qk_tile("q", 1, 2, pqkv)),
                (42, lambda: qk_tile("k", 1, 3, pqkv)),
                (43, lambda: qk_tile("q", 1, 3, pqkv)),
            ]

            pending = []   # (ready_step, thunk) deferred PE work
            step = [0]
            NSTEPS = 80    # total attention i steps; clamp deferrals

            def tick():
                step[0] += 1
                while filler and filler[0][0] <= step[0]:
                    filler.pop(0)[1]()
                while pending and pending[0][0] <= step[0]:
                    pending.pop(0)[1]()

            def proj_tile(tt, last):
                """Output projection for one 128-row t tile: [128, 1024]."""
                ot = opool.tile([128, 1024], bf16, tag="ot", name=f"ot_{tt}")
                for cc in range(2):
                    pp = pqkv.tile([128, 512], f32, tag="qkv",
                                   name=f"po_{tt}_{cc}")
                    for dc in range(2):
                        nc.tensor.matmul(
                            pp[:, :],
                            yT[:, dc * T + tt * 128: dc * T + (tt + 1) * 128],
                            wp[:, dc * C + cc * 512: dc * C + (cc + 1) * 512],
                            start=(dc == 0), stop=(dc == 1),
                        )
                    nc.vector.tensor_copy(ot[:, cc * 512:(cc + 1) * 512],
                                          pp[:, :])
                    if last:
                        nc.sync.dma_start(
                            out=out_d[tt * 128:(tt + 1) * 128,
                                      cc * 512:(cc + 1) * 512],
                            in_=ot[:, cc * 512:(cc + 1) * 512])

                def out_dma(tt=tt, ot=ot):
                    nc.sync.dma_start(
                        out=out_d[tt * 128:(tt + 1) * 128, :], in_=ot[:, :])
                if last:
                    pass  # halves DMA'd inline below
                else:
                    pending.append((min(step[0] + 2, NSTEPS - 1), out_dma))

            def finish_tile(hp, j, tl, yu, last_group):
                """After tile tl's diagonal yu: stage, normalize, transpose,
                and (in the hp1 phase) queue the output projection."""
                tt = 4 * j + tl
                stg = spool.tile([128, 130], f32, tag="stg",
                                 name=f"stg_{hp}_{tt}")
                nc.vector.tensor_copy(stg[:, :],
                                      yu[:, YOFF[tl]:YOFF[tl] + 130])
                if dbg:
                    nc.sync.dma_start(
                        out=dbgstg_d[:, (hp * NT + tt) * 130:
                                     (hp * NT + tt + 1) * 130],
                        in_=stg[:, :])
                yn = npool.tile([128, 128], bf16, tag="yn",
                                name=f"yn_{hp}_{tt}")
                for lh in range(2):
                    nc.gpsimd.normalize_recip(
                        yn[:, lh * 64:(lh + 1) * 64],
                        stg[:, lh * 65: lh * 65 + 64],
                        stg[:, lh * 65 + 64: lh * 65 + 65],
                    )
                if dbg:
                    nc.sync.dma_start(
                        out=dbgyn_d[:, hp * T + tt * 128:
                                    hp * T + (tt + 1) * 128],
                        in_=yn[:, :])

                def transp(hp=hp, tt=tt, yn=yn):
                    nc.sync.dma_start(
                        out=yT[:, hp * T + tt * 128: hp * T + (tt + 1) * 128],
                        in_=yn[:, :], transpose=True)
                tail = last_group and tl == 3
                if tail:
                    # PE transpose through a spare score-ring bank: the
                    # score stream is over, and it is ~1.5us faster than
                    # the XBAR DMA path on the critical tail
                    pt = psc.tile([128, 128], bf16, tag="sc", name="pt_tail")
                    nc.tensor.transpose(pt[:, :], yn[:, :], eye[:, :])
                    nc.vector.tensor_copy(
                        yT[:, hp * T + tt * 128: hp * T + (tt + 1) * 128],
                        pt[:, :])
                    proj_tile(tt, True)
                else:
                    dt_, dp_ = (1, 2) if last_group else (1, 4)
                    pending.append((min(step[0] + dt_, NSTEPS - 2), transp))
                    if hp == 1:
                        pending.append((min(step[0] + dp_, NSTEPS - 1),
                                        lambda tt=tt: proj_tile(tt, False)))

            yu_tiles = {}

            def make_step(hp, j, i, last_group):
                """Returns (sc_thunk, yu_thunk) for one i step."""
                ni = 4 * j + 4
                fb = hp * T
                d0 = max(128 * (i - 4 * j), 0)
                box = {}

                def sc_emit():
                    if i == 0:
                        yu_tiles[(hp, j)] = pyu.tile(
                            [128, 642], f32, tag="yu", name=f"yu_{hp}_{j}")
                    sc = psc.tile([128, 1024], f32, tag="sc",
                                  name=f"sc_{hp}_{j}_{i}")
                    for half in (0, 1):
                        po = 64 * half
                        nc.tensor.matmul(
                            sc[:, half * 512 + d0:(half + 1) * 512],
                            kT[po:po + 64, fb + i * 128: fb + (i + 1) * 128],
                            qT[po:po + 64,
                               fb + j * 512 + d0: fb + (j + 1) * 512],
                            start=True, stop=True,
                        )
                    et = epool.tile([128, 1024], bf16, tag="exp",
                                    name=f"et_{hp}_{j}_{i}")
                    nc.scalar.activation(
                        et[:, :].rearrange("p (g q) -> p g q", g=2)[:, :, d0:512],
                        sc[:, :].rearrange("p (g q) -> p g q", g=2)[:, :, d0:512],
                        Exp, scale=float(SCALE / (WSC * WSC)),
                    )
                    if i >= 4 * j:
                        for half in (0, 1):
                            sl = slice(half * 512 + d0, half * 512 + d0 + 128)
                            nc.vector.tensor_mul(et[:, sl], et[:, sl],
                                                 mlo[:, :])
                    box["et"] = et

                def yu_emit():
                    # start=True clears has_written for the WHOLE PSUM bank,
                    # so only the first matmul into each bank of the yu tile
                    # may carry it; later subtiles first-write on cleared
                    # bits (overwrite) with start=False.
                    yu, et = yu_tiles[(hp, j)], box["et"]
                    for tl in range(4):
                        if 128 * tl < d0:
                            continue
                        for half in (0, 1):
                            h = 2 * hp + half
                            nc.tensor.matmul(
                                yu[:, YOFF[tl] + 65 * half:
                                   YOFF[tl] + 65 * half + 65],
                                et[:, half * 512 + tl * 128:
                                   half * 512 + (tl + 1) * 128],
                                V[:, i * VSTR + 65 * h:
                                  i * VSTR + 65 * h + 65],
                                start=(i == 0 and half == 0 and tl in (0, 3)),
                                stop=(i == 4 * j + tl),
                                skip_group_check=True,
                            )
                    if i >= 4 * j:
                        finish_tile(hp, j, i - 4 * j, yu, last_group)

                return sc_emit, yu_emit

            steps = []
            for hp in range(2):
                for j in range(NJ):
                    for i in range(4 * j + 4):
                        steps.append(make_step(hp, j, i,
                                               hp == 1 and j == NJ - 1))
            # flat software pipeline: sc(i+1) is emitted before yu(i), incl.
            # across group boundaries, so the PE never heads-of-line on exp
            from collections import deque
            inflight = deque()
            for sc_emit, yu_emit in steps:
                sc_emit()
                if len(inflight) == 2:
                    inflight.popleft()()
                inflight.append(yu_emit)
                tick()
            while inflight:
                inflight.popleft()()
                tick()
            while filler:
                filler.pop(0)[1]()
            while pending:
                pending.pop(0)[1]()
            if dbg:
                for n, sb in (("qT", qT), ("kT", kT), ("yT", yT)):
                    nc.sync.dma_start(out=dbg_d[n][:, :], in_=sb[:, :])
                nc.sync.dma_start(out=dbgv_d[:, :], in_=V[:, :])

    nc.compile()
    return nc


def get_program():
    if "nc" not in _CACHE:
        _CACHE["nc"] = _build_program()
    return _CACHE["nc"]


def _pack_cmajor(a):
    """[C_rows, N] -> [128, (C_rows/128)*N] with chunk c at [:, c*N:(c+1)*N]."""
    rows, n = a.shape
    return np.ascontiguousarray(
        a.reshape(rows // 128, 128, n).transpose(1, 0, 2).reshape(128, -1))


def make_in_maps(x, W_attn, b_attn, W_proj):
    """Host-side sharding: per-core input dict."""
    x = np.asarray(x, np.float32)
    W_attn = np.asarray(W_attn, np.float32) * WSC
    b_attn = np.asarray(b_attn, np.float32) * WSC
    W_proj = np.asarray(W_proj, np.float32) / WSC

    mlo = (np.arange(128)[None, :] >= np.arange(128)[:, None]).astype(BF16)
    nei = (-3.0e6 * np.eye(128)).astype(BF16)

    x8_b, xr_b = [], []
    for b in range(B):
        xt = x[b].T.astype(np.float32)                    # [C, T]
        x8 = xt.astype(F8)
        xres = (xt - x8.astype(np.float32)).astype(F8)
        x8_b.append(_pack_cmajor(x8))
        xr_b.append(_pack_cmajor(xres))

    in_maps = []
    for g in range(N_CORES):
        b, hg = divmod(g, 4)
        cs = slice(hg * DL, (hg + 1) * DL)
        m = {"x8p": x8_b[b], "xrp": xr_b[b], "mlo": mlo, "nei": nei,
             "eye": np.eye(128).astype(BF16)}
        for ki, key in enumerate("qkv"):
            Wk = W_attn[:, ki * C:(ki + 1) * C][:, cs]
            W8 = Wk.astype(F8)
            Wr = (Wk - W8.astype(np.float32)).astype(F8)
            m[f"w8{key}"] = _pack_cmajor(W8)
            m[f"wr{key}"] = _pack_cmajor(Wr)
        m["wpp"] = _pack_cmajor(W_proj[cs, :].astype(BF16))
        m["bq"] = np.ascontiguousarray(
            b_attn[0 * C:1 * C][cs].reshape(2, 128).T).astype(np.float32)
        m["bk"] = np.ascontiguousarray(
            b_attn[1 * C:2 * C][cs].reshape(2, 128).T).astype(np.float32)
        m["bvr"] = np.ascontiguousarray(
            np.tile(b_attn[2 * C:3 * C][cs][None, :], (128, 1))).astype(np.float32)
        in_maps.append(m)
    return in_maps


def assemble_output(results, b_proj):
    """results: per-core dicts with 'out' [T, C] bf16 partials."""
    b_proj = np.asarray(b_proj, np.float32)
    out = np.zeros((B, T, C), np.float32)
    for g in range(N_CORES):
        out[g // 4] += np.asarray(results[g]["out"], np.float32)
    out += b_proj[None, None, :]
    return out


def kernel(x, W_attn, b_attn, W_proj, b_proj):
    from concourse.bass_utils import run_bass_kernel_spmd

    nc = get_program()
    in_maps = make_in_maps(x, W_attn, b_attn, W_proj)
    res = run_bass_kernel_spmd(nc, in_maps, list(range(N_CORES)))
    return assemble_output(res.results, b_proj)


# revision 46
# speedup vs baseline: 1.2992x; 1.0041x over previous
"""GPT2 causal attention (B=2, T=2048, C=1024, H=16) on 8 TRN2 NeuronCores.

Sharding: core g = (batch b = g//4, head-group hg = g%4 of 4 heads).
Tensor-parallel over heads x data-parallel over batch. Each core emits a
full [T, C] bf16 partial of the output projection for its 4 heads; host
sums the 4 partials per batch and adds b_proj. No collectives.

Per-core kernel:
  QKV projections run in compensated fp8 (e4m3) DoubleRow matmuls:
  x = x8 + xr and W = W8 + Wr host-split (W pre-scaled by 64), with
  x@W ~= x8@W8 + x8@Wr + xr@W8 (error ~ xr@Wr = O(eps^2), below bf16).
  Each DoubleRow matmul contracts 256 rows (a pair of 128-row k-tiles).

  Scores per head pair, transposed: S^T[tk, tq] = kT^T @ qT in bf16,
  causally trimmed to [d0:512] at 128-col granularity (both the matmuls
  and the exp), exp'd on ACT without max subtraction, diagonal 128-blocks
  masked with a host tri mask on DVE.

  attention*V runs in the natural orientation: per 128-row tq tile,
  yu[tq, (h, 65)] = et_chunk^T @ V_aug accumulates over tk tiles in PSUM;
  V_aug carries a ones-column per head so column 64 of each head group is
  the softmax row-sum, landing on the free dim. Normalization is then
  per-partition: gpsimd normalize_recip (attn library) divides by the sum
  and writes bf16. y is transposed back to [d, t] with XBAR DMA
  transposes for the bf16 output projection.
"""

import numpy as np
import ml_dtypes

BF16 = ml_dtypes.bfloat16
F8 = ml_dtypes.float8_e4m3

B, T, C, H, D = 2, 2048, 1024, 16, 64
HL = 4          # heads per core
DL = HL * D     # 256 local head dims
N_CORES = 8
NT = T // 128   # 16 t tiles
NJ = T // 512   # 4 tq groups
SCALE = 1.0 / np.sqrt(D)
WSC = 64.0      # host pre-scale on W_attn for fp8 range
VSTR = HL * 65  # V tile col stride (4 heads x (64 d + ones col))
YOFF = (0, 130, 260, 512)  # yu subtile col offsets (none crosses a bank)

_CACHE = {}


def _build_program():
    import concourse.tile as tile
    from concourse import bacc
    from concourse import library_config
    import concourse.mybir as mybir

    f32 = mybir.dt.float32
    bf16 = mybir.dt.bfloat16
    fp8 = mybir.dt.float8e4
    Exp = mybir.ActivationFunctionType.Exp
    DR = mybir.MatmulPerfMode.DoubleRow

    nc = bacc.Bacc("TRN2", target_bir_lowering=False, debug=False)

    x8_d = nc.dram_tensor("x8p", [128, 8 * T], fp8, kind="ExternalInput").ap()
    xr_d = nc.dram_tensor("xrp", [128, 8 * T], fp8, kind="ExternalInput").ap()
    w8_d = {k: nc.dram_tensor(f"w8{k}", [128, 8 * DL], fp8,
                              kind="ExternalInput").ap() for k in "qkv"}
    wr_d = {k: nc.dram_tensor(f"wr{k}", [128, 8 * DL], fp8,
                              kind="ExternalInput").ap() for k in "qkv"}
    wp_d = nc.dram_tensor("wpp", [128, 2 * C], bf16, kind="ExternalInput").ap()
    bq_d = nc.dram_tensor("bq", [128, 2], f32, kind="ExternalInput").ap()
    bk_d = nc.dram_tensor("bk", [128, 2], f32, kind="ExternalInput").ap()
    bvr_d = nc.dram_tensor("bvr", [128, DL], f32, kind="ExternalInput").ap()
    mlo_d = nc.dram_tensor("mlo", [128, 128], bf16, kind="ExternalInput").ap()
    eye_d = nc.dram_tensor("eye", [128, 128], bf16, kind="ExternalInput").ap()
    out_d = nc.dram_tensor("out", [T, C], bf16, kind="ExternalOutput").ap()
    import os
    dbg = os.environ.get("K_DEBUG") == "1"
    if dbg:
        dbg_d = {n: nc.dram_tensor(f"dbg_{n}", [128, 2 * T], bf16,
                                   kind="ExternalOutput").ap()
                 for n in ("qT", "kT", "yT")}
        dbgv_d = nc.dram_tensor("dbg_V", [128, NT * VSTR], bf16,
                                kind="ExternalOutput").ap()
        dbgyn_d = nc.dram_tensor("dbg_yn", [128, 2 * T], bf16,
                                 kind="ExternalOutput").ap()
        dbgstg_d = nc.dram_tensor("dbg_stg", [128, 32 * 130], f32,
                                  kind="ExternalOutput").ap()

    with tile.TileContext(nc) as tc:
        with (
            tc.tile_pool(name="const", bufs=1) as cpool,
            tc.tile_pool(name="exp", bufs=6) as epool,
            tc.tile_pool(name="ystg", bufs=4) as spool,
            tc.tile_pool(name="ynat", bufs=4) as npool,
            tc.tile_pool(name="ostage", bufs=4) as opool,
            tc.tile_pool(name="pssc", bufs=2, space="PSUM") as psc,
            tc.tile_pool(name="psqkv", bufs=2, space="PSUM") as pqkv,
            tc.tile_pool(name="psyu", bufs=1, space="PSUM") as pyu,
        ):
            # ---- persistent SBUF ----
            x8 = cpool.tile([128, 8 * T], fp8, tag="x8")     # chunk c at c*T
            xr = cpool.tile([128, 8 * T], fp8, tag="xr")
            w8 = {k: cpool.tile([128, 8 * DL], fp8, tag=f"w8{k}", name=f"w8{k}")
                  for k in "qkv"}
            wr = {k: cpool.tile([128, 8 * DL], fp8, tag=f"wr{k}", name=f"wr{k}")
                  for k in "qkv"}
            wp = cpool.tile([128, 2 * C], bf16, tag="wp")    # d-chunk dc at dc*C
            bq = cpool.tile([128, 2], f32, tag="bq")
            bk = cpool.tile([128, 2], f32, tag="bk")
            bvr = cpool.tile([128, DL], f32, tag="bvr")
            mlo = cpool.tile([128, 128], bf16, tag="mlo")
            eye = cpool.tile([128, 128], bf16, tag="eye")
            qT = cpool.tile([128, 2 * T], bf16, tag="qT")    # head h: rows 64*(h%2), cols (h//2)*T+t
            kT = cpool.tile([128, 2 * T], bf16, tag="kT")
            yT = cpool.tile([128, 2 * T], bf16, tag="yT")    # d-chunk dc at dc*T
            V = cpool.tile([128, NT * VSTR], bf16, tag="V")  # tile tt, head h at tt*VSTR+65h

            nc.gpsimd.load_library(library_config.attn)

            # ---- input DMAs, ordered for earliest PE start ----
            nc.sync.dma_start(out=bq[:, :], in_=bq_d[:, :])
            nc.sync.dma_start(out=bk[:, :], in_=bk_d[:, :])
            nc.sync.dma_start(out=w8["q"][:, :], in_=w8_d["q"][:, :])
            nc.sync.dma_start(out=x8[:, 0:2 * T], in_=x8_d[:, 0:2 * T])
            nc.sync.dma_start(out=w8["k"][:, :], in_=w8_d["k"][:, :])
            for c in range(1, 4):  # x8 in DoubleRow 2-chunk pairs
                nc.sync.dma_start(out=x8[:, 2 * c * T:(2 * c + 2) * T],
                                  in_=x8_d[:, 2 * c * T:(2 * c + 2) * T])
            for c in range(4):
                nc.sync.dma_start(out=xr[:, 2 * c * T:(2 * c + 2) * T],
                                  in_=xr_d[:, 2 * c * T:(2 * c + 2) * T])
            nc.sync.dma_start(out=wr["q"][:, :], in_=wr_d["q"][:, :])
            nc.sync.dma_start(out=wr["k"][:, :], in_=wr_d["k"][:, :])
            nc.sync.dma_start(out=w8["v"][:, :], in_=w8_d["v"][:, :])
            nc.sync.dma_start(out=bvr[:, :], in_=bvr_d[:, :])
            nc.sync.dma_start(out=wr["v"][:, :], in_=wr_d["v"][:, :])
            nc.sync.dma_start(out=mlo[:, :], in_=mlo_d[:, :])
            nc.sync.dma_start(out=eye[:, :], in_=eye_d[:, :])
            nc.sync.dma_start(out=wp[:, :], in_=wp_d[:, :])

            warm = epool.tile([128, 2], bf16, tag="exp", name="warm")
            nc.scalar.activation(warm[:, :], bq[:, :],
                                 Exp, scale=1e-6)

            x8c = x8[:, :].rearrange("p (c t) -> p c t", c=8)
            xrc = xr[:, :].rearrange("p (c t) -> p c t", c=8)
            w8c = {k: w8[k][:, :].rearrange("p (c m) -> p c m", c=8) for k in "qkv"}
            wrc = {k: wr[k][:, :].rearrange("p (c m) -> p c m", c=8) for k in "qkv"}
            TERMS = [(x8c, w8c), (xrc, w8c), (x8c, wrc)]

            def qk_tile(key, dc, ts, pool):
                """One [128d, 512t] q/k projection tile + bias eviction."""
                b_sb, dst = (bq, qT) if key == "q" else (bk, kT)
                ps = pool.tile([128, 512], f32, tag=pool._qkv_tag,
                               name=f"p{key}_{dc}_{ts}")
                msl = slice(dc * 128, (dc + 1) * 128)
                nsl = slice(ts * 512, (ts + 1) * 512)
                for term, (xa, wd) in enumerate(TERMS):
                    for c in range(4):
                        nc.tensor.matmul(
                            ps[:, 0:512],
                            wd[key][:, 2 * c:2 * c + 2, msl],
                            xa[:, 2 * c:2 * c + 2, nsl],
                            start=(term == 0 and c == 0),
                            stop=(term == 2 and c == 3),
                            perf_mode=DR,
                        )
                nc.vector.tensor_scalar_add(
                    dst[:, dc * T + ts * 512: dc * T + (ts + 1) * 512],
                    ps[:, 0:512], b_sb[:, dc:dc + 1])

            def v_tile(tt, pool):
                """One [128t, 256d] V tile (natural layout) + bias eviction."""
                ps = pool.tile([128, 512], f32, tag=pool._qkv_tag,
                               name=f"pv_{tt}")
                tsl = slice(tt * 128, (tt + 1) * 128)
                for term, (xa, wd) in enumerate(TERMS):
                    for c in range(4):
                        nc.tensor.matmul(
                            ps[:, 0:DL],
                            xa[:, 2 * c:2 * c + 2, tsl],
                            wd["v"][:, 2 * c:2 * c + 2, :],
                            start=(term == 0 and c == 0),
                            stop=(term == 2 and c == 3),
                            perf_mode=DR,
                        )
                vdst = V[:, tt * VSTR:(tt + 1) * VSTR].rearrange(
                    "p (h e) -> p h e", h=HL)[:, :, 0:64]
                nc.vector.tensor_add(
                    vdst, ps[:, 0:DL].rearrange("p (h e) -> p h e", h=HL),
                    bvr[:, :].rearrange("p (h e) -> p h e", h=HL))

            psc._qkv_tag = "sc"
            pqkv._qkv_tag = "qkv"

            # ---- startup QKV: only what group (hp0, j0) needs ----
            nc.vector.memset(V[:, :], 1.0)  # ones cols; data overwritten
            # q0/k0 startup tiles, chunk-major interleaved so neither
            # blocks the other's ready matmuls in the PE FIFO
            ps_q = psc.tile([128, 512], f32, tag="sc", name="pq_0_0")
            ps_k = psc.tile([128, 512], f32, tag="sc", name="pk_0_0")
            for term, (xa, wd) in enumerate(TERMS):
                for c in range(4):
                    for key, ps in (("q", ps_q), ("k", ps_k)):
                        nc.tensor.matmul(
                            ps[:, 0:512],
                            wd[key][:, 2 * c:2 * c + 2, 0:128],
                            xa[:, 2 * c:2 * c + 2, 0:512],
                            start=(term == 0 and c == 0),
                            stop=(term == 2 and c == 3),
                            perf_mode=DR,
                        )
            nc.vector.tensor_scalar_add(qT[:, 0:512], ps_q[:, 0:512],
                                        bq[:, 0:1])
            nc.vector.tensor_scalar_add(kT[:, 0:512], ps_k[:, 0:512],
                                        bk[:, 0:1])
            v_tile(0, psc)
            v_tile(1, psc)

            # filler thunks: spread through attention on the pqkv ring, in
            # deadline order — each must be EMITTED before the attention
            # group that consumes it (program order is the dataflow).
            # (deadline_step, thunk): emitted no later than deadline, so
            # every attention read sees its producer earlier in program order
            filler = [
                (1, lambda: v_tile(2, pqkv)),
                (2, lambda: v_tile(3, pqkv)),
                (3, lambda: qk_tile("k", 0, 1, pqkv)),
                (3, lambda: qk_tile("q", 0, 1, pqkv)),
                (5, lambda: v_tile(4, pqkv)),
                (6, lambda: v_tile(5, pqkv)),
                (7, lambda: v_tile(6, pqkv)),
                (8, lambda: v_tile(7, pqkv)),
                (10, lambda: qk_tile("k", 0, 2, pqkv)),
                (10, lambda: qk_tile("q", 0, 2, pqkv)),
                (13, lambda: v_tile(8, pqkv)),
                (15, lambda: v_tile(9, pqkv)),
                (17, lambda: v_tile(10, pqkv)),
                (19, lambda: v_tile(11, pqkv)),
                (21, lambda: qk_tile("k", 0, 3, pqkv)),
                (21, lambda: qk_tile("q", 0, 3, pqkv)),
                (24, lambda: v_tile(12, pqkv)),
                (27, lambda: v_tile(13, pqkv)),
                (30, lambda: v_tile(14, pqkv)),
                (33, lambda: v_tile(15, pqkv)),
                (36, lambda: qk_tile("k", 1, 0, pqkv)),
                (37, lambda: qk_tile("q", 1, 0, pqkv)),
                (38, lambda: qk_tile("k", 1, 1, pqkv)),
                (39, lambda: qk_tile("q", 1, 1, pqkv)),
                (40, lambda: qk_tile("k", 1, 2, pqkv)),
                (41, lambda: qk_tile("q", 1, 2, pqkv)),
                (42, lambda: qk_tile("k", 1, 3, pqkv)),
                (43, lambda: qk_tile("q", 1, 3, pqkv)),
            ]

            pending = []   # (ready_step, thunk) deferred PE work
            step = [0]
            NSTEPS = 80    # total attention i steps; clamp deferrals

            def tick():
                step[0] += 1
                while filler and filler[0][0] <= step[0]:
                    filler.pop(0)[1]()
                while pending and pending[0][0] <= step[0]:
                    pending.pop(0)[1]()

            def proj_tile(tt, last):
                """Output projection for one 128-row t tile: [128, 1024]."""
                ot = opool.tile([128, 1024], bf16, tag="ot", name=f"ot_{tt}")
                for cc in range(2):
                    pp = pqkv.tile([128, 512], f32, tag="qkv",
                                   name=f"po_{tt}_{cc}")
                    for dc in range(2):
                        nc.tensor.matmul(
                            pp[:, :],
                            yT[:, dc * T + tt * 128: dc * T + (tt + 1) * 128],
                            wp[:, dc * C + cc * 512: dc * C + (cc + 1) * 512],
                            start=(dc == 0), stop=(dc == 1),
                        )
                    nc.vector.tensor_copy(ot[:, cc * 512:(cc + 1) * 512],
                                          pp[:, :])
                    if last:
                        nc.sync.dma_start(
                            out=out_d[tt * 128:(tt + 1) * 128,
                                      cc * 512:(cc + 1) * 512],
                            in_=ot[:, cc * 512:(cc + 1) * 512])

                def out_dma(tt=tt, ot=ot):
                    nc.sync.dma_start(
                        out=out_d[tt * 128:(tt + 1) * 128, :], in_=ot[:, :])
                if last:
                    pass  # halves DMA'd inline below
                else:
                    pending.append((min(step[0] + 2, NSTEPS - 1), out_dma))

            def finish_tile(hp, j, tl, yu, last_group):
                """After tile tl's diagonal yu: stage, normalize, transpose,
                and (in the hp1 phase) queue the output projection."""
                tt = 4 * j + tl
                stg = spool.tile([128, 130], f32, tag="stg",
                                 name=f"stg_{hp}_{tt}")
                nc.vector.tensor_copy(stg[:, :],
                                      yu[:, YOFF[tl]:YOFF[tl] + 130])
                if dbg:
                    nc.sync.dma_start(
                        out=dbgstg_d[:, (hp * NT + tt) * 130:
                                     (hp * NT + tt + 1) * 130],
                        in_=stg[:, :])
                yn = npool.tile([128, 128], bf16, tag="yn",
                                name=f"yn_{hp}_{tt}")
                for lh in range(2):
                    nc.gpsimd.normalize_recip(
                        yn[:, lh * 64:(lh + 1) * 64],
                        stg[:, lh * 65: lh * 65 + 64],
                        stg[:, lh * 65 + 64: lh * 65 + 65],
                    )
                if dbg:
                    nc.sync.dma_start(
                        out=dbgyn_d[:, hp * T + tt * 128:
                                    hp * T + (tt + 1) * 128],
                        in_=yn[:, :])

                def transp(hp=hp, tt=tt, yn=yn):
                    nc.sync.dma_start(
                        out=yT[:, hp * T + tt * 128: hp * T + (tt + 1) * 128],
                        in_=yn[:, :], transpose=True)
                tail = last_group and tl == 3
                if tail:
                    # PE transpose through a spare score-ring bank: the
                    # score stream is over, and it is ~1.5us faster than
                    # the XBAR DMA path on the critical tail
                    pt = psc.tile([128, 128], bf16, tag="sc", name="pt_tail")
                    nc.tensor.transpose(pt[:, :], yn[:, :], eye[:, :])
                    nc.vector.tensor_copy(
                        yT[:, hp * T + tt * 128: hp * T + (tt + 1) * 128],
                        pt[:, :])
                    proj_tile(tt, True)
                else:
                    dt_, dp_ = (1, 2) if last_group else (1, 5)
                    pending.append((min(step[0] + 2, NSTEPS - 2), transp))
                    if hp == 1:
                        pending.append((min(step[0] + dp_, NSTEPS - 1),
                                        lambda tt=tt: proj_tile(tt, False)))

            yu_tiles = {}

            def make_step(hp, j, i, last_group):
                """Returns (sc_thunk, yu_thunk) for one i step."""
                ni = 4 * j + 4
                fb = hp * T
                d0 = max(128 * (i - 4 * j), 0)
                box = {}

                def sc_emit():
                    if i == 0:
                        yu_tiles[(hp, j)] = pyu.tile(
                            [128, 642], f32, tag="yu", name=f"yu_{hp}_{j}")
                    sc = psc.tile([128, 1024], f32, tag="sc",
                                  name=f"sc_{hp}_{j}_{i}")
                    for half in (0, 1):
                        po = 64 * half
                        nc.tensor.matmul(
                            sc[:, half * 512 + d0:(half + 1) * 512],
                            kT[po:po + 64, fb + i * 128: fb + (i + 1) * 128],
                            qT[po:po + 64,
                               fb + j * 512 + d0: fb + (j + 1) * 512],
                            start=True, stop=True,
                        )
                    et = epool.tile([128, 1024], bf16, tag="exp",
                                    name=f"et_{hp}_{j}_{i}")
                    nc.scalar.activation(
                        et[:, :].rearrange("p (g q) -> p g q", g=2)[:, :, d0:512],
                        sc[:, :].rearrange("p (g q) -> p g q", g=2)[:, :, d0:512],
                        Exp, scale=float(SCALE / (WSC * WSC)),
                    )
                    if i >= 4 * j:
                        for half in (0, 1):
                            sl = slice(half * 512 + d0, half * 512 + d0 + 128)
                            nc.vector.tensor_mul(et[:, sl], et[:, sl],
                                                 mlo[:, :])
                    box["et"] = et

                def yu_emit():
                    # start=True clears has_written for the WHOLE PSUM bank,
                    # so only the first matmul into each bank of the yu tile
                    # may carry it; later subtiles first-write on cleared
                    # bits (overwrite) with start=False.
                    yu, et = yu_tiles[(hp, j)], box["et"]
                    for tl in range(4):
                        if 128 * tl < d0:
                            continue
                        for half in (0, 1):
                            h = 2 * hp + half
                            nc.tensor.matmul(
                                yu[:, YOFF[tl] + 65 * half:
                                   YOFF[tl] + 65 * half + 65],
                                et[:, half * 512 + tl * 128:
                                   half * 512 + (tl + 1) * 128],
                                V[:, i * VSTR + 65 * h:
                                  i * VSTR + 65 * h + 65],
                                start=(i == 0 and half == 0 and tl in (0, 3)),
                                stop=(i == 4 * j + tl),
                                skip_group_check=True,
                            )
                    if i >= 4 * j:
                        finish_tile(hp, j, i - 4 * j, yu, last_group)

                return sc_emit, yu_emit

            steps = []
            for hp in range(2):
                for j in range(NJ):
                    for i in range(4 * j + 4):
                        steps.append(make_step(hp, j, i,
                                               hp == 1 and j == NJ - 1))
            # flat software pipeline: sc(i+1) is emitted before yu(i), incl.
            # across group boundaries, so the PE never heads-of-line on exp
            from collections import deque
            inflight = deque()
            for sc_emit, yu_emit in steps:
                sc_emit()
                if len(inflight) == 2:
                    inflight.popleft()()
                inflight.append(yu_emit)
                tick()
            while inflight:
                inflight.popleft()()
                tick()
            while filler:
                filler.pop(0)[1]()
            while pending:
                pending.pop(0)[1]()
            if dbg:
                for n, sb in (("qT", qT), ("kT", kT), ("yT", yT)):
                    nc.sync.dma_start(out=dbg_d[n][:, :], in_=sb[:, :])
                nc.sync.dma_start(out=dbgv_d[:, :], in_=V[:, :])

    nc.compile()
    return nc


def get_program():
    if "nc" not in _CACHE:
        _CACHE["nc"] = _build_program()
    return _CACHE["nc"]


def _pack_cmajor(a):
    """[C_rows, N] -> [128, (C_rows/128)*N] with chunk c at [:, c*N:(c+1)*N]."""
    rows, n = a.shape
    return np.ascontiguousarray(
        a.reshape(rows // 128, 128, n).transpose(1, 0, 2).reshape(128, -1))


def make_in_maps(x, W_attn, b_attn, W_proj):
    """Host-side sharding: per-core input dict."""
    x = np.asarray(x, np.float32)
    W_attn = np.asarray(W_attn, np.float32) * WSC
    b_attn = np.asarray(b_attn, np.float32) * WSC
    W_proj = np.asarray(W_proj, np.float32) / WSC

    mlo = (np.arange(128)[None, :] >= np.arange(128)[:, None]).astype(BF16)

    x8_b, xr_b = [], []
    for b in range(B):
        xt = x[b].T.astype(np.float32)                    # [C, T]
        x8 = xt.astype(F8)
        xres = (xt - x8.astype(np.float32)).astype(F8)
        x8_b.append(_pack_cmajor(x8))
        xr_b.append(_pack_cmajor(xres))

    in_maps = []
    for g in range(N_CORES):
        b, hg = divmod(g, 4)
        cs = slice(hg * DL, (hg + 1) * DL)
        m = {"x8p": x8_b[b], "xrp": xr_b[b], "mlo": mlo,
             "eye": np.eye(128).astype(BF16)}
        for ki, key in enumerate("qkv"):
            Wk = W_attn[:, ki * C:(ki + 1) * C][:, cs]
            W8 = Wk.astype(F8)
            Wr = (Wk - W8.astype(np.float32)).astype(F8)
            m[f"w8{key}"] = _pack_cmajor(W8)
            m[f"wr{key}"] = _pack_cmajor(Wr)
        m["wpp"] = _pack_cmajor(W_proj[cs, :].astype(BF16))
        m["bq"] = np.ascontiguousarray(
            b_attn[0 * C:1 * C][cs].reshape(2, 128).T).astype(np.float32)
        m["bk"] = np.ascontiguousarray(
            b_attn[1 * C:2 * C][cs].reshape(2, 128).T).astype(np.float32)
        m["bvr"] = np.ascontiguousarray(
            np.tile(b_attn[2 * C:3 * C][cs][None, :], (128, 1))).astype(np.float32)
        in_maps.append(m)
    return in_maps


def assemble_output(results, b_proj):
    """results: per-core dicts with 'out' [T, C] bf16 partials."""
    b_proj = np.asarray(b_proj, np.float32)
    out = np.zeros((B, T, C), np.float32)
    for g in range(N_CORES):
        out[g // 4] += np.asarray(results[g]["out"], np.float32)
    out += b_proj[None, None, :]
    return out


def kernel(x, W_attn, b_attn, W_proj, b_proj):
    from concourse.bass_utils import run_bass_kernel_spmd

    nc = get_program()
    in_maps = make_in_maps(x, W_attn, b_attn, W_proj)
    res = run_bass_kernel_spmd(nc, in_maps, list(range(N_CORES)))
    return assemble_output(res.results, b_proj)


# revision 52
# speedup vs baseline: 1.3887x; 1.0688x over previous
"""GPT2 causal attention (B=2, T=2048, C=1024, H=16) on 8 TRN2 NeuronCores.

Sharding: core g = (batch b = g//4, head-group hg = g%4 of 4 heads).
Tensor-parallel over heads x data-parallel over batch. Each core emits a
full [T, C] bf16 partial of the output projection for its 4 heads; host
sums the 4 partials per batch and adds b_proj. No collectives.

Per-core kernel:
  QKV projections run in compensated fp8 (e4m3) DoubleRow matmuls:
  x = x8 + xr and W = W8 + Wr host-split (W pre-scaled by 64), with
  x@W ~= x8@W8 + x8@Wr + xr@W8 (error ~ xr@Wr = O(eps^2), below bf16).
  Each DoubleRow matmul contracts 256 rows (a pair of 128-row k-tiles).

  Scores per head pair, transposed: S^T[tk, tq] = kT^T @ qT in bf16,
  causally trimmed to [d0:512] at 128-col granularity (both the matmuls
  and the exp), exp'd on ACT without max subtraction, diagonal 128-blocks
  masked with a host tri mask on DVE.

  attention*V runs in the natural orientation: per 128-row tq tile,
  yu[tq, (h, 65)] = et_chunk^T @ V_aug accumulates over tk tiles in PSUM;
  V_aug carries a ones-column per head so column 64 of each head group is
  the softmax row-sum, landing on the free dim. Normalization is then
  per-partition: gpsimd normalize_recip (attn library) divides by the sum
  and writes bf16. y is transposed back to [d, t] with XBAR DMA
  transposes for the bf16 output projection.
"""

import numpy as np
import ml_dtypes

BF16 = ml_dtypes.bfloat16
F8 = ml_dtypes.float8_e4m3

B, T, C, H, D = 2, 2048, 1024, 16, 64
HL = 4          # heads per core
DL = HL * D     # 256 local head dims
N_CORES = 8
NT = T // 128   # 16 t tiles
NJ = T // 512   # 4 tq groups
SCALE = 1.0 / np.sqrt(D)
WSC = 64.0      # host pre-scale on W_attn for fp8 range
VSTR = HL * 65  # V tile col stride (4 heads x (64 d + ones col))
YOFF = (0, 130, 260, 512)  # yu subtile col offsets (none crosses a bank)

_CACHE = {}


def _build_program():
    import concourse.tile as tile
    from concourse import bacc
    from concourse import library_config
    import concourse.mybir as mybir

    f32 = mybir.dt.float32
    bf16 = mybir.dt.bfloat16
    fp8 = mybir.dt.float8e4
    Exp = mybir.ActivationFunctionType.Exp
    DR = mybir.MatmulPerfMode.DoubleRow

    nc = bacc.Bacc("TRN2", target_bir_lowering=False, debug=False)

    x8_d = nc.dram_tensor("x8p", [128, 8 * T], fp8, kind="ExternalInput").ap()
    xr_d = nc.dram_tensor("xrp", [128, 8 * T], fp8, kind="ExternalInput").ap()
    w8_d = {k: nc.dram_tensor(f"w8{k}", [128, 8 * DL], fp8,
                              kind="ExternalInput").ap() for k in "qkv"}
    wr_d = {k: nc.dram_tensor(f"wr{k}", [128, 8 * DL], fp8,
                              kind="ExternalInput").ap() for k in "qkv"}
    wp_d = nc.dram_tensor("wpp", [128, 2 * C], bf16, kind="ExternalInput").ap()
    bq_d = nc.dram_tensor("bq", [128, 2], f32, kind="ExternalInput").ap()
    bk_d = nc.dram_tensor("bk", [128, 2], f32, kind="ExternalInput").ap()
    bvr_d = nc.dram_tensor("bvr", [128, DL], f32, kind="ExternalInput").ap()
    mlo_d = nc.dram_tensor("mlo", [128, 128], bf16, kind="ExternalInput").ap()
    eye_d = nc.dram_tensor("eye", [128, 128], bf16, kind="ExternalInput").ap()
    out_d = nc.dram_tensor("out", [T, C], bf16, kind="ExternalOutput").ap()
    import os
    dbg = os.environ.get("K_DEBUG") == "1"
    if dbg:
        dbg_d = {n: nc.dram_tensor(f"dbg_{n}", [128, 2 * T], bf16,
                                   kind="ExternalOutput").ap()
                 for n in ("qT", "kT", "yT")}
        dbgv_d = nc.dram_tensor("dbg_V", [128, NT * VSTR], bf16,
                                kind="ExternalOutput").ap()
        dbgyn_d = nc.dram_tensor("dbg_yn", [128, 2 * T], bf16,
                                 kind="ExternalOutput").ap()
        dbgstg_d = nc.dram_tensor("dbg_stg", [128, 32 * 130], f32,
                                  kind="ExternalOutput").ap()

    with tile.TileContext(nc) as tc:
        with (
            tc.tile_pool(name="const", bufs=1) as cpool,
            tc.tile_pool(name="exp", bufs=6) as epool,
            tc.tile_pool(name="ystg", bufs=4) as spool,
            tc.tile_pool(name="ynat", bufs=4) as npool,
            tc.tile_pool(name="ostage", bufs=4) as opool,
            tc.tile_pool(name="pssc", bufs=2, space="PSUM") as psc,
            tc.tile_pool(name="psqkv", bufs=2, space="PSUM") as pqkv,
            tc.tile_pool(name="psyu", bufs=1, space="PSUM") as pyu,
        ):
            # ---- persistent SBUF ----
            x8 = cpool.tile([128, 8 * T], fp8, tag="x8")     # chunk c at c*T
            xr = cpool.tile([128, 8 * T], fp8, tag="xr")
            w8 = {k: cpool.tile([128, 8 * DL], fp8, tag=f"w8{k}", name=f"w8{k}")
                  for k in "qkv"}
            wr = {k: cpool.tile([128, 8 * DL], fp8, tag=f"wr{k}", name=f"wr{k}")
                  for k in "qkv"}
            wp = cpool.tile([128, 2 * C], bf16, tag="wp")    # d-chunk dc at dc*C
            bq = cpool.tile([128, 2], f32, tag="bq")
            bk = cpool.tile([128, 2], f32, tag="bk")
            bvr = cpool.tile([128, DL], f32, tag="bvr")
            mlo = cpool.tile([128, 128], bf16, tag="mlo")
            eye = cpool.tile([128, 128], bf16, tag="eye")
            qT = cpool.tile([128, 2 * T], bf16, tag="qT")    # head h: rows 64*(h%2), cols (h//2)*T+t
            kT = cpool.tile([128, 2 * T], bf16, tag="kT")
            yT = cpool.tile([128, 2 * T], bf16, tag="yT")    # d-chunk dc at dc*T
            V = cpool.tile([128, NT * VSTR], bf16, tag="V")  # tile tt, head h at tt*VSTR+65h

            nc.gpsimd.load_library(library_config.attn)

            # ---- input DMAs ----
            # x streams in t-block-major quarters: one 3D-AP DMA per
            # quarter covers that t-range of all 8 c-chunks (512B runs),
            # so the first q/k/V tiles only wait for a quarter of x
            x8v = x8[:, :].rearrange("p (c t) -> p c t", c=8)
            xrv = xr[:, :].rearrange("p (c t) -> p c t", c=8)
            x8dv = x8_d[:, :].rearrange("p (c t) -> p c t", c=8)
            xrdv = xr_d[:, :].rearrange("p (c t) -> p c t", c=8)
            nc.sync.dma_start(out=bq[:, :], in_=bq_d[:, :])
            nc.sync.dma_start(out=bk[:, :], in_=bk_d[:, :])
            nc.sync.dma_start(out=w8["q"][:, :], in_=w8_d["q"][:, :])
            nc.sync.dma_start(out=x8v[:, :, 0:512], in_=x8dv[:, :, 0:512])
            nc.sync.dma_start(out=w8["k"][:, :], in_=w8_d["k"][:, :])
            nc.sync.dma_start(out=xrv[:, :, 0:512], in_=xrdv[:, :, 0:512])
            nc.sync.dma_start(out=wr["q"][:, :], in_=wr_d["q"][:, :])
            nc.sync.dma_start(out=wr["k"][:, :], in_=wr_d["k"][:, :])
            nc.sync.dma_start(out=w8["v"][:, :], in_=w8_d["v"][:, :])
            nc.sync.dma_start(out=wr["v"][:, :], in_=wr_d["v"][:, :])
            nc.sync.dma_start(out=bvr[:, :], in_=bvr_d[:, :])
            for tb in range(1, 4):
                tsl = slice(tb * 512, (tb + 1) * 512)
                nc.sync.dma_start(out=x8v[:, :, tsl], in_=x8dv[:, :, tsl])
                nc.sync.dma_start(out=xrv[:, :, tsl], in_=xrdv[:, :, tsl])
            nc.sync.dma_start(out=mlo[:, :], in_=mlo_d[:, :])
            nc.sync.dma_start(out=eye[:, :], in_=eye_d[:, :])
            nc.sync.dma_start(out=wp[:, :], in_=wp_d[:, :])

            warm = epool.tile([128, 2], bf16, tag="exp", name="warm")
            nc.scalar.activation(warm[:, :], bq[:, :],
                                 Exp, scale=1e-6)

            x8c = x8[:, :].rearrange("p (c t) -> p c t", c=8)
            xrc = xr[:, :].rearrange("p (c t) -> p c t", c=8)
            w8c = {k: w8[k][:, :].rearrange("p (c m) -> p c m", c=8) for k in "qkv"}
            wrc = {k: wr[k][:, :].rearrange("p (c m) -> p c m", c=8) for k in "qkv"}
            TERMS = [(x8c, w8c), (xrc, w8c), (x8c, wrc)]

            def qk_tile(key, dc, ts, pool):
                """One [128d, 512t] q/k projection tile + bias eviction."""
                b_sb, dst = (bq, qT) if key == "q" else (bk, kT)
                ps = pool.tile([128, 512], f32, tag=pool._qkv_tag,
                               name=f"p{key}_{dc}_{ts}")
                msl = slice(dc * 128, (dc + 1) * 128)
                nsl = slice(ts * 512, (ts + 1) * 512)
                for term, (xa, wd) in enumerate(TERMS):
                    for c in range(4):
                        nc.tensor.matmul(
                            ps[:, 0:512],
                            wd[key][:, 2 * c:2 * c + 2, msl],
                            xa[:, 2 * c:2 * c + 2, nsl],
                            start=(term == 0 and c == 0),
                            stop=(term == 2 and c == 3),
                            perf_mode=DR,
                        )
                nc.vector.tensor_scalar_add(
                    dst[:, dc * T + ts * 512: dc * T + (ts + 1) * 512],
                    ps[:, 0:512], b_sb[:, dc:dc + 1])

            def v_tile(tt, pool):
                """One [128t, 256d] V tile (natural layout) + bias eviction."""
                ps = pool.tile([128, 512], f32, tag=pool._qkv_tag,
                               name=f"pv_{tt}")
                tsl = slice(tt * 128, (tt + 1) * 128)
                for term, (xa, wd) in enumerate(TERMS):
                    for c in range(4):
                        nc.tensor.matmul(
                            ps[:, 0:DL],
                            xa[:, 2 * c:2 * c + 2, tsl],
                            wd["v"][:, 2 * c:2 * c + 2, :],
                            start=(term == 0 and c == 0),
                            stop=(term == 2 and c == 3),
                            perf_mode=DR,
                        )
                vdst = V[:, tt * VSTR:(tt + 1) * VSTR].rearrange(
                    "p (h e) -> p h e", h=HL)[:, :, 0:64]
                nc.vector.tensor_add(
                    vdst, ps[:, 0:DL].rearrange("p (h e) -> p h e", h=HL),
                    bvr[:, :].rearrange("p (h e) -> p h e", h=HL))

            psc._qkv_tag = "sc"
            pqkv._qkv_tag = "qkv"

            # ---- startup QKV: only what group (hp0, j0) needs ----
            nc.vector.memset(V[:, :], 1.0)  # ones cols; data overwritten
            # q0/k0 startup tiles, chunk-major interleaved so neither
            # blocks the other's ready matmuls in the PE FIFO
            ps_q = psc.tile([128, 512], f32, tag="sc", name="pq_0_0")
            ps_k = psc.tile([128, 512], f32, tag="sc", name="pk_0_0")
            for term, (xa, wd) in enumerate(TERMS):
                for c in range(4):
                    for key, ps in (("q", ps_q), ("k", ps_k)):
                        nc.tensor.matmul(
                            ps[:, 0:512],
                            wd[key][:, 2 * c:2 * c + 2, 0:128],
                            xa[:, 2 * c:2 * c + 2, 0:512],
                            start=(term == 0 and c == 0),
                            stop=(term == 2 and c == 3),
                            perf_mode=DR,
                        )
            nc.vector.tensor_scalar_add(qT[:, 0:512], ps_q[:, 0:512],
                                        bq[:, 0:1])
            nc.vector.tensor_scalar_add(kT[:, 0:512], ps_k[:, 0:512],
                                        bk[:, 0:1])
            v_tile(0, psc)
            v_tile(1, psc)

            # filler thunks: spread through attention on the pqkv ring, in
            # deadline order — each must be EMITTED before the attention
            # group that consumes it (program order is the dataflow).
            # (deadline_step, thunk): emitted no later than deadline, so
            # every attention read sees its producer earlier in program order
            filler = [
                (1, lambda: v_tile(2, pqkv)),
                (2, lambda: v_tile(3, pqkv)),
                (3, lambda: qk_tile("k", 0, 1, pqkv)),
                (3, lambda: qk_tile("q", 0, 1, pqkv)),
                (5, lambda: v_tile(4, pqkv)),
                (6, lambda: v_tile(5, pqkv)),
                (7, lambda: v_tile(6, pqkv)),
                (8, lambda: v_tile(7, pqkv)),
                (10, lambda: qk_tile("k", 0, 2, pqkv)),
                (10, lambda: qk_tile("q", 0, 2, pqkv)),
                (13, lambda: v_tile(8, pqkv)),
                (15, lambda: v_tile(9, pqkv)),
                (17, lambda: v_tile(10, pqkv)),
                (19, lambda: v_tile(11, pqkv)),
                (21, lambda: qk_tile("k", 0, 3, pqkv)),
                (21, lambda: qk_tile("q", 0, 3, pqkv)),
                (24, lambda: v_tile(12, pqkv)),
                (27, lambda: v_tile(13, pqkv)),
                (30, lambda: v_tile(14, pqkv)),
                (33, lambda: v_tile(15, pqkv)),
                (36, lambda: qk_tile("k", 1, 0, pqkv)),
                (37, lambda: qk_tile("q", 1, 0, pqkv)),
                (38, lambda: qk_tile("k", 1, 1, pqkv)),
                (39, lambda: qk_tile("q", 1, 1, pqkv)),
                (40, lambda: qk_tile("k", 1, 2, pqkv)),
                (41, lambda: qk_tile("q", 1, 2, pqkv)),
                (42, lambda: qk_tile("k", 1, 3, pqkv)),
                (43, lambda: qk_tile("q", 1, 3, pqkv)),
            ]

            pending = []   # (ready_step, thunk) deferred PE work
            step = [0]
            NSTEPS = 80    # total attention i steps; clamp deferrals

            def tick():
                step[0] += 1
                while filler and filler[0][0] <= step[0]:
                    filler.pop(0)[1]()
                while pending and pending[0][0] <= step[0]:
                    pending.pop(0)[1]()

            def proj_tile(tt, last):
                """Output projection for one 128-row t tile: [128, 1024]."""
                ot = opool.tile([128, 1024], bf16, tag="ot", name=f"ot_{tt}")
                for cc in range(2):
                    pp = pqkv.tile([128, 512], f32, tag="qkv",
                                   name=f"po_{tt}_{cc}")
                    for dc in range(2):
                        nc.tensor.matmul(
                            pp[:, :],
                            yT[:, dc * T + tt * 128: dc * T + (tt + 1) * 128],
                            wp[:, dc * C + cc * 512: dc * C + (cc + 1) * 512],
                            start=(dc == 0), stop=(dc == 1),
                        )
                    nc.vector.tensor_copy(ot[:, cc * 512:(cc + 1) * 512],
                                          pp[:, :])
                    if last:
                        nc.sync.dma_start(
                            out=out_d[tt * 128:(tt + 1) * 128,
                                      cc * 512:(cc + 1) * 512],
                            in_=ot[:, cc * 512:(cc + 1) * 512])

                def out_dma(tt=tt, ot=ot):
                    nc.sync.dma_start(
                        out=out_d[tt * 128:(tt + 1) * 128, :], in_=ot[:, :])
                if last:
                    pass  # halves DMA'd inline below
                else:
                    pending.append((min(step[0] + 2, NSTEPS - 1), out_dma))

            def finish_tile(hp, j, tl, yu, last_group):
                """After tile tl's diagonal yu: stage, normalize, transpose,
                and (in the hp1 phase) queue the output projection."""
                tt = 4 * j + tl
                stg = spool.tile([128, 130], f32, tag="stg",
                                 name=f"stg_{hp}_{tt}")
                nc.vector.tensor_copy(stg[:, :],
                                      yu[:, YOFF[tl]:YOFF[tl] + 130])
                if dbg:
                    nc.sync.dma_start(
                        out=dbgstg_d[:, (hp * NT + tt) * 130:
                                     (hp * NT + tt + 1) * 130],
                        in_=stg[:, :])
                yn = npool.tile([128, 128], bf16, tag="yn",
                                name=f"yn_{hp}_{tt}")
                for lh in range(2):
                    nc.gpsimd.normalize_recip(
                        yn[:, lh * 64:(lh + 1) * 64],
                        stg[:, lh * 65: lh * 65 + 64],
                        stg[:, lh * 65 + 64: lh * 65 + 65],
                    )
                if dbg:
                    nc.sync.dma_start(
                        out=dbgyn_d[:, hp * T + tt * 128:
                                    hp * T + (tt + 1) * 128],
                        in_=yn[:, :])

                def transp(hp=hp, tt=tt, yn=yn):
                    nc.sync.dma_start(
                        out=yT[:, hp * T + tt * 128: hp * T + (tt + 1) * 128],
                        in_=yn[:, :], transpose=True)
                tail = last_group and tl == 3
                if tail:
                    # PE transpose through a spare score-ring bank: the
                    # score stream is over, and it is ~1.5us faster than
                    # the XBAR DMA path on the critical tail
                    pt = psc.tile([128, 128], bf16, tag="sc", name="pt_tail")
                    nc.tensor.transpose(pt[:, :], yn[:, :], eye[:, :])
                    nc.vector.tensor_copy(
                        yT[:, hp * T + tt * 128: hp * T + (tt + 1) * 128],
                        pt[:, :])
                    proj_tile(tt, True)
                else:
                    dt_, dp_ = (1, 2) if last_group else (1, 5)
                    pending.append((min(step[0] + 2, NSTEPS - 2), transp))
                    if hp == 1:
                        pending.append((min(step[0] + dp_, NSTEPS - 1),
                                        lambda tt=tt: proj_tile(tt, False)))

            yu_tiles = {}

            def make_step(hp, j, i, last_group):
                """Returns (sc_thunk, yu_thunk) for one i step."""
                ni = 4 * j + 4
                fb = hp * T
                d0 = max(128 * (i - 4 * j), 0)
                box = {}

                def sc_emit():
                    if i == 0:
                        yu_tiles[(hp, j)] = pyu.tile(
                            [128, 642], f32, tag="yu", name=f"yu_{hp}_{j}")
                    sc = psc.tile([128, 1024], f32, tag="sc",
                                  name=f"sc_{hp}_{j}_{i}")
                    for half in (0, 1):
                        po = 64 * half
                        nc.tensor.matmul(
                            sc[:, half * 512 + d0:(half + 1) * 512],
                            kT[po:po + 64, fb + i * 128: fb + (i + 1) * 128],
                            qT[po:po + 64,
                               fb + j * 512 + d0: fb + (j + 1) * 512],
                            start=True, stop=True,
                        )
                    et = epool.tile([128, 1024], bf16, tag="exp",
                                    name=f"et_{hp}_{j}_{i}")
                    nc.scalar.activation(
                        et[:, :].rearrange("p (g q) -> p g q", g=2)[:, :, d0:512],
                        sc[:, :].rearrange("p (g q) -> p g q", g=2)[:, :, d0:512],
                        Exp, scale=float(SCALE / (WSC * WSC)),
                    )
                    if i >= 4 * j:
                        for half in (0, 1):
                            sl = slice(half * 512 + d0, half * 512 + d0 + 128)
                            nc.vector.tensor_mul(et[:, sl], et[:, sl],
                                                 mlo[:, :])
                    box["et"] = et

                def yu_emit():
                    # start=True clears has_written for the WHOLE PSUM bank,
                    # so only the first matmul into each bank of the yu tile
                    # may carry it; later subtiles first-write on cleared
                    # bits (overwrite) with start=False.
                    yu, et = yu_tiles[(hp, j)], box["et"]
                    for tl in range(4):
                        if 128 * tl < d0:
                            continue
                        for half in (0, 1):
                            h = 2 * hp + half
                            nc.tensor.matmul(
                                yu[:, YOFF[tl] + 65 * half:
                                   YOFF[tl] + 65 * half + 65],
                                et[:, half * 512 + tl * 128:
                                   half * 512 + (tl + 1) * 128],
                                V[:, i * VSTR + 65 * h:
                                  i * VSTR + 65 * h + 65],
                                start=(i == 0 and half == 0 and tl in (0, 3)),
                                stop=(i == 4 * j + tl),
                                skip_group_check=True,
                            )
                    if i >= 4 * j:
                        finish_tile(hp, j, i - 4 * j, yu, last_group)

                return sc_emit, yu_emit

            steps = []
            for hp in range(2):
                for j in range(NJ):
                    for i in range(4 * j + 4):
                        steps.append(make_step(hp, j, i,
                                               hp == 1 and j == NJ - 1))
            # flat software pipeline: sc(i+1) is emitted before yu(i), incl.
            # across group boundaries, so the PE never heads-of-line on exp
            from collections import deque
            inflight = deque()
            for sc_emit, yu_emit in steps:
                sc_emit()
                if len(inflight) == 2:
                    inflight.popleft()()
                inflight.append(yu_emit)
                tick()
            while inflight:
                inflight.popleft()()
                tick()
            while filler:
                filler.pop(0)[1]()
            while pending:
                pending.pop(0)[1]()
            if dbg:
                for n, sb in (("qT", qT), ("kT", kT), ("yT", yT)):
                    nc.sync.dma_start(out=dbg_d[n][:, :], in_=sb[:, :])
                nc.sync.dma_start(out=dbgv_d[:, :], in_=V[:, :])

    nc.compile()
    return nc


def get_program():
    if "nc" not in _CACHE:
        _CACHE["nc"] = _build_program()
    return _CACHE["nc"]


def _pack_cmajor(a):
    """[C_rows, N] -> [128, (C_rows/128)*N] with chunk c at [:, c*N:(c+1)*N]."""
    rows, n = a.shape
    return np.ascontiguousarray(
        a.reshape(rows // 128, 128, n).transpose(1, 0, 2).reshape(128, -1))


def make_in_maps(x, W_attn, b_attn, W_proj):
    """Host-side sharding: per-core input dict."""
    x = np.asarray(x, np.float32)
    W_attn = np.asarray(W_attn, np.float32) * WSC
    b_attn = np.asarray(b_attn, np.float32) * WSC
    W_proj = np.asarray(W_proj, np.float32) / WSC

    mlo = (np.arange(128)[None, :] >= np.arange(128)[:, None]).astype(BF16)

    x8_b, xr_b = [], []
    for b in range(B):
        xt = x[b].T.astype(np.float32)                    # [C, T]
        x8 = xt.astype(F8)
        xres = (xt - x8.astype(np.float32)).astype(F8)
        x8_b.append(_pack_cmajor(x8))
        xr_b.append(_pack_cmajor(xres))

    in_maps = []
    for g in range(N_CORES):
        b, hg = divmod(g, 4)
        cs = slice(hg * DL, (hg + 1) * DL)
        m = {"x8p": x8_b[b], "xrp": xr_b[b], "mlo": mlo,
             "eye": np.eye(128).astype(BF16)}
        for ki, key in enumerate("qkv"):
            Wk = W_attn[:, ki * C:(ki + 1) * C][:, cs]
            W8 = Wk.astype(F8)
            Wr = (Wk - W8.astype(np.float32)).astype(F8)
            m[f"w8{key}"] = _pack_cmajor(W8)
            m[f"wr{key}"] = _pack_cmajor(Wr)
        m["wpp"] = _pack_cmajor(W_proj[cs, :].astype(BF16))
        m["bq"] = np.ascontiguousarray(
            b_attn[0 * C:1 * C][cs].reshape(2, 128).T).astype(np.float32)
        m["bk"] = np.ascontiguousarray(
            b_attn[1 * C:2 * C][cs].reshape(2, 128).T).astype(np.float32)
        m["bvr"] = np.ascontiguousarray(
            np.tile(b_attn[2 * C:3 * C][cs][None, :], (128, 1))).astype(np.float32)
        in_maps.append(m)
    return in_maps


def assemble_output(results, b_proj):
    """results: per-core dicts with 'out' [T, C] bf16 partials."""
    b_proj = np.asarray(b_proj, np.float32)
    out = np.zeros((B, T, C), np.float32)
    for g in range(N_CORES):
        out[g // 4] += np.asarray(results[g]["out"], np.float32)
    out += b_proj[None, None, :]
    return out


def kernel(x, W_attn, b_attn, W_proj, b_proj):
    from concourse.bass_utils import run_bass_kernel_spmd

    nc = get_program()
    in_maps = make_in_maps(x, W_attn, b_attn, W_proj)
    res = run_bass_kernel_spmd(nc, in_maps, list(range(N_CORES)))
    return assemble_output(res.results, b_proj)


# revision 56
# speedup vs baseline: 1.3972x; 1.0061x over previous
"""GPT2 causal attention (B=2, T=2048, C=1024, H=16) on 8 TRN2 NeuronCores.

Sharding: core g = (batch b = g//4, head-group hg = g%4 of 4 heads).
Tensor-parallel over heads x data-parallel over batch. Each core emits a
full [T, C] bf16 partial of the output projection for its 4 heads; host
sums the 4 partials per batch and adds b_proj. No collectives.

Per-core kernel:
  QKV projections run in compensated fp8 (e4m3) DoubleRow matmuls:
  x = x8 + xr and W = W8 + Wr host-split (W pre-scaled by 64), with
  x@W ~= x8@W8 + x8@Wr + xr@W8 (error ~ xr@Wr = O(eps^2), below bf16).
  Each DoubleRow matmul contracts 256 rows (a pair of 128-row k-tiles).

  Scores per head pair, transposed: S^T[tk, tq] = kT^T @ qT in bf16,
  causally trimmed to [d0:512] at 128-col granularity (both the matmuls
  and the exp), exp'd on ACT without max subtraction, diagonal 128-blocks
  masked with a host tri mask on DVE.

  attention*V runs in the natural orientation: per 128-row tq tile,
  yu[tq, (h, 65)] = et_chunk^T @ V_aug accumulates over tk tiles in PSUM;
  V_aug carries a ones-column per head so column 64 of each head group is
  the softmax row-sum, landing on the free dim. Normalization is then
  per-partition: gpsimd normalize_recip (attn library) divides by the sum
  and writes bf16. y is transposed back to [d, t] with XBAR DMA
  transposes for the bf16 output projection.
"""

import numpy as np
import ml_dtypes

BF16 = ml_dtypes.bfloat16
F8 = ml_dtypes.float8_e4m3

B, T, C, H, D = 2, 2048, 1024, 16, 64
HL = 4          # heads per core
DL = HL * D     # 256 local head dims
N_CORES = 8
NT = T // 128   # 16 t tiles
NJ = T // 512   # 4 tq groups
SCALE = 1.0 / np.sqrt(D)
WSC = 64.0      # host pre-scale on W_attn for fp8 range
VSTR = HL * 65  # V tile col stride (4 heads x (64 d + ones col))
YOFF = (0, 130, 260, 512)  # yu subtile col offsets (none crosses a bank)

_CACHE = {}


def _build_program():
    import concourse.tile as tile
    from concourse import bacc
    from concourse import library_config
    import concourse.mybir as mybir

    f32 = mybir.dt.float32
    bf16 = mybir.dt.bfloat16
    fp8 = mybir.dt.float8e4
    Exp = mybir.ActivationFunctionType.Exp
    DR = mybir.MatmulPerfMode.DoubleRow

    nc = bacc.Bacc("TRN2", target_bir_lowering=False, debug=False)

    x8_d = nc.dram_tensor("x8p", [128, 8 * T], fp8, kind="ExternalInput").ap()
    xr_d = nc.dram_tensor("xrp", [128, 8 * T], fp8, kind="ExternalInput").ap()
    w8_d = {k: nc.dram_tensor(f"w8{k}", [128, 8 * DL], fp8,
                              kind="ExternalInput").ap() for k in "qkv"}
    wr_d = {k: nc.dram_tensor(f"wr{k}", [128, 8 * DL], fp8,
                              kind="ExternalInput").ap() for k in "qkv"}
    wp_d = nc.dram_tensor("wpp", [128, 2 * C], bf16, kind="ExternalInput").ap()
    bq_d = nc.dram_tensor("bq", [128, 2], f32, kind="ExternalInput").ap()
    bk_d = nc.dram_tensor("bk", [128, 2], f32, kind="ExternalInput").ap()
    bvr_d = nc.dram_tensor("bvr", [128, DL], f32, kind="ExternalInput").ap()
    mlo_d = nc.dram_tensor("mlo", [128, 128], bf16, kind="ExternalInput").ap()
    eye_d = nc.dram_tensor("eye", [128, 128], bf16, kind="ExternalInput").ap()
    out_d = nc.dram_tensor("out", [T, C], bf16, kind="ExternalOutput").ap()
    import os
    dbg = os.environ.get("K_DEBUG") == "1"
    if dbg:
        dbg_d = {n: nc.dram_tensor(f"dbg_{n}", [128, 2 * T], bf16,
                                   kind="ExternalOutput").ap()
                 for n in ("qT", "kT", "yT")}
        dbgv_d = nc.dram_tensor("dbg_V", [128, NT * VSTR], bf16,
                                kind="ExternalOutput").ap()
        dbgyn_d = nc.dram_tensor("dbg_yn", [128, 2 * T], bf16,
                                 kind="ExternalOutput").ap()
        dbgstg_d = nc.dram_tensor("dbg_stg", [128, 32 * 130], f32,
                                  kind="ExternalOutput").ap()

    with tile.TileContext(nc) as tc:
        with (
            tc.tile_pool(name="const", bufs=1) as cpool,
            tc.tile_pool(name="exp", bufs=6) as epool,
            tc.tile_pool(name="ystg", bufs=4) as spool,
            tc.tile_pool(name="ynat", bufs=4) as npool,
            tc.tile_pool(name="ostage", bufs=4) as opool,
            tc.tile_pool(name="pssc", bufs=2, space="PSUM") as psc,
            tc.tile_pool(name="psqkv", bufs=2, space="PSUM") as pqkv,
            tc.tile_pool(name="psyu", bufs=1, space="PSUM") as pyu,
        ):
            # ---- persistent SBUF ----
            x8 = cpool.tile([128, 8 * T], fp8, tag="x8")     # chunk c at c*T
            xr = cpool.tile([128, 8 * T], fp8, tag="xr")
            w8 = {k: cpool.tile([128, 8 * DL], fp8, tag=f"w8{k}", name=f"w8{k}")
                  for k in "qkv"}
            wr = {k: cpool.tile([128, 8 * DL], fp8, tag=f"wr{k}", name=f"wr{k}")
                  for k in "qkv"}
            wp = cpool.tile([128, 2 * C], bf16, tag="wp")    # d-chunk dc at dc*C
            bq = cpool.tile([128, 2], f32, tag="bq")
            bk = cpool.tile([128, 2], f32, tag="bk")
            bvr = cpool.tile([128, DL], f32, tag="bvr")
            mlo = cpool.tile([128, 128], bf16, tag="mlo")
            eye = cpool.tile([128, 128], bf16, tag="eye")
            qT = cpool.tile([128, 2 * T], bf16, tag="qT")    # head h: rows 64*(h%2), cols (h//2)*T+t
            kT = cpool.tile([128, 2 * T], bf16, tag="kT")
            yT = cpool.tile([128, 2 * T], bf16, tag="yT")    # d-chunk dc at dc*T
            V = cpool.tile([128, NT * VSTR], bf16, tag="V")  # tile tt, head h at tt*VSTR+65h

            nc.gpsimd.load_library(library_config.attn)

            # ---- input DMAs ----
            # x streams in t-block-major quarters: one 3D-AP DMA per
            # quarter covers that t-range of all 8 c-chunks (512B runs),
            # so the first q/k/V tiles only wait for a quarter of x
            x8v = x8[:, :].rearrange("p (c t) -> p c t", c=8)
            xrv = xr[:, :].rearrange("p (c t) -> p c t", c=8)
            x8dv = x8_d[:, :].rearrange("p (c t) -> p c t", c=8)
            xrdv = xr_d[:, :].rearrange("p (c t) -> p c t", c=8)
            nc.sync.dma_start(out=w8["q"][:, :], in_=w8_d["q"][:, :])
            nc.sync.dma_start(out=x8v[:, :, 0:512], in_=x8dv[:, :, 0:512])
            nc.sync.dma_start(out=bq[:, :], in_=bq_d[:, :])
            nc.sync.dma_start(out=bk[:, :], in_=bk_d[:, :])
            nc.sync.dma_start(out=w8["k"][:, :], in_=w8_d["k"][:, :])
            nc.sync.dma_start(out=xrv[:, :, 0:512], in_=xrdv[:, :, 0:512])
            nc.sync.dma_start(out=wr["q"][:, :], in_=wr_d["q"][:, :])
            nc.sync.dma_start(out=wr["k"][:, :], in_=wr_d["k"][:, :])
            nc.sync.dma_start(out=w8["v"][:, :], in_=w8_d["v"][:, :])
            nc.sync.dma_start(out=wr["v"][:, :], in_=wr_d["v"][:, :])
            nc.sync.dma_start(out=bvr[:, :], in_=bvr_d[:, :])
            for tb in range(1, 4):
                tsl = slice(tb * 512, (tb + 1) * 512)
                nc.sync.dma_start(out=x8v[:, :, tsl], in_=x8dv[:, :, tsl])
                nc.sync.dma_start(out=xrv[:, :, tsl], in_=xrdv[:, :, tsl])
            nc.sync.dma_start(out=mlo[:, :], in_=mlo_d[:, :])
            nc.sync.dma_start(out=eye[:, :], in_=eye_d[:, :])
            nc.sync.dma_start(out=wp[:, :], in_=wp_d[:, :])

            warm = epool.tile([128, 2], bf16, tag="exp", name="warm")
            nc.scalar.activation(warm[:, :], bq[:, :],
                                 Exp, scale=1e-6)

            x8c = x8[:, :].rearrange("p (c t) -> p c t", c=8)
            xrc = xr[:, :].rearrange("p (c t) -> p c t", c=8)
            w8c = {k: w8[k][:, :].rearrange("p (c m) -> p c m", c=8) for k in "qkv"}
            wrc = {k: wr[k][:, :].rearrange("p (c m) -> p c m", c=8) for k in "qkv"}
            TERMS = [(x8c, w8c), (xrc, w8c), (x8c, wrc)]

            def qk_tile(key, dc, ts, pool):
                """One [128d, 512t] q/k projection tile + bias eviction."""
                b_sb, dst = (bq, qT) if key == "q" else (bk, kT)
                ps = pool.tile([128, 512], f32, tag=pool._qkv_tag,
                               name=f"p{key}_{dc}_{ts}")
                msl = slice(dc * 128, (dc + 1) * 128)
                nsl = slice(ts * 512, (ts + 1) * 512)
                for term, (xa, wd) in enumerate(TERMS):
                    for c in range(4):
                        nc.tensor.matmul(
                            ps[:, 0:512],
                            wd[key][:, 2 * c:2 * c + 2, msl],
                            xa[:, 2 * c:2 * c + 2, nsl],
                            start=(term == 0 and c == 0),
                            stop=(term == 2 and c == 3),
                            perf_mode=DR,
                        )
                nc.vector.tensor_scalar_add(
                    dst[:, dc * T + ts * 512: dc * T + (ts + 1) * 512],
                    ps[:, 0:512], b_sb[:, dc:dc + 1])

            def v_tile(tt, pool):
                """One [128t, 256d] V tile (natural layout) + bias eviction."""
                ps = pool.tile([128, 512], f32, tag=pool._qkv_tag,
                               name=f"pv_{tt}")
                tsl = slice(tt * 128, (tt + 1) * 128)
                for term, (xa, wd) in enumerate(TERMS):
                    for c in range(4):
                        nc.tensor.matmul(
                            ps[:, 0:DL],
                            xa[:, 2 * c:2 * c + 2, tsl],
                            wd["v"][:, 2 * c:2 * c + 2, :],
                            start=(term == 0 and c == 0),
                            stop=(term == 2 and c == 3),
                            perf_mode=DR,
                        )
                vdst = V[:, tt * VSTR:(tt + 1) * VSTR].rearrange(
                    "p (h e) -> p h e", h=HL)[:, :, 0:64]
                nc.vector.tensor_add(
                    vdst, ps[:, 0:DL].rearrange("p (h e) -> p h e", h=HL),
                    bvr[:, :].rearrange("p (h e) -> p h e", h=HL))

            psc._qkv_tag = "sc"
            pqkv._qkv_tag = "qkv"

            # ---- startup QKV: only what group (hp0, j0) needs ----
            nc.vector.memset(V[:, :], 1.0)  # ones cols; data overwritten
            # q0/k0 startup tiles, chunk-major interleaved so neither
            # blocks the other's ready matmuls in the PE FIFO
            ps_q = psc.tile([128, 512], f32, tag="sc", name="pq_0_0")
            ps_k = psc.tile([128, 512], f32, tag="sc", name="pk_0_0")
            for term, (xa, wd) in enumerate(TERMS):
                for c in range(4):
                    for key, ps in (("q", ps_q), ("k", ps_k)):
                        nc.tensor.matmul(
                            ps[:, 0:512],
                            wd[key][:, 2 * c:2 * c + 2, 0:128],
                            xa[:, 2 * c:2 * c + 2, 0:512],
                            start=(term == 0 and c == 0),
                            stop=(term == 2 and c == 3),
                            perf_mode=DR,
                        )
            nc.vector.tensor_scalar_add(qT[:, 0:512], ps_q[:, 0:512],
                                        bq[:, 0:1])
            nc.vector.tensor_scalar_add(kT[:, 0:512], ps_k[:, 0:512],
                                        bk[:, 0:1])
            v_tile(0, psc)
            v_tile(1, psc)

            # filler thunks: spread through attention on the pqkv ring, in
            # deadline order — each must be EMITTED before the attention
            # group that consumes it (program order is the dataflow).
            # (deadline_step, thunk): emitted no later than deadline, so
            # every attention read sees its producer earlier in program order
            filler = [
                (1, lambda: v_tile(2, pqkv)),
                (2, lambda: v_tile(3, pqkv)),
                (3, lambda: qk_tile("k", 0, 1, pqkv)),
                (3, lambda: qk_tile("q", 0, 1, pqkv)),
                (5, lambda: v_tile(4, pqkv)),
                (6, lambda: v_tile(5, pqkv)),
                (7, lambda: v_tile(6, pqkv)),
                (8, lambda: v_tile(7, pqkv)),
                (10, lambda: qk_tile("k", 0, 2, pqkv)),
                (10, lambda: qk_tile("q", 0, 2, pqkv)),
                (13, lambda: v_tile(8, pqkv)),
                (15, lambda: v_tile(9, pqkv)),
                (17, lambda: v_tile(10, pqkv)),
                (19, lambda: v_tile(11, pqkv)),
                (21, lambda: qk_tile("k", 0, 3, pqkv)),
                (21, lambda: qk_tile("q", 0, 3, pqkv)),
                (24, lambda: v_tile(12, pqkv)),
                (27, lambda: v_tile(13, pqkv)),
                (30, lambda: v_tile(14, pqkv)),
                (33, lambda: v_tile(15, pqkv)),
                (36, lambda: qk_tile("k", 1, 0, pqkv)),
                (37, lambda: qk_tile("q", 1, 0, pqkv)),
                (38, lambda: qk_tile("k", 1, 1, pqkv)),
                (39, lambda: qk_tile("q", 1, 1, pqkv)),
                (40, lambda: qk_tile("k", 1, 2, pqkv)),
                (41, lambda: qk_tile("q", 1, 2, pqkv)),
                (42, lambda: qk_tile("k", 1, 3, pqkv)),
                (43, lambda: qk_tile("q", 1, 3, pqkv)),
            ]

            pending = []   # (ready_step, thunk) deferred PE work
            step = [0]
            NSTEPS = 80    # total attention i steps; clamp deferrals

            def tick():
                step[0] += 1
                while filler and filler[0][0] <= step[0]:
                    filler.pop(0)[1]()
                while pending and pending[0][0] <= step[0]:
                    pending.pop(0)[1]()

            def proj_tile(tt, last):
                """Output projection for one 128-row t tile: [128, 1024]."""
                ot = opool.tile([128, 1024], bf16, tag="ot", name=f"ot_{tt}")
                for cc in range(2):
                    pp = pqkv.tile([128, 512], f32, tag="qkv",
                                   name=f"po_{tt}_{cc}")
                    for dc in range(2):
                        nc.tensor.matmul(
                            pp[:, :],
                            yT[:, dc * T + tt * 128: dc * T + (tt + 1) * 128],
                            wp[:, dc * C + cc * 512: dc * C + (cc + 1) * 512],
                            start=(dc == 0), stop=(dc == 1),
                        )
                    nc.vector.tensor_copy(ot[:, cc * 512:(cc + 1) * 512],
                                          pp[:, :])
                    if last:
                        nc.sync.dma_start(
                            out=out_d[tt * 128:(tt + 1) * 128,
                                      cc * 512:(cc + 1) * 512],
                            in_=ot[:, cc * 512:(cc + 1) * 512])

                def out_dma(tt=tt, ot=ot):
                    nc.sync.dma_start(
                        out=out_d[tt * 128:(tt + 1) * 128, :], in_=ot[:, :])
                if last:
                    pass  # halves DMA'd inline below
                else:
                    pending.append((min(step[0] + 2, NSTEPS - 1), out_dma))

            def finish_tile(hp, j, tl, yu, last_group):
                """After tile tl's diagonal yu: stage, normalize, transpose,
                and (in the hp1 phase) queue the output projection."""
                tt = 4 * j + tl
                stg = spool.tile([128, 130], f32, tag="stg",
                                 name=f"stg_{hp}_{tt}")
                nc.vector.tensor_copy(stg[:, :],
                                      yu[:, YOFF[tl]:YOFF[tl] + 130])
                if dbg:
                    nc.sync.dma_start(
                        out=dbgstg_d[:, (hp * NT + tt) * 130:
                                     (hp * NT + tt + 1) * 130],
                        in_=stg[:, :])
                yn = npool.tile([128, 128], bf16, tag="yn",
                                name=f"yn_{hp}_{tt}")
                for lh in range(2):
                    nc.gpsimd.normalize_recip(
                        yn[:, lh * 64:(lh + 1) * 64],
                        stg[:, lh * 65: lh * 65 + 64],
                        stg[:, lh * 65 + 64: lh * 65 + 65],
                    )
                if dbg:
                    nc.sync.dma_start(
                        out=dbgyn_d[:, hp * T + tt * 128:
                                    hp * T + (tt + 1) * 128],
                        in_=yn[:, :])

                def transp(hp=hp, tt=tt, yn=yn):
                    nc.sync.dma_start(
                        out=yT[:, hp * T + tt * 128: hp * T + (tt + 1) * 128],
                        in_=yn[:, :], transpose=True)
                tail = last_group and tl == 3
                if last_group:
                    # tail tiles: PE transpose (~0.6us) instead of the XBAR
                    # DMA path (~1.9us of DGE+sem latency); the filler ring
                    # is quiet by now, the score ring after the last score
                    pool_, tag_ = (psc, "sc") if tail else (pqkv, "qkv")
                    pt = pool_.tile([128, 128], bf16, tag=tag_,
                                    name=f"pt_{hp}_{tt}")
                    nc.tensor.transpose(pt[:, :], yn[:, :], eye[:, :])
                    nc.vector.tensor_copy(
                        yT[:, hp * T + tt * 128: hp * T + (tt + 1) * 128],
                        pt[:, :])
                    if tail:
                        proj_tile(tt, True)
                    else:
                        pending.append((min(step[0] + 2, NSTEPS - 1),
                                        lambda tt=tt: proj_tile(tt, True)))
                else:
                    pending.append((min(step[0] + 2, NSTEPS - 2), transp))
                    if hp == 1:
                        pending.append((min(step[0] + 5, NSTEPS - 1),
                                        lambda tt=tt: proj_tile(tt, False)))

            yu_tiles = {}

            def make_step(hp, j, i, last_group):
                """Returns (sc_thunk, yu_thunk) for one i step."""
                ni = 4 * j + 4
                fb = hp * T
                d0 = max(128 * (i - 4 * j), 0)
                box = {}

                def sc_emit():
                    if i == 0:
                        yu_tiles[(hp, j)] = pyu.tile(
                            [128, 642], f32, tag="yu", name=f"yu_{hp}_{j}")
                    sc = psc.tile([128, 1024], f32, tag="sc",
                                  name=f"sc_{hp}_{j}_{i}")
                    for half in (0, 1):
                        po = 64 * half
                        nc.tensor.matmul(
                            sc[:, half * 512 + d0:(half + 1) * 512],
                            kT[po:po + 64, fb + i * 128: fb + (i + 1) * 128],
                            qT[po:po + 64,
                               fb + j * 512 + d0: fb + (j + 1) * 512],
                            start=True, stop=True,
                        )
                    et = epool.tile([128, 1024], bf16, tag="exp",
                                    name=f"et_{hp}_{j}_{i}")
                    nc.scalar.activation(
                        et[:, :].rearrange("p (g q) -> p g q", g=2)[:, :, d0:512],
                        sc[:, :].rearrange("p (g q) -> p g q", g=2)[:, :, d0:512],
                        Exp, scale=float(SCALE / (WSC * WSC)),
                    )
                    if i >= 4 * j:
                        for half in (0, 1):
                            sl = slice(half * 512 + d0, half * 512 + d0 + 128)
                            nc.vector.tensor_mul(et[:, sl], et[:, sl],
                                                 mlo[:, :])
                    box["et"] = et

                def yu_emit():
                    # start=True clears has_written for the WHOLE PSUM bank,
                    # so only the first matmul into each bank of the yu tile
                    # may carry it; later subtiles first-write on cleared
                    # bits (overwrite) with start=False.
                    yu, et = yu_tiles[(hp, j)], box["et"]
                    for tl in range(4):
                        if 128 * tl < d0:
                            continue
                        for half in (0, 1):
                            h = 2 * hp + half
                            nc.tensor.matmul(
                                yu[:, YOFF[tl] + 65 * half:
                                   YOFF[tl] + 65 * half + 65],
                                et[:, half * 512 + tl * 128:
                                   half * 512 + (tl + 1) * 128],
                                V[:, i * VSTR + 65 * h:
                                  i * VSTR + 65 * h + 65],
                                start=(i == 0 and half == 0 and tl in (0, 3)),
                                stop=(i == 4 * j + tl),
                                skip_group_check=True,
                            )
                    if i >= 4 * j:
                        finish_tile(hp, j, i - 4 * j, yu, last_group)

                return sc_emit, yu_emit

            steps = []
            for hp in range(2):
                for j in range(NJ):
                    for i in range(4 * j + 4):
                        steps.append(make_step(hp, j, i,
                                               hp == 1 and j == NJ - 1))
            # flat software pipeline: sc(i+1) is emitted before yu(i), incl.
            # across group boundaries, so the PE never heads-of-line on exp
            from collections import deque
            inflight = deque()
            for sc_emit, yu_emit in steps:
                sc_emit()
                if len(inflight) == 2:
                    inflight.popleft()()
                inflight.append(yu_emit)
                tick()
            while inflight:
                inflight.popleft()()
                tick()
            while filler:
                filler.pop(0)[1]()
            while pending:
                pending.pop(0)[1]()
            if dbg:
                for n, sb in (("qT", qT), ("kT", kT), ("yT", yT)):
                    nc.sync.dma_start(out=dbg_d[n][:, :], in_=sb[:, :])
                nc.sync.dma_start(out=dbgv_d[:, :], in_=V[:, :])

    nc.compile()
    return nc


def get_program():
    if "nc" not in _CACHE:
        _CACHE["nc"] = _build_program()
    return _CACHE["nc"]


def _pack_cmajor(a):
    """[C_rows, N] -> [128, (C_rows/128)*N] with chunk c at [:, c*N:(c+1)*N]."""
    rows, n = a.shape
    return np.ascontiguousarray(
        a.reshape(rows // 128, 128, n).transpose(1, 0, 2).reshape(128, -1))


def make_in_maps(x, W_attn, b_attn, W_proj):
    """Host-side sharding: per-core input dict."""
    x = np.asarray(x, np.float32)
    W_attn = np.asarray(W_attn, np.float32) * WSC
    b_attn = np.asarray(b_attn, np.float32) * WSC
    W_proj = np.asarray(W_proj, np.float32) / WSC

    mlo = (np.arange(128)[None, :] >= np.arange(128)[:, None]).astype(BF16)

    x8_b, xr_b = [], []
    for b in range(B):
        xt = x[b].T.astype(np.float32)                    # [C, T]
        x8 = xt.astype(F8)
        xres = (xt - x8.astype(np.float32)).astype(F8)
        x8_b.append(_pack_cmajor(x8))
        xr_b.append(_pack_cmajor(xres))

    in_maps = []
    for g in range(N_CORES):
        b, hg = divmod(g, 4)
        cs = slice(hg * DL, (hg + 1) * DL)
        m = {"x8p": x8_b[b], "xrp": xr_b[b], "mlo": mlo,
             "eye": np.eye(128).astype(BF16)}
        for ki, key in enumerate("qkv"):
            Wk = W_attn[:, ki * C:(ki + 1) * C][:, cs]
            W8 = Wk.astype(F8)
            Wr = (Wk - W8.astype(np.float32)).astype(F8)
            m[f"w8{key}"] = _pack_cmajor(W8)
            m[f"wr{key}"] = _pack_cmajor(Wr)
        m["wpp"] = _pack_cmajor(W_proj[cs, :].astype(BF16))
        m["bq"] = np.ascontiguousarray(
            b_attn[0 * C:1 * C][cs].reshape(2, 128).T).astype(np.float32)
        m["bk"] = np.ascontiguousarray(
            b_attn[1 * C:2 * C][cs].reshape(2, 128).T).astype(np.float32)
        m["bvr"] = np.ascontiguousarray(
            np.tile(b_attn[2 * C:3 * C][cs][None, :], (128, 1))).astype(np.float32)
        in_maps.append(m)
    return in_maps


def assemble_output(results, b_proj):
    """results: per-core dicts with 'out' [T, C] bf16 partials."""
    b_proj = np.asarray(b_proj, np.float32)
    out = np.zeros((B, T, C), np.float32)
    for g in range(N_CORES):
        out[g // 4] += np.asarray(results[g]["out"], np.float32)
    out += b_proj[None, None, :]
    return out


def kernel(x, W_attn, b_attn, W_proj, b_proj):
    from concourse.bass_utils import run_bass_kernel_spmd

    nc = get_program()
    in_maps = make_in_maps(x, W_attn, b_attn, W_proj)
    res = run_bass_kernel_spmd(nc, in_maps, list(range(N_CORES)))
    return assemble_output(res.results, b_proj)


# revision 60
# speedup vs baseline: 1.4077x; 1.0075x over previous
"""GPT2 causal attention (B=2, T=2048, C=1024, H=16) on 8 TRN2 NeuronCores.

Sharding: core g = (batch b = g//4, head-group hg = g%4 of 4 heads).
Tensor-parallel over heads x data-parallel over batch. Each core emits a
full [T, C] bf16 partial of the output projection for its 4 heads; host
sums the 4 partials per batch and adds b_proj. No collectives.

Per-core kernel:
  QKV projections run in compensated fp8 (e4m3) DoubleRow matmuls:
  x = x8 + xr and W = W8 + Wr host-split (W pre-scaled by 64), with
  x@W ~= x8@W8 + x8@Wr + xr@W8 (error ~ xr@Wr = O(eps^2), below bf16).
  Each DoubleRow matmul contracts 256 rows (a pair of 128-row k-tiles).

  Scores per head pair, transposed: S^T[tk, tq] = kT^T @ qT in bf16,
  causally trimmed to [d0:512] at 128-col granularity (both the matmuls
  and the exp), exp'd on ACT without max subtraction, diagonal 128-blocks
  masked with a host tri mask on DVE.

  attention*V runs in the natural orientation: per 128-row tq tile,
  yu[tq, (h, 65)] = et_chunk^T @ V_aug accumulates over tk tiles in PSUM;
  V_aug carries a ones-column per head so column 64 of each head group is
  the softmax row-sum, landing on the free dim. Normalization is then
  per-partition: gpsimd normalize_recip (attn library) divides by the sum
  and writes bf16. y is transposed back to [d, t] with XBAR DMA
  transposes for the bf16 output projection.
"""

import numpy as np
import ml_dtypes

BF16 = ml_dtypes.bfloat16
F8 = ml_dtypes.float8_e4m3

B, T, C, H, D = 2, 2048, 1024, 16, 64
HL = 4          # heads per core
DL = HL * D     # 256 local head dims
N_CORES = 8
NT = T // 128   # 16 t tiles
NJ = T // 512   # 4 tq groups
SCALE = 1.0 / np.sqrt(D)
WSC = 64.0      # host pre-scale on W_attn for fp8 range
VSTR = HL * 65  # V tile col stride (4 heads x (64 d + ones col))
YOFF = (0, 130, 260, 512)  # yu subtile col offsets (none crosses a bank)

_CACHE = {}


def _build_program():
    import concourse.tile as tile
    from concourse import bacc
    from concourse import library_config
    import concourse.mybir as mybir

    f32 = mybir.dt.float32
    bf16 = mybir.dt.bfloat16
    fp8 = mybir.dt.float8e4
    Exp = mybir.ActivationFunctionType.Exp
    DR = mybir.MatmulPerfMode.DoubleRow

    nc = bacc.Bacc("TRN2", target_bir_lowering=False, debug=False)

    x8_d = nc.dram_tensor("x8p", [128, 8 * T], fp8, kind="ExternalInput").ap()
    xr_d = nc.dram_tensor("xrp", [128, 8 * T], fp8, kind="ExternalInput").ap()
    w8_d = {k: nc.dram_tensor(f"w8{k}", [128, 8 * DL], fp8,
                              kind="ExternalInput").ap() for k in "qkv"}
    wr_d = {k: nc.dram_tensor(f"wr{k}", [128, 8 * DL], fp8,
                              kind="ExternalInput").ap() for k in "qkv"}
    wp_d = nc.dram_tensor("wpp", [128, 2 * C], bf16, kind="ExternalInput").ap()
    bq_d = nc.dram_tensor("bq", [128, 2], f32, kind="ExternalInput").ap()
    bk_d = nc.dram_tensor("bk", [128, 2], f32, kind="ExternalInput").ap()
    bvr_d = nc.dram_tensor("bvr", [128, DL], f32, kind="ExternalInput").ap()
    mlo_d = nc.dram_tensor("mlo", [128, 128], bf16, kind="ExternalInput").ap()
    eye_d = nc.dram_tensor("eye", [128, 128], bf16, kind="ExternalInput").ap()
    out_d = nc.dram_tensor("out", [T, C], bf16, kind="ExternalOutput").ap()
    import os
    dbg = os.environ.get("K_DEBUG") == "1"
    if dbg:
        dbg_d = {n: nc.dram_tensor(f"dbg_{n}", [128, 2 * T], bf16,
                                   kind="ExternalOutput").ap()
                 for n in ("qT", "kT", "yT")}
        dbgv_d = nc.dram_tensor("dbg_V", [128, NT * VSTR], bf16,
                                kind="ExternalOutput").ap()
        dbgyn_d = nc.dram_tensor("dbg_yn", [128, 2 * T], bf16,
                                 kind="ExternalOutput").ap()
        dbgstg_d = nc.dram_tensor("dbg_stg", [128, 32 * 130], f32,
                                  kind="ExternalOutput").ap()

    with tile.TileContext(nc) as tc:
        with (
            tc.tile_pool(name="const", bufs=1) as cpool,
            tc.tile_pool(name="exp", bufs=6) as epool,
            tc.tile_pool(name="ystg", bufs=6) as spool,
            tc.tile_pool(name="ynat", bufs=6) as npool,
            tc.tile_pool(name="ostage", bufs=6) as opool,
            tc.tile_pool(name="pssc", bufs=2, space="PSUM") as psc,
            tc.tile_pool(name="psqkv", bufs=2, space="PSUM") as pqkv,
            tc.tile_pool(name="psyu", bufs=1, space="PSUM") as pyu,
        ):
            # ---- persistent SBUF ----
            x8 = cpool.tile([128, 8 * T], fp8, tag="x8")     # chunk c at c*T
            xr = cpool.tile([128, 8 * T], fp8, tag="xr")
            w8 = {k: cpool.tile([128, 8 * DL], fp8, tag=f"w8{k}", name=f"w8{k}")
                  for k in "qkv"}
            wr = {k: cpool.tile([128, 8 * DL], fp8, tag=f"wr{k}", name=f"wr{k}")
                  for k in "qkv"}
            wp = cpool.tile([128, 2 * C], bf16, tag="wp")    # d-chunk dc at dc*C
            bq = cpool.tile([128, 2], f32, tag="bq")
            bk = cpool.tile([128, 2], f32, tag="bk")
            bvr = cpool.tile([128, DL], f32, tag="bvr")
            mlo = cpool.tile([128, 128], bf16, tag="mlo")
            eye = cpool.tile([128, 128], bf16, tag="eye")
            qT = cpool.tile([128, 2 * T], bf16, tag="qT")    # head h: rows 64*(h%2), cols (h//2)*T+t
            kT = cpool.tile([128, 2 * T], bf16, tag="kT")
            yT = cpool.tile([128, 2 * T], bf16, tag="yT")    # d-chunk dc at dc*T
            V = cpool.tile([128, NT * VSTR], bf16, tag="V")  # tile tt, head h at tt*VSTR+65h

            nc.gpsimd.load_library(library_config.attn)

            # ---- input DMAs ----
            # x streams in t-block-major quarters: one 3D-AP DMA per
            # quarter covers that t-range of all 8 c-chunks (512B runs),
            # so the first q/k/V tiles only wait for a quarter of x
            x8v = x8[:, :].rearrange("p (c t) -> p c t", c=8)
            xrv = xr[:, :].rearrange("p (c t) -> p c t", c=8)
            x8dv = x8_d[:, :].rearrange("p (c t) -> p c t", c=8)
            xrdv = xr_d[:, :].rearrange("p (c t) -> p c t", c=8)
            nc.sync.dma_start(out=w8["q"][:, :], in_=w8_d["q"][:, :])
            nc.sync.dma_start(out=x8v[:, :, 0:512], in_=x8dv[:, :, 0:512])
            nc.sync.dma_start(out=bq[:, :], in_=bq_d[:, :])
            nc.sync.dma_start(out=bk[:, :], in_=bk_d[:, :])
            nc.sync.dma_start(out=w8["k"][:, :], in_=w8_d["k"][:, :])
            nc.sync.dma_start(out=xrv[:, :, 0:512], in_=xrdv[:, :, 0:512])
            nc.sync.dma_start(out=wr["q"][:, :], in_=wr_d["q"][:, :])
            nc.sync.dma_start(out=wr["k"][:, :], in_=wr_d["k"][:, :])
            nc.sync.dma_start(out=w8["v"][:, :], in_=w8_d["v"][:, :])
            nc.sync.dma_start(out=wr["v"][:, :], in_=wr_d["v"][:, :])
            nc.sync.dma_start(out=bvr[:, :], in_=bvr_d[:, :])
            for tb in range(1, 4):
                tsl = slice(tb * 512, (tb + 1) * 512)
                nc.sync.dma_start(out=x8v[:, :, tsl], in_=x8dv[:, :, tsl])
                nc.sync.dma_start(out=xrv[:, :, tsl], in_=xrdv[:, :, tsl])
            nc.sync.dma_start(out=mlo[:, :], in_=mlo_d[:, :])
            nc.sync.dma_start(out=eye[:, :], in_=eye_d[:, :])
            nc.sync.dma_start(out=wp[:, :], in_=wp_d[:, :])

            warm = epool.tile([128, 2], bf16, tag="exp", name="warm")
            nc.scalar.activation(warm[:, :], bq[:, :],
                                 Exp, scale=1e-6)

            x8c = x8[:, :].rearrange("p (c t) -> p c t", c=8)
            xrc = xr[:, :].rearrange("p (c t) -> p c t", c=8)
            w8c = {k: w8[k][:, :].rearrange("p (c m) -> p c m", c=8) for k in "qkv"}
            wrc = {k: wr[k][:, :].rearrange("p (c m) -> p c m", c=8) for k in "qkv"}
            TERMS = [(x8c, w8c), (xrc, w8c), (x8c, wrc)]

            def qk_tile(key, dc, ts, pool):
                """One [128d, 512t] q/k projection tile + bias eviction."""
                b_sb, dst = (bq, qT) if key == "q" else (bk, kT)
                ps = pool.tile([128, 512], f32, tag=pool._qkv_tag,
                               name=f"p{key}_{dc}_{ts}")
                msl = slice(dc * 128, (dc + 1) * 128)
                nsl = slice(ts * 512, (ts + 1) * 512)
                for term, (xa, wd) in enumerate(TERMS):
                    for c in range(4):
                        nc.tensor.matmul(
                            ps[:, 0:512],
                            wd[key][:, 2 * c:2 * c + 2, msl],
                            xa[:, 2 * c:2 * c + 2, nsl],
                            start=(term == 0 and c == 0),
                            stop=(term == 2 and c == 3),
                            perf_mode=DR,
                        )
                nc.vector.tensor_scalar_add(
                    dst[:, dc * T + ts * 512: dc * T + (ts + 1) * 512],
                    ps[:, 0:512], b_sb[:, dc:dc + 1])

            def v_tile(tt, pool):
                """One [128t, 256d] V tile (natural layout) + bias eviction."""
                ps = pool.tile([128, 512], f32, tag=pool._qkv_tag,
                               name=f"pv_{tt}")
                tsl = slice(tt * 128, (tt + 1) * 128)
                for term, (xa, wd) in enumerate(TERMS):
                    for c in range(4):
                        nc.tensor.matmul(
                            ps[:, 0:DL],
                            xa[:, 2 * c:2 * c + 2, tsl],
                            wd["v"][:, 2 * c:2 * c + 2, :],
                            start=(term == 0 and c == 0),
                            stop=(term == 2 and c == 3),
                            perf_mode=DR,
                        )
                vdst = V[:, tt * VSTR:(tt + 1) * VSTR].rearrange(
                    "p (h e) -> p h e", h=HL)[:, :, 0:64]
                nc.vector.tensor_add(
                    vdst, ps[:, 0:DL].rearrange("p (h e) -> p h e", h=HL),
                    bvr[:, :].rearrange("p (h e) -> p h e", h=HL))

            psc._qkv_tag = "sc"
            pqkv._qkv_tag = "qkv"

            # ---- startup QKV: only what group (hp0, j0) needs ----
            nc.vector.memset(V[:, :], 1.0)  # ones cols; data overwritten
            # q0/k0 startup tiles, chunk-major interleaved so neither
            # blocks the other's ready matmuls in the PE FIFO
            ps_q = psc.tile([128, 512], f32, tag="sc", name="pq_0_0")
            ps_k = psc.tile([128, 512], f32, tag="sc", name="pk_0_0")
            for term, (xa, wd) in enumerate(TERMS):
                for c in range(4):
                    for key, ps in (("q", ps_q), ("k", ps_k)):
                        nc.tensor.matmul(
                            ps[:, 0:512],
                            wd[key][:, 2 * c:2 * c + 2, 0:128],
                            xa[:, 2 * c:2 * c + 2, 0:512],
                            start=(term == 0 and c == 0),
                            stop=(term == 2 and c == 3),
                            perf_mode=DR,
                        )
            nc.vector.tensor_scalar_add(qT[:, 0:512], ps_q[:, 0:512],
                                        bq[:, 0:1])
            nc.vector.tensor_scalar_add(kT[:, 0:512], ps_k[:, 0:512],
                                        bk[:, 0:1])
            v_tile(0, psc)
            v_tile(1, psc)

            # filler thunks: spread through attention on the pqkv ring, in
            # deadline order — each must be EMITTED before the attention
            # group that consumes it (program order is the dataflow).
            # (deadline_step, thunk): emitted no later than deadline, so
            # every attention read sees its producer earlier in program order
            filler = [
                (1, lambda: v_tile(2, pqkv)),
                (2, lambda: v_tile(3, pqkv)),
                (3, lambda: qk_tile("k", 0, 1, pqkv)),
                (3, lambda: qk_tile("q", 0, 1, pqkv)),
                (5, lambda: v_tile(4, pqkv)),
                (6, lambda: v_tile(5, pqkv)),
                (7, lambda: v_tile(6, pqkv)),
                (8, lambda: v_tile(7, pqkv)),
                (10, lambda: qk_tile("k", 0, 2, pqkv)),
                (10, lambda: qk_tile("q", 0, 2, pqkv)),
                (13, lambda: v_tile(8, pqkv)),
                (15, lambda: v_tile(9, pqkv)),
                (17, lambda: v_tile(10, pqkv)),
                (19, lambda: v_tile(11, pqkv)),
                (21, lambda: qk_tile("k", 0, 3, pqkv)),
                (21, lambda: qk_tile("q", 0, 3, pqkv)),
                (24, lambda: v_tile(12, pqkv)),
                (27, lambda: v_tile(13, pqkv)),
                (30, lambda: v_tile(14, pqkv)),
                (33, lambda: v_tile(15, pqkv)),
                (36, lambda: qk_tile("k", 1, 0, pqkv)),
                (37, lambda: qk_tile("q", 1, 0, pqkv)),
                (38, lambda: qk_tile("k", 1, 1, pqkv)),
                (39, lambda: qk_tile("q", 1, 1, pqkv)),
                (40, lambda: qk_tile("k", 1, 2, pqkv)),
                (41, lambda: qk_tile("q", 1, 2, pqkv)),
                (42, lambda: qk_tile("k", 1, 3, pqkv)),
                (43, lambda: qk_tile("q", 1, 3, pqkv)),
            ]

            pending = []   # (ready_step, thunk) deferred PE work
            step = [0]
            NSTEPS = 80    # total attention i steps; clamp deferrals

            def tick():
                step[0] += 1
                while filler and filler[0][0] <= step[0]:
                    filler.pop(0)[1]()
                while pending and pending[0][0] <= step[0]:
                    pending.pop(0)[1]()

            def proj_tile(tt, last):
                """Output projection for one 128-row t tile: [128, 1024]."""
                ot = opool.tile([128, 1024], bf16, tag="ot", name=f"ot_{tt}")
                for cc in range(2):
                    pp = pqkv.tile([128, 512], f32, tag="qkv",
                                   name=f"po_{tt}_{cc}")
                    for dc in range(2):
                        nc.tensor.matmul(
                            pp[:, :],
                            yT[:, dc * T + tt * 128: dc * T + (tt + 1) * 128],
                            wp[:, dc * C + cc * 512: dc * C + (cc + 1) * 512],
                            start=(dc == 0), stop=(dc == 1),
                        )
                    if last and cc == 0:
                        nc.scalar.copy(ot[:, cc * 512:(cc + 1) * 512],
                                       pp[:, :])
                    else:
                        nc.vector.tensor_copy(ot[:, cc * 512:(cc + 1) * 512],
                                              pp[:, :])
                    if last:
                        nc.sync.dma_start(
                            out=out_d[tt * 128:(tt + 1) * 128,
                                      cc * 512:(cc + 1) * 512],
                            in_=ot[:, cc * 512:(cc + 1) * 512])

                def out_dma(tt=tt, ot=ot):
                    nc.sync.dma_start(
                        out=out_d[tt * 128:(tt + 1) * 128, :], in_=ot[:, :])
                if last:
                    pass  # halves DMA'd inline below
                else:
                    pending.append((min(step[0] + 2, NSTEPS - 1), out_dma))

            def finish_tile(hp, j, tl, yu, last_group):
                """After tile tl's diagonal yu: stage, normalize, transpose,
                and (in the hp1 phase) queue the output projection."""
                tt = 4 * j + tl
                stg = spool.tile([128, 130], f32, tag="stg",
                                 name=f"stg_{hp}_{tt}")
                if last_group:
                    nc.scalar.copy(stg[:, :], yu[:, YOFF[tl]:YOFF[tl] + 130])
                else:
                    nc.vector.tensor_copy(stg[:, :],
                                          yu[:, YOFF[tl]:YOFF[tl] + 130])
                if dbg:
                    nc.sync.dma_start(
                        out=dbgstg_d[:, (hp * NT + tt) * 130:
                                     (hp * NT + tt + 1) * 130],
                        in_=stg[:, :])
                yn = npool.tile([128, 128], bf16, tag="yn",
                                name=f"yn_{hp}_{tt}")
                for lh in range(2):
                    nc.gpsimd.normalize_recip(
                        yn[:, lh * 64:(lh + 1) * 64],
                        stg[:, lh * 65: lh * 65 + 64],
                        stg[:, lh * 65 + 64: lh * 65 + 65],
                    )
                if dbg:
                    nc.sync.dma_start(
                        out=dbgyn_d[:, hp * T + tt * 128:
                                    hp * T + (tt + 1) * 128],
                        in_=yn[:, :])

                def transp(hp=hp, tt=tt, yn=yn):
                    nc.sync.dma_start(
                        out=yT[:, hp * T + tt * 128: hp * T + (tt + 1) * 128],
                        in_=yn[:, :], transpose=True)
                tail = last_group and tl == 3
                if last_group:
                    # tail tiles: PE transpose (~0.6us) instead of the XBAR
                    # DMA path (~1.9us of DGE+sem latency); the filler ring
                    # is quiet by now, the score ring after the last score
                    pool_, tag_ = (psc, "sc") if tail else (pqkv, "qkv")
                    pt = pool_.tile([128, 128], bf16, tag=tag_,
                                    name=f"pt_{hp}_{tt}")
                    nc.tensor.transpose(pt[:, :], yn[:, :], eye[:, :])
                    nc.vector.tensor_copy(
                        yT[:, hp * T + tt * 128: hp * T + (tt + 1) * 128],
                        pt[:, :])
                    if tail:
                        proj_tile(tt, True)
                    else:
                        pending.append((min(step[0] + 1, NSTEPS - 1),
                                        lambda tt=tt: proj_tile(tt, True)))
                else:
                    pending.append((min(step[0] + 2, NSTEPS - 2), transp))
                    if hp == 1:
                        pending.append((min(step[0] + 5, NSTEPS - 1),
                                        lambda tt=tt: proj_tile(tt, False)))

            yu_tiles = {}

            def make_step(hp, j, i, last_group):
                """Returns (sc_thunk, yu_thunk) for one i step."""
                ni = 4 * j + 4
                fb = hp * T
                d0 = max(128 * (i - 4 * j), 0)
                box = {}

                def sc_emit():
                    if i == 0:
                        yu_tiles[(hp, j)] = pyu.tile(
                            [128, 642], f32, tag="yu", name=f"yu_{hp}_{j}")
                    sc = psc.tile([128, 1024], f32, tag="sc",
                                  name=f"sc_{hp}_{j}_{i}")
                    for half in (0, 1):
                        po = 64 * half
                        nc.tensor.matmul(
                            sc[:, half * 512 + d0:(half + 1) * 512],
                            kT[po:po + 64, fb + i * 128: fb + (i + 1) * 128],
                            qT[po:po + 64,
                               fb + j * 512 + d0: fb + (j + 1) * 512],
                            start=True, stop=True,
                        )
                    et = epool.tile([128, 1024], bf16, tag="exp",
                                    name=f"et_{hp}_{j}_{i}")
                    nc.scalar.activation(
                        et[:, :].rearrange("p (g q) -> p g q", g=2)[:, :, d0:512],
                        sc[:, :].rearrange("p (g q) -> p g q", g=2)[:, :, d0:512],
                        Exp, scale=float(SCALE / (WSC * WSC)),
                    )
                    if i >= 4 * j:
                        for half in (0, 1):
                            sl = slice(half * 512 + d0, half * 512 + d0 + 128)
                            nc.vector.tensor_mul(et[:, sl], et[:, sl],
                                                 mlo[:, :])
                    box["et"] = et

                def yu_emit():
                    # start=True clears has_written for the WHOLE PSUM bank,
                    # so only the first matmul into each bank of the yu tile
                    # may carry it; later subtiles first-write on cleared
                    # bits (overwrite) with start=False.
                    yu, et = yu_tiles[(hp, j)], box["et"]
                    for tl in range(4):
                        if 128 * tl < d0:
                            continue
                        for half in (0, 1):
                            h = 2 * hp + half
                            nc.tensor.matmul(
                                yu[:, YOFF[tl] + 65 * half:
                                   YOFF[tl] + 65 * half + 65],
                                et[:, half * 512 + tl * 128:
                                   half * 512 + (tl + 1) * 128],
                                V[:, i * VSTR + 65 * h:
                                  i * VSTR + 65 * h + 65],
                                start=(i == 0 and half == 0 and tl in (0, 3)),
                                stop=(i == 4 * j + tl),
                                skip_group_check=True,
                            )
                    if i >= 4 * j:
                        finish_tile(hp, j, i - 4 * j, yu, last_group)

                return sc_emit, yu_emit

            steps = []
            for hp in range(2):
                for j in range(NJ):
                    for i in range(4 * j + 4):
                        steps.append(make_step(hp, j, i,
                                               hp == 1 and j == NJ - 1))
            # flat software pipeline: sc(i+1) is emitted before yu(i), incl.
            # across group boundaries, so the PE never heads-of-line on exp
            from collections import deque
            inflight = deque()
            for sc_emit, yu_emit in steps:
                sc_emit()
                if len(inflight) == 2:
                    inflight.popleft()()
                inflight.append(yu_emit)
                tick()
            while inflight:
                inflight.popleft()()
                tick()
            while filler:
                filler.pop(0)[1]()
            while pending:
                pending.pop(0)[1]()
            if dbg:
                for n, sb in (("qT", qT), ("kT", kT), ("yT", yT)):
                    nc.sync.dma_start(out=dbg_d[n][:, :], in_=sb[:, :])
                nc.sync.dma_start(out=dbgv_d[:, :], in_=V[:, :])

    nc.compile()
    return nc


def get_program():
    if "nc" not in _CACHE:
        _CACHE["nc"] = _build_program()
    return _CACHE["nc"]


def _pack_cmajor(a):
    """[C_rows, N] -> [128, (C_rows/128)*N] with chunk c at [:, c*N:(c+1)*N]."""
    rows, n = a.shape
    return np.ascontiguousarray(
        a.reshape(rows // 128, 128, n).transpose(1, 0, 2).reshape(128, -1))


def make_in_maps(x, W_attn, b_attn, W_proj):
    """Host-side sharding: per-core input dict."""
    x = np.asarray(x, np.float32)
    W_attn = np.asarray(W_attn, np.float32) * WSC
    b_attn = np.asarray(b_attn, np.float32) * WSC
    W_proj = np.asarray(W_proj, np.float32) / WSC

    mlo = (np.arange(128)[None, :] >= np.arange(128)[:, None]).astype(BF16)

    x8_b, xr_b = [], []
    for b in range(B):
        xt = x[b].T.astype(np.float32)                    # [C, T]
        x8 = xt.astype(F8)
        xres = (xt - x8.astype(np.float32)).astype(F8)
        x8_b.append(_pack_cmajor(x8))
        xr_b.append(_pack_cmajor(xres))

    in_maps = []
    for g in range(N_CORES):
        b, hg = divmod(g, 4)
        cs = slice(hg * DL, (hg + 1) * DL)
        m = {"x8p": x8_b[b], "xrp": xr_b[b], "mlo": mlo,
             "eye": np.eye(128).astype(BF16)}
        for ki, key in enumerate("qkv"):
            Wk = W_attn[:, ki * C:(ki + 1) * C][:, cs]
            W8 = Wk.astype(F8)
            Wr = (Wk - W8.astype(np.float32)).astype(F8)
            m[f"w8{key}"] = _pack_cmajor(W8)
            m[f"wr{key}"] = _pack_cmajor(Wr)
        m["wpp"] = _pack_cmajor(W_proj[cs, :].astype(BF16))
        m["bq"] = np.ascontiguousarray(
            b_attn[0 * C:1 * C][cs].reshape(2, 128).T).astype(np.float32)
        m["bk"] = np.ascontiguousarray(
            b_attn[1 * C:2 * C][cs].reshape(2, 128).T).astype(np.float32)
        m["bvr"] = np.ascontiguousarray(
            np.tile(b_attn[2 * C:3 * C][cs][None, :], (128, 1))).astype(np.float32)
        in_maps.append(m)
    return in_maps


def assemble_output(results, b_proj):
    """results: per-core dicts with 'out' [T, C] bf16 partials."""
    b_proj = np.asarray(b_proj, np.float32)
    out = np.zeros((B, T, C), np.float32)
    for g in range(N_CORES):
        out[g // 4] += np.asarray(results[g]["out"], np.float32)
    out += b_proj[None, None, :]
    return out


def kernel(x, W_attn, b_attn, W_proj, b_proj):
    from concourse.bass_utils import run_bass_kernel_spmd

    nc = get_program()
    in_maps = make_in_maps(x, W_attn, b_attn, W_proj)
    res = run_bass_kernel_spmd(nc, in_maps, list(range(N_CORES)))
    return assemble_output(res.results, b_proj)


# revision 66
# speedup vs baseline: 1.4095x; 1.0013x over previous
"""GPT2 causal attention (B=2, T=2048, C=1024, H=16) on 8 TRN2 NeuronCores.

Sharding: core g = (batch b = g//4, head-group hg = g%4 of 4 heads).
Tensor-parallel over heads x data-parallel over batch. Each core emits a
full [T, C] bf16 partial of the output projection for its 4 heads; host
sums the 4 partials per batch and adds b_proj. No collectives.

Per-core kernel:
  QKV projections run in compensated fp8 (e4m3) DoubleRow matmuls:
  x = x8 + xr and W = W8 + Wr host-split (W pre-scaled by 64), with
  x@W ~= x8@W8 + x8@Wr + xr@W8 (error ~ xr@Wr = O(eps^2), below bf16).
  Each DoubleRow matmul contracts 256 rows (a pair of 128-row k-tiles).

  Scores per head pair, transposed: S^T[tk, tq] = kT^T @ qT in bf16,
  causally trimmed to [d0:512] at 128-col granularity (both the matmuls
  and the exp), exp'd on ACT without max subtraction, diagonal 128-blocks
  masked with a host tri mask on DVE.

  attention*V runs in the natural orientation: per 128-row tq tile,
  yu[tq, (h, 65)] = et_chunk^T @ V_aug accumulates over tk tiles in PSUM;
  V_aug carries a ones-column per head so column 64 of each head group is
  the softmax row-sum, landing on the free dim. Normalization is then
  per-partition: gpsimd normalize_recip (attn library) divides by the sum
  and writes bf16. y is transposed back to [d, t] with XBAR DMA
  transposes for the bf16 output projection.
"""

import numpy as np
import ml_dtypes

BF16 = ml_dtypes.bfloat16
F8 = ml_dtypes.float8_e4m3

B, T, C, H, D = 2, 2048, 1024, 16, 64
HL = 4          # heads per core
DL = HL * D     # 256 local head dims
N_CORES = 8
NT = T // 128   # 16 t tiles
NJ = T // 512   # 4 tq groups
SCALE = 1.0 / np.sqrt(D)
WSC = 64.0      # host pre-scale on W_attn for fp8 range
VSTR = HL * 65  # V tile col stride (4 heads x (64 d + ones col))
YOFF = (0, 130, 260, 512)  # yu subtile col offsets (none crosses a bank)

_CACHE = {}


def _build_program():
    import concourse.tile as tile
    from concourse import bacc
    from concourse import library_config
    import concourse.mybir as mybir

    f32 = mybir.dt.float32
    bf16 = mybir.dt.bfloat16
    fp8 = mybir.dt.float8e4
    Exp = mybir.ActivationFunctionType.Exp
    DR = mybir.MatmulPerfMode.DoubleRow

    nc = bacc.Bacc("TRN2", target_bir_lowering=False, debug=False)

    x8_d = nc.dram_tensor("x8p", [128, 8 * T], fp8, kind="ExternalInput").ap()
    xr_d = nc.dram_tensor("xrp", [128, 8 * T], fp8, kind="ExternalInput").ap()
    w8_d = {k: nc.dram_tensor(f"w8{k}", [128, 8 * DL], fp8,
                              kind="ExternalInput").ap() for k in "qkv"}
    wr_d = {k: nc.dram_tensor(f"wr{k}", [128, 8 * DL], fp8,
                              kind="ExternalInput").ap() for k in "qkv"}
    wp_d = nc.dram_tensor("wpp", [128, 2 * C], bf16, kind="ExternalInput").ap()
    bq_d = nc.dram_tensor("bq", [128, 2], f32, kind="ExternalInput").ap()
    bk_d = nc.dram_tensor("bk", [128, 2], f32, kind="ExternalInput").ap()
    bvr_d = nc.dram_tensor("bvr", [128, DL], f32, kind="ExternalInput").ap()
    mlo_d = nc.dram_tensor("mlo", [128, 128], bf16, kind="ExternalInput").ap()
    eye_d = nc.dram_tensor("eye", [128, 128], bf16, kind="ExternalInput").ap()
    out_d = nc.dram_tensor("out", [T, C], bf16, kind="ExternalOutput").ap()
    import os
    dbg = os.environ.get("K_DEBUG") == "1"
    if dbg:
        dbg_d = {n: nc.dram_tensor(f"dbg_{n}", [128, 2 * T], bf16,
                                   kind="ExternalOutput").ap()
                 for n in ("qT", "kT", "yT")}
        dbgv_d = nc.dram_tensor("dbg_V", [128, NT * VSTR], bf16,
                                kind="ExternalOutput").ap()
        dbgyn_d = nc.dram_tensor("dbg_yn", [128, 2 * T], bf16,
                                 kind="ExternalOutput").ap()
        dbgstg_d = nc.dram_tensor("dbg_stg", [128, 32 * 130], f32,
                                  kind="ExternalOutput").ap()

    with tile.TileContext(nc) as tc:
        with (
            tc.tile_pool(name="const", bufs=1) as cpool,
            tc.tile_pool(name="exp", bufs=6) as epool,
            tc.tile_pool(name="ystg", bufs=6) as spool,
            tc.tile_pool(name="ynat", bufs=6) as npool,
            tc.tile_pool(name="ostage", bufs=6) as opool,
            tc.tile_pool(name="pssc", bufs=2, space="PSUM") as psc,
            tc.tile_pool(name="psqkv", bufs=2, space="PSUM") as pqkv,
            tc.tile_pool(name="psyu", bufs=1, space="PSUM") as pyu,
        ):
            # ---- persistent SBUF ----
            x8 = cpool.tile([128, 8 * T], fp8, tag="x8")     # chunk c at c*T
            xr = cpool.tile([128, 8 * T], fp8, tag="xr")
            w8 = {k: cpool.tile([128, 8 * DL], fp8, tag=f"w8{k}", name=f"w8{k}")
                  for k in "qkv"}
            wr = {k: cpool.tile([128, 8 * DL], fp8, tag=f"wr{k}", name=f"wr{k}")
                  for k in "qkv"}
            wp = cpool.tile([128, 2 * C], bf16, tag="wp")    # d-chunk dc at dc*C
            bq = cpool.tile([128, 2], f32, tag="bq")
            bk = cpool.tile([128, 2], f32, tag="bk")
            bvr = cpool.tile([128, DL], f32, tag="bvr")
            mlo = cpool.tile([128, 128], bf16, tag="mlo")
            eye = cpool.tile([128, 128], bf16, tag="eye")
            qT = cpool.tile([128, 2 * T], bf16, tag="qT")    # head h: rows 64*(h%2), cols (h//2)*T+t
            kT = cpool.tile([128, 2 * T], bf16, tag="kT")
            yT = cpool.tile([128, 2 * T], bf16, tag="yT")    # d-chunk dc at dc*T
            V = cpool.tile([128, NT * VSTR], bf16, tag="V")  # tile tt, head h at tt*VSTR+65h

            nc.gpsimd.load_library(library_config.attn)

            # ---- input DMAs ----
            # x streams in t-block-major quarters: one 3D-AP DMA per
            # quarter covers that t-range of all 8 c-chunks (512B runs),
            # so the first q/k/V tiles only wait for a quarter of x
            x8v = x8[:, :].rearrange("p (c t) -> p c t", c=8)
            xrv = xr[:, :].rearrange("p (c t) -> p c t", c=8)
            x8dv = x8_d[:, :].rearrange("p (c t) -> p c t", c=8)
            xrdv = xr_d[:, :].rearrange("p (c t) -> p c t", c=8)
            nc.sync.dma_start(out=w8["q"][:, :], in_=w8_d["q"][:, :])
            nc.sync.dma_start(out=x8v[:, :, 0:512], in_=x8dv[:, :, 0:512])
            nc.sync.dma_start(out=bq[:, :], in_=bq_d[:, :])
            nc.sync.dma_start(out=bk[:, :], in_=bk_d[:, :])
            nc.sync.dma_start(out=w8["k"][:, :], in_=w8_d["k"][:, :])
            nc.sync.dma_start(out=xrv[:, :, 0:512], in_=xrdv[:, :, 0:512])
            nc.sync.dma_start(out=wr["q"][:, :], in_=wr_d["q"][:, :])
            nc.sync.dma_start(out=wr["k"][:, :], in_=wr_d["k"][:, :])
            nc.sync.dma_start(out=w8["v"][:, :], in_=w8_d["v"][:, :])
            nc.sync.dma_start(out=wr["v"][:, :], in_=wr_d["v"][:, :])
            nc.sync.dma_start(out=bvr[:, :], in_=bvr_d[:, :])
            for tb in range(1, 4):
                tsl = slice(tb * 512, (tb + 1) * 512)
                nc.sync.dma_start(out=x8v[:, :, tsl], in_=x8dv[:, :, tsl])
                nc.sync.dma_start(out=xrv[:, :, tsl], in_=xrdv[:, :, tsl])
            nc.sync.dma_start(out=mlo[:, :], in_=mlo_d[:, :])
            nc.sync.dma_start(out=eye[:, :], in_=eye_d[:, :])
            nc.sync.dma_start(out=wp[:, :], in_=wp_d[:, :])

            warm = epool.tile([128, 2], bf16, tag="exp", name="warm")
            nc.scalar.activation(warm[:, :], bq[:, :],
                                 Exp, scale=1e-6)

            x8c = x8[:, :].rearrange("p (c t) -> p c t", c=8)
            xrc = xr[:, :].rearrange("p (c t) -> p c t", c=8)
            w8c = {k: w8[k][:, :].rearrange("p (c m) -> p c m", c=8) for k in "qkv"}
            wrc = {k: wr[k][:, :].rearrange("p (c m) -> p c m", c=8) for k in "qkv"}
            TERMS = [(x8c, w8c), (xrc, w8c), (x8c, wrc)]

            def qk_tile(key, dc, ts, pool):
                """One [128d, 512t] q/k projection tile + bias eviction."""
                b_sb, dst = (bq, qT) if key == "q" else (bk, kT)
                ps = pool.tile([128, 512], f32, tag=pool._qkv_tag,
                               name=f"p{key}_{dc}_{ts}")
                msl = slice(dc * 128, (dc + 1) * 128)
                nsl = slice(ts * 512, (ts + 1) * 512)
                for term, (xa, wd) in enumerate(TERMS):
                    for c in range(4):
                        nc.tensor.matmul(
                            ps[:, 0:512],
                            wd[key][:, 2 * c:2 * c + 2, msl],
                            xa[:, 2 * c:2 * c + 2, nsl],
                            start=(term == 0 and c == 0),
                            stop=(term == 2 and c == 3),
                            perf_mode=DR,
                        )
                nc.vector.tensor_scalar_add(
                    dst[:, dc * T + ts * 512: dc * T + (ts + 1) * 512],
                    ps[:, 0:512], b_sb[:, dc:dc + 1])

            def v_tile(tt, pool):
                """One [128t, 256d] V tile (natural layout) + bias eviction."""
                ps = pool.tile([128, 512], f32, tag=pool._qkv_tag,
                               name=f"pv_{tt}")
                tsl = slice(tt * 128, (tt + 1) * 128)
                for term, (xa, wd) in enumerate(TERMS):
                    for c in range(4):
                        nc.tensor.matmul(
                            ps[:, 0:DL],
                            xa[:, 2 * c:2 * c + 2, tsl],
                            wd["v"][:, 2 * c:2 * c + 2, :],
                            start=(term == 0 and c == 0),
                            stop=(term == 2 and c == 3),
                            perf_mode=DR,
                        )
                vdst = V[:, tt * VSTR:(tt + 1) * VSTR].rearrange(
                    "p (h e) -> p h e", h=HL)[:, :, 0:64]
                nc.vector.tensor_add(
                    vdst, ps[:, 0:DL].rearrange("p (h e) -> p h e", h=HL),
                    bvr[:, :].rearrange("p (h e) -> p h e", h=HL))

            psc._qkv_tag = "sc"
            pqkv._qkv_tag = "qkv"

            # ---- startup QKV: only what group (hp0, j0) needs ----
            nc.vector.memset(V[:, :], 1.0)  # ones cols; data overwritten
            # q0/k0 startup tiles, chunk-major interleaved so neither
            # blocks the other's ready matmuls in the PE FIFO
            ps_q = psc.tile([128, 512], f32, tag="sc", name="pq_0_0")
            ps_k = psc.tile([128, 512], f32, tag="sc", name="pk_0_0")
            for term, (xa, wd) in enumerate(TERMS):
                for c in range(4):
                    for key, ps in (("q", ps_q), ("k", ps_k)):
                        nc.tensor.matmul(
                            ps[:, 0:512],
                            wd[key][:, 2 * c:2 * c + 2, 0:128],
                            xa[:, 2 * c:2 * c + 2, 0:512],
                            start=(term == 0 and c == 0),
                            stop=(term == 2 and c == 3),
                            perf_mode=DR,
                        )
            nc.vector.tensor_scalar_add(qT[:, 0:512], ps_q[:, 0:512],
                                        bq[:, 0:1])
            nc.vector.tensor_scalar_add(kT[:, 0:512], ps_k[:, 0:512],
                                        bk[:, 0:1])
            v_tile(0, psc)
            v_tile(1, psc)

            # filler thunks: spread through attention on the pqkv ring, in
            # deadline order — each must be EMITTED before the attention
            # group that consumes it (program order is the dataflow).
            # (deadline_step, thunk): emitted no later than deadline, so
            # every attention read sees its producer earlier in program order
            filler = [
                (1, lambda: v_tile(2, pqkv)),
                (2, lambda: v_tile(3, pqkv)),
                (3, lambda: qk_tile("k", 0, 1, pqkv)),
                (3, lambda: qk_tile("q", 0, 1, pqkv)),
                (7, lambda: v_tile(4, pqkv)),
                (8, lambda: v_tile(5, pqkv)),
                (9, lambda: v_tile(6, pqkv)),
                (10, lambda: v_tile(7, pqkv)),
                (10, lambda: qk_tile("k", 0, 2, pqkv)),
                (10, lambda: qk_tile("q", 0, 2, pqkv)),
                (17, lambda: v_tile(8, pqkv)),
                (18, lambda: v_tile(9, pqkv)),
                (19, lambda: v_tile(10, pqkv)),
                (20, lambda: v_tile(11, pqkv)),
                (22, lambda: qk_tile("k", 0, 3, pqkv)),
                (22, lambda: qk_tile("q", 0, 3, pqkv)),
                (31, lambda: v_tile(12, pqkv)),
                (32, lambda: v_tile(13, pqkv)),
                (33, lambda: v_tile(14, pqkv)),
                (34, lambda: v_tile(15, pqkv)),
                (36, lambda: qk_tile("k", 1, 0, pqkv)),
                (37, lambda: qk_tile("q", 1, 0, pqkv)),
                (38, lambda: qk_tile("k", 1, 1, pqkv)),
                (39, lambda: qk_tile("q", 1, 1, pqkv)),
                (40, lambda: qk_tile("k", 1, 2, pqkv)),
                (41, lambda: qk_tile("q", 1, 2, pqkv)),
                (42, lambda: qk_tile("k", 1, 3, pqkv)),
                (43, lambda: qk_tile("q", 1, 3, pqkv)),
            ]

            pending = []   # (ready_step, thunk) deferred PE work
            step = [0]
            NSTEPS = 80    # total attention i steps; clamp deferrals

            def tick():
                step[0] += 1
                while filler and filler[0][0] <= step[0]:
                    filler.pop(0)[1]()
                while pending and pending[0][0] <= step[0]:
                    pending.pop(0)[1]()

            def proj_tile(tt, last):
                """Output projection for one 128-row t tile: [128, 1024]."""
                ot = opool.tile([128, 1024], bf16, tag="ot", name=f"ot_{tt}")
                for cc in range(2):
                    pp = pqkv.tile([128, 512], f32, tag="qkv",
                                   name=f"po_{tt}_{cc}")
                    for dc in range(2):
                        nc.tensor.matmul(
                            pp[:, :],
                            yT[:, dc * T + tt * 128: dc * T + (tt + 1) * 128],
                            wp[:, dc * C + cc * 512: dc * C + (cc + 1) * 512],
                            start=(dc == 0), stop=(dc == 1),
                        )
                    if last and cc == 0:
                        nc.scalar.copy(ot[:, cc * 512:(cc + 1) * 512],
                                       pp[:, :])
                    else:
                        nc.vector.tensor_copy(ot[:, cc * 512:(cc + 1) * 512],
                                              pp[:, :])
                    if last:
                        nc.sync.dma_start(
                            out=out_d[tt * 128:(tt + 1) * 128,
                                      cc * 512:(cc + 1) * 512],
                            in_=ot[:, cc * 512:(cc + 1) * 512])

                def out_dma(tt=tt, ot=ot):
                    nc.sync.dma_start(
                        out=out_d[tt * 128:(tt + 1) * 128, :], in_=ot[:, :])
                if last:
                    pass  # halves DMA'd inline below
                else:
                    pending.append((min(step[0] + 2, NSTEPS - 1), out_dma))

            def finish_tile(hp, j, tl, yu, last_group):
                """After tile tl's diagonal yu: stage, normalize, transpose,
                and (in the hp1 phase) queue the output projection."""
                tt = 4 * j + tl
                stg = spool.tile([128, 130], f32, tag="stg",
                                 name=f"stg_{hp}_{tt}")
                if last_group:
                    nc.scalar.copy(stg[:, :], yu[:, YOFF[tl]:YOFF[tl] + 130])
                else:
                    nc.vector.tensor_copy(stg[:, :],
                                          yu[:, YOFF[tl]:YOFF[tl] + 130])
                if dbg:
                    nc.sync.dma_start(
                        out=dbgstg_d[:, (hp * NT + tt) * 130:
                                     (hp * NT + tt + 1) * 130],
                        in_=stg[:, :])
                yn = npool.tile([128, 128], bf16, tag="yn",
                                name=f"yn_{hp}_{tt}")
                for lh in range(2):
                    nc.gpsimd.normalize_recip(
                        yn[:, lh * 64:(lh + 1) * 64],
                        stg[:, lh * 65: lh * 65 + 64],
                        stg[:, lh * 65 + 64: lh * 65 + 65],
                    )
                if dbg:
                    nc.sync.dma_start(
                        out=dbgyn_d[:, hp * T + tt * 128:
                                    hp * T + (tt + 1) * 128],
                        in_=yn[:, :])

                def transp(hp=hp, tt=tt, yn=yn):
                    nc.sync.dma_start(
                        out=yT[:, hp * T + tt * 128: hp * T + (tt + 1) * 128],
                        in_=yn[:, :], transpose=True)
                tail = last_group and tl == 3
                if last_group:
                    # tail tiles: PE transpose (~0.6us) instead of the XBAR
                    # DMA path (~1.9us of DGE+sem latency); the filler ring
                    # is quiet by now, the score ring after the last score
                    pool_, tag_ = (psc, "sc") if tail else (pqkv, "qkv")
                    pt = pool_.tile([128, 128], bf16, tag=tag_,
                                    name=f"pt_{hp}_{tt}")
                    nc.tensor.transpose(pt[:, :], yn[:, :], eye[:, :])
                    nc.vector.tensor_copy(
                        yT[:, hp * T + tt * 128: hp * T + (tt + 1) * 128],
                        pt[:, :])
                    if tail:
                        proj_tile(tt, True)
                    else:
                        pending.append((min(step[0] + 1, NSTEPS - 1),
                                        lambda tt=tt: proj_tile(tt, True)))
                else:
                    pending.append((min(step[0] + 2, NSTEPS - 2), transp))
                    if hp == 1:
                        pending.append((min(step[0] + 5, NSTEPS - 1),
                                        lambda tt=tt: proj_tile(tt, False)))

            yu_tiles = {}

            def make_step(hp, j, i, last_group):
                """Returns (sc_thunk, yu_thunk) for one i step."""
                ni = 4 * j + 4
                fb = hp * T
                d0 = max(128 * (i - 4 * j), 0)
                box = {}

                def sc_emit():
                    if i == 0:
                        yu_tiles[(hp, j)] = pyu.tile(
                            [128, 642], f32, tag="yu", name=f"yu_{hp}_{j}")
                    sc = psc.tile([128, 1024], f32, tag="sc",
                                  name=f"sc_{hp}_{j}_{i}")
                    for half in (0, 1):
                        po = 64 * half
                        nc.tensor.matmul(
                            sc[:, half * 512 + d0:(half + 1) * 512],
                            kT[po:po + 64, fb + i * 128: fb + (i + 1) * 128],
                            qT[po:po + 64,
                               fb + j * 512 + d0: fb + (j + 1) * 512],
                            start=True, stop=True,
                        )
                    et = epool.tile([128, 1024], bf16, tag="exp",
                                    name=f"et_{hp}_{j}_{i}")
                    nc.scalar.activation(
                        et[:, :].rearrange("p (g q) -> p g q", g=2)[:, :, d0:512],
                        sc[:, :].rearrange("p (g q) -> p g q", g=2)[:, :, d0:512],
                        Exp, scale=float(SCALE / (WSC * WSC)),
                    )
                    if i >= 4 * j:
                        for half in (0, 1):
                            sl = slice(half * 512 + d0, half * 512 + d0 + 128)
                            nc.vector.tensor_mul(et[:, sl], et[:, sl],
                                                 mlo[:, :])
                    box["et"] = et

                def yu_emit():
                    # start=True clears has_written for the WHOLE PSUM bank,
                    # so only the first matmul into each bank of the yu tile
                    # may carry it; later subtiles first-write on cleared
                    # bits (overwrite) with start=False.
                    yu, et = yu_tiles[(hp, j)], box["et"]
                    for tl in range(4):
                        if 128 * tl < d0:
                            continue
                        for half in (0, 1):
                            h = 2 * hp + half
                            nc.tensor.matmul(
                                yu[:, YOFF[tl] + 65 * half:
                                   YOFF[tl] + 65 * half + 65],
                                et[:, half * 512 + tl * 128:
                                   half * 512 + (tl + 1) * 128],
                                V[:, i * VSTR + 65 * h:
                                  i * VSTR + 65 * h + 65],
                                start=(i == 0 and half == 0 and tl in (0, 3)),
                                stop=(i == 4 * j + tl),
                                skip_group_check=True,
                            )
                    if i >= 4 * j:
                        finish_tile(hp, j, i - 4 * j, yu, last_group)

                return sc_emit, yu_emit

            steps = []
            for hp in range(2):
                for j in range(NJ):
                    for i in range(4 * j + 4):
                        steps.append(make_step(hp, j, i,
                                               hp == 1 and j == NJ - 1))
            # flat software pipeline: sc(i+1) is emitted before yu(i), incl.
            # across group boundaries, so the PE never heads-of-line on exp
            from collections import deque
            inflight = deque()
            for sc_emit, yu_emit in steps:
                sc_emit()
                if len(inflight) == 2:
                    inflight.popleft()()
                inflight.append(yu_emit)
                tick()
            while inflight:
                inflight.popleft()()
                tick()
            while filler:
                filler.pop(0)[1]()
            while pending:
                pending.pop(0)[1]()
            if dbg:
                for n, sb in (("qT", qT), ("kT", kT), ("yT", yT)):
                    nc.sync.dma_start(out=dbg_d[n][:, :], in_=sb[:, :])
                nc.sync.dma_start(out=dbgv_d[:, :], in_=V[:, :])

    nc.compile()
    return nc


def get_program():
    if "nc" not in _CACHE:
        _CACHE["nc"] = _build_program()
    return _CACHE["nc"]


def _pack_cmajor(a):
    """[C_rows, N] -> [128, (C_rows/128)*N] with chunk c at [:, c*N:(c+1)*N]."""
    rows, n = a.shape
    return np.ascontiguousarray(
        a.reshape(rows // 128, 128, n).transpose(1, 0, 2).reshape(128, -1))


def make_in_maps(x, W_attn, b_attn, W_proj):
    """Host-side sharding: per-core input dict."""
    x = np.asarray(x, np.float32)
    W_attn = np.asarray(W_attn, np.float32) * WSC
    b_attn = np.asarray(b_attn, np.float32) * WSC
    W_proj = np.asarray(W_proj, np.float32) / WSC

    mlo = (np.arange(128)[None, :] >= np.arange(128)[:, None]).astype(BF16)

    x8_b, xr_b = [], []
    for b in range(B):
        xt = x[b].T.astype(np.float32)                    # [C, T]
        x8 = xt.astype(F8)
        xres = (xt - x8.astype(np.float32)).astype(F8)
        x8_b.append(_pack_cmajor(x8))
        xr_b.append(_pack_cmajor(xres))

    in_maps = []
    for g in range(N_CORES):
        b, hg = divmod(g, 4)
        cs = slice(hg * DL, (hg + 1) * DL)
        m = {"x8p": x8_b[b], "xrp": xr_b[b], "mlo": mlo,
             "eye": np.eye(128).astype(BF16)}
        for ki, key in enumerate("qkv"):
            Wk = W_attn[:, ki * C:(ki + 1) * C][:, cs]
            W8 = Wk.astype(F8)
            Wr = (Wk - W8.astype(np.float32)).astype(F8)
            m[f"w8{key}"] = _pack_cmajor(W8)
            m[f"wr{key}"] = _pack_cmajor(Wr)
        m["wpp"] = _pack_cmajor(W_proj[cs, :].astype(BF16))
        m["bq"] = np.ascontiguousarray(
            b_attn[0 * C:1 * C][cs].reshape(2, 128).T).astype(np.float32)
        m["bk"] = np.ascontiguousarray(
            b_attn[1 * C:2 * C][cs].reshape(2, 128).T).astype(np.float32)
        m["bvr"] = np.ascontiguousarray(
            np.tile(b_attn[2 * C:3 * C][cs][None, :], (128, 1))).astype(np.float32)
        in_maps.append(m)
    return in_maps


def assemble_output(results, b_proj):
    """results: per-core dicts with 'out' [T, C] bf16 partials."""
    b_proj = np.asarray(b_proj, np.float32)
    out = np.zeros((B, T, C), np.float32)
    for g in range(N_CORES):
        out[g // 4] += np.asarray(results[g]["out"], np.float32)
    out += b_proj[None, None, :]
    return out


def kernel(x, W_attn, b_attn, W_proj, b_proj):
    from concourse.bass_utils import run_bass_kernel_spmd

    nc = get_program()
    in_maps = make_in_maps(x, W_attn, b_attn, W_proj)
    res = run_bass_kernel_spmd(nc, in_maps, list(range(N_CORES)))
    return assemble_output(res.results, b_proj)


# revision 73
# speedup vs baseline: 1.4155x; 1.0042x over previous
"""GPT2 causal attention (B=2, T=2048, C=1024, H=16) on 8 TRN2 NeuronCores.

Sharding: core g = (batch b = g//4, head-group hg = g%4 of 4 heads).
Tensor-parallel over heads x data-parallel over batch. Each core emits a
full [T, C] bf16 partial of the output projection for its 4 heads; host
sums the 4 partials per batch and adds b_proj. No collectives.

Per-core kernel:
  QKV projections run in compensated fp8 (e4m3) DoubleRow matmuls:
  x = x8 + xr and W = W8 + Wr host-split (W pre-scaled by 64), with
  x@W ~= x8@W8 + x8@Wr + xr@W8 (error ~ xr@Wr = O(eps^2), below bf16).
  Each DoubleRow matmul contracts 256 rows (a pair of 128-row k-tiles).

  Scores per head pair, transposed: S^T[tk, tq] = kT^T @ qT in bf16,
  causally trimmed to [d0:512] at 128-col granularity (both the matmuls
  and the exp), exp'd on ACT without max subtraction, diagonal 128-blocks
  masked with a host tri mask on DVE.

  attention*V runs in the natural orientation: per 128-row tq tile,
  yu[tq, (h, 65)] = et_chunk^T @ V_aug accumulates over tk tiles in PSUM;
  V_aug carries a ones-column per head so column 64 of each head group is
  the softmax row-sum, landing on the free dim. Normalization is then
  per-partition: gpsimd normalize_recip (attn library) divides by the sum
  and writes bf16. y is transposed back to [d, t] with XBAR DMA
  transposes for the bf16 output projection.
"""

import numpy as np
import ml_dtypes

BF16 = ml_dtypes.bfloat16
F8 = ml_dtypes.float8_e4m3

B, T, C, H, D = 2, 2048, 1024, 16, 64
HL = 4          # heads per core
DL = HL * D     # 256 local head dims
N_CORES = 8
NT = T // 128   # 16 t tiles
NJ = T // 512   # 4 tq groups
SCALE = 1.0 / np.sqrt(D)
WSC = 64.0      # host pre-scale on W_attn for fp8 range
VSTR = HL * 65  # V tile col stride (4 heads x (64 d + ones col))
YOFF = (0, 130, 260, 512)  # yu subtile col offsets (none crosses a bank)

_CACHE = {}


def _build_program():
    import concourse.tile as tile
    from concourse import bacc
    from concourse import library_config
    import concourse.mybir as mybir

    f32 = mybir.dt.float32
    bf16 = mybir.dt.bfloat16
    fp8 = mybir.dt.float8e4
    Exp = mybir.ActivationFunctionType.Exp
    DR = mybir.MatmulPerfMode.DoubleRow

    nc = bacc.Bacc("TRN2", target_bir_lowering=False, debug=False)

    x8_d = nc.dram_tensor("x8p", [128, 8 * T], fp8, kind="ExternalInput").ap()
    xr_d = nc.dram_tensor("xrp", [128, 8 * T], fp8, kind="ExternalInput").ap()
    w8_d = {k: nc.dram_tensor(f"w8{k}", [128, 8 * DL], fp8,
                              kind="ExternalInput").ap() for k in "qkv"}
    wr_d = {k: nc.dram_tensor(f"wr{k}", [128, 8 * DL], fp8,
                              kind="ExternalInput").ap() for k in "qkv"}
    wp_d = nc.dram_tensor("wpp", [128, 2 * C], bf16, kind="ExternalInput").ap()
    bq_d = nc.dram_tensor("bq", [128, 2], f32, kind="ExternalInput").ap()
    bk_d = nc.dram_tensor("bk", [128, 2], f32, kind="ExternalInput").ap()
    bvr_d = nc.dram_tensor("bvr", [128, DL], f32, kind="ExternalInput").ap()
    mlo_d = nc.dram_tensor("mlo", [128, 128], bf16, kind="ExternalInput").ap()
    eye_d = nc.dram_tensor("eye", [128, 128], bf16, kind="ExternalInput").ap()
    out_d = nc.dram_tensor("out", [T, C], bf16, kind="ExternalOutput").ap()
    import os
    dbg = os.environ.get("K_DEBUG") == "1"
    if dbg:
        dbg_d = {n: nc.dram_tensor(f"dbg_{n}", [128, 2 * T], bf16,
                                   kind="ExternalOutput").ap()
                 for n in ("qT", "kT", "yT")}
        dbgv_d = nc.dram_tensor("dbg_V", [128, NT * VSTR], bf16,
                                kind="ExternalOutput").ap()
        dbgyn_d = nc.dram_tensor("dbg_yn", [128, 2 * T], bf16,
                                 kind="ExternalOutput").ap()
        dbgstg_d = nc.dram_tensor("dbg_stg", [128, 32 * 130], f32,
                                  kind="ExternalOutput").ap()

    with tile.TileContext(nc) as tc:
        with (
            tc.tile_pool(name="const", bufs=1) as cpool,
            tc.tile_pool(name="exp", bufs=6) as epool,
            tc.tile_pool(name="ystg", bufs=6) as spool,
            tc.tile_pool(name="ynat", bufs=6) as npool,
            tc.tile_pool(name="ostage", bufs=6) as opool,
            tc.tile_pool(name="pssc", bufs=2, space="PSUM") as psc,
            tc.tile_pool(name="psqkv", bufs=2, space="PSUM") as pqkv,
            tc.tile_pool(name="psyu", bufs=1, space="PSUM") as pyu,
        ):
            # ---- persistent SBUF ----
            x8 = cpool.tile([128, 8 * T], fp8, tag="x8")     # chunk c at c*T
            xr = cpool.tile([128, 8 * T], fp8, tag="xr")
            w8 = {k: cpool.tile([128, 8 * DL], fp8, tag=f"w8{k}", name=f"w8{k}")
                  for k in "qkv"}
            wr = {k: cpool.tile([128, 8 * DL], fp8, tag=f"wr{k}", name=f"wr{k}")
                  for k in "qkv"}
            wp = cpool.tile([128, 2 * C], bf16, tag="wp")    # d-chunk dc at dc*C
            bq = cpool.tile([128, 2], f32, tag="bq")
            bk = cpool.tile([128, 2], f32, tag="bk")
            bvr = cpool.tile([128, DL], f32, tag="bvr")
            mlo = cpool.tile([128, 128], bf16, tag="mlo")
            eye = cpool.tile([128, 128], bf16, tag="eye")
            qT = cpool.tile([128, 2 * T], bf16, tag="qT")    # head h: rows 64*(h%2), cols (h//2)*T+t
            kT = cpool.tile([128, 2 * T], bf16, tag="kT")
            yT = cpool.tile([128, 2 * T], bf16, tag="yT")    # d-chunk dc at dc*T
            V = cpool.tile([128, NT * VSTR], bf16, tag="V")  # tile tt, head h at tt*VSTR+65h

            nc.gpsimd.load_library(library_config.attn)

            # ---- input DMAs ----
            # x streams in t-block-major quarters: one 3D-AP DMA per
            # quarter covers that t-range of all 8 c-chunks (512B runs),
            # so the first q/k/V tiles only wait for a quarter of x
            x8v = x8[:, :].rearrange("p (c t) -> p c t", c=8)
            xrv = xr[:, :].rearrange("p (c t) -> p c t", c=8)
            x8dv = x8_d[:, :].rearrange("p (c t) -> p c t", c=8)
            xrdv = xr_d[:, :].rearrange("p (c t) -> p c t", c=8)
            nc.sync.dma_start(out=w8["q"][:, :], in_=w8_d["q"][:, :])
            nc.sync.dma_start(out=x8v[:, :, 0:512], in_=x8dv[:, :, 0:512])
            nc.sync.dma_start(out=bq[:, :], in_=bq_d[:, :])
            nc.sync.dma_start(out=bk[:, :], in_=bk_d[:, :])
            nc.sync.dma_start(out=w8["k"][:, :], in_=w8_d["k"][:, :])
            nc.sync.dma_start(out=xrv[:, :, 0:512], in_=xrdv[:, :, 0:512])
            nc.sync.dma_start(out=wr["q"][:, :], in_=wr_d["q"][:, :])
            nc.sync.dma_start(out=wr["k"][:, :], in_=wr_d["k"][:, :])
            nc.sync.dma_start(out=w8["v"][:, :], in_=w8_d["v"][:, :])
            nc.sync.dma_start(out=wr["v"][:, :], in_=wr_d["v"][:, :])
            nc.sync.dma_start(out=bvr[:, :], in_=bvr_d[:, :])
            for tb in range(1, 4):
                tsl = slice(tb * 512, (tb + 1) * 512)
                nc.sync.dma_start(out=x8v[:, :, tsl], in_=x8dv[:, :, tsl])
                nc.sync.dma_start(out=xrv[:, :, tsl], in_=xrdv[:, :, tsl])
            nc.sync.dma_start(out=mlo[:, :], in_=mlo_d[:, :])
            nc.sync.dma_start(out=eye[:, :], in_=eye_d[:, :])
            nc.sync.dma_start(out=wp[:, :], in_=wp_d[:, :])

            warm = epool.tile([128, 2], bf16, tag="exp", name="warm")
            nc.scalar.activation(warm[:, :], bq[:, :],
                                 Exp, scale=1e-6)

            x8c = x8[:, :].rearrange("p (c t) -> p c t", c=8)
            xrc = xr[:, :].rearrange("p (c t) -> p c t", c=8)
            w8c = {k: w8[k][:, :].rearrange("p (c m) -> p c m", c=8) for k in "qkv"}
            wrc = {k: wr[k][:, :].rearrange("p (c m) -> p c m", c=8) for k in "qkv"}
            TERMS = [(x8c, w8c), (xrc, w8c), (x8c, wrc)]

            def qk_tile(key, dc, ts, pool):
                """One [128d, 512t] q/k projection tile + bias eviction."""
                b_sb, dst = (bq, qT) if key == "q" else (bk, kT)
                ps = pool.tile([128, 512], f32, tag=pool._qkv_tag,
                               name=f"p{key}_{dc}_{ts}")
                msl = slice(dc * 128, (dc + 1) * 128)
                nsl = slice(ts * 512, (ts + 1) * 512)
                for term, (xa, wd) in enumerate(TERMS):
                    for c in range(4):
                        nc.tensor.matmul(
                            ps[:, 0:512],
                            wd[key][:, 2 * c:2 * c + 2, msl],
                            xa[:, 2 * c:2 * c + 2, nsl],
                            start=(term == 0 and c == 0),
                            stop=(term == 2 and c == 3),
                            perf_mode=DR,
                        )
                nc.vector.tensor_scalar_add(
                    dst[:, dc * T + ts * 512: dc * T + (ts + 1) * 512],
                    ps[:, 0:512], b_sb[:, dc:dc + 1])

            def v_tile(tt, pool):
                """One [128t, 256d] V tile (natural layout) + bias eviction."""
                ps = pool.tile([128, 512], f32, tag=pool._qkv_tag,
                               name=f"pv_{tt}")
                tsl = slice(tt * 128, (tt + 1) * 128)
                for term, (xa, wd) in enumerate(TERMS):
                    for c in range(4):
                        nc.tensor.matmul(
                            ps[:, 0:DL],
                            xa[:, 2 * c:2 * c + 2, tsl],
                            wd["v"][:, 2 * c:2 * c + 2, :],
                            start=(term == 0 and c == 0),
                            stop=(term == 2 and c == 3),
                            perf_mode=DR,
                        )
                vdst = V[:, tt * VSTR:(tt + 1) * VSTR].rearrange(
                    "p (h e) -> p h e", h=HL)[:, :, 0:64]
                nc.vector.tensor_add(
                    vdst, ps[:, 0:DL].rearrange("p (h e) -> p h e", h=HL),
                    bvr[:, :].rearrange("p (h e) -> p h e", h=HL))

            psc._qkv_tag = "sc"
            pqkv._qkv_tag = "qkv"

            # ---- startup QKV: only what group (hp0, j0) needs ----
            nc.vector.memset(V[:, :], 1.0)  # ones cols; data overwritten
            # q0/k0 startup tiles, chunk-major interleaved so neither
            # blocks the other's ready matmuls in the PE FIFO
            ps_q = psc.tile([128, 512], f32, tag="sc", name="pq_0_0")
            ps_k = psc.tile([128, 512], f32, tag="sc", name="pk_0_0")
            for term, (xa, wd) in enumerate(TERMS):
                for c in range(4):
                    for key, ps in (("q", ps_q), ("k", ps_k)):
                        nc.tensor.matmul(
                            ps[:, 0:512],
                            wd[key][:, 2 * c:2 * c + 2, 0:128],
                            xa[:, 2 * c:2 * c + 2, 0:512],
                            start=(term == 0 and c == 0),
                            stop=(term == 2 and c == 3),
                            perf_mode=DR,
                        )
            nc.vector.tensor_scalar_add(qT[:, 0:512], ps_q[:, 0:512],
                                        bq[:, 0:1])
            nc.vector.tensor_scalar_add(kT[:, 0:512], ps_k[:, 0:512],
                                        bk[:, 0:1])
            v_tile(0, pqkv)
            v_tile(1, pqkv)

            # filler thunks: spread through attention on the pqkv ring, in
            # deadline order — each must be EMITTED before the attention
            # group that consumes it (program order is the dataflow).
            # (deadline_step, thunk): emitted no later than deadline, so
            # every attention read sees its producer earlier in program order
            filler = [
                (1, lambda: v_tile(2, pqkv)),
                (2, lambda: v_tile(3, pqkv)),
                (3, lambda: qk_tile("k", 0, 1, pqkv)),
                (3, lambda: qk_tile("q", 0, 1, pqkv)),
                (7, lambda: v_tile(4, pqkv)),
                (8, lambda: v_tile(5, pqkv)),
                (9, lambda: v_tile(6, pqkv)),
                (10, lambda: v_tile(7, pqkv)),
                (10, lambda: qk_tile("k", 0, 2, pqkv)),
                (10, lambda: qk_tile("q", 0, 2, pqkv)),
                (17, lambda: v_tile(8, pqkv)),
                (18, lambda: v_tile(9, pqkv)),
                (19, lambda: v_tile(10, pqkv)),
                (20, lambda: v_tile(11, pqkv)),
                (22, lambda: qk_tile("k", 0, 3, pqkv)),
                (22, lambda: qk_tile("q", 0, 3, pqkv)),
                (31, lambda: v_tile(12, pqkv)),
                (32, lambda: v_tile(13, pqkv)),
                (33, lambda: v_tile(14, pqkv)),
                (34, lambda: v_tile(15, pqkv)),
                (36, lambda: qk_tile("k", 1, 0, pqkv)),
                (37, lambda: qk_tile("q", 1, 0, pqkv)),
                (38, lambda: qk_tile("k", 1, 1, pqkv)),
                (39, lambda: qk_tile("q", 1, 1, pqkv)),
                (40, lambda: qk_tile("k", 1, 2, pqkv)),
                (41, lambda: qk_tile("q", 1, 2, pqkv)),
                (42, lambda: qk_tile("k", 1, 3, pqkv)),
                (43, lambda: qk_tile("q", 1, 3, pqkv)),
            ]

            pending = []   # (ready_step, thunk) deferred PE work
            step = [0]
            NSTEPS = 80    # total attention i steps; clamp deferrals

            def tick():
                step[0] += 1
                while filler and filler[0][0] <= step[0]:
                    filler.pop(0)[1]()
                while pending and pending[0][0] <= step[0]:
                    pending.pop(0)[1]()

            def proj_tile(tt, last):
                """Output projection for one 128-row t tile: [128, 1024]."""
                ot = opool.tile([128, 1024], bf16, tag="ot", name=f"ot_{tt}")
                for cc in range(2):
                    pp = pqkv.tile([128, 512], f32, tag="qkv",
                                   name=f"po_{tt}_{cc}")
                    for dc in range(2):
                        nc.tensor.matmul(
                            pp[:, :],
                            yT[:, dc * T + tt * 128: dc * T + (tt + 1) * 128],
                            wp[:, dc * C + cc * 512: dc * C + (cc + 1) * 512],
                            start=(dc == 0), stop=(dc == 1),
                        )
                    if last and cc == 0:
                        nc.scalar.copy(ot[:, cc * 512:(cc + 1) * 512],
                                       pp[:, :])
                    else:
                        nc.vector.tensor_copy(ot[:, cc * 512:(cc + 1) * 512],
                                              pp[:, :])
                    if last:
                        nc.sync.dma_start(
                            out=out_d[tt * 128:(tt + 1) * 128,
                                      cc * 512:(cc + 1) * 512],
                            in_=ot[:, cc * 512:(cc + 1) * 512])

                def out_dma(tt=tt, ot=ot):
                    nc.sync.dma_start(
                        out=out_d[tt * 128:(tt + 1) * 128, :], in_=ot[:, :])
                if last:
                    pass  # halves DMA'd inline below
                else:
                    pending.append((min(step[0] + 2, NSTEPS - 1), out_dma))

            def finish_tile(hp, j, tl, yu, last_group):
                """After tile tl's diagonal yu: stage, normalize, transpose,
                and (in the hp1 phase) queue the output projection."""
                tt = 4 * j + tl
                stg = spool.tile([128, 130], f32, tag="stg",
                                 name=f"stg_{hp}_{tt}")
                if last_group:
                    nc.scalar.copy(stg[:, :], yu[:, YOFF[tl]:YOFF[tl] + 130])
                else:
                    nc.vector.tensor_copy(stg[:, :],
                                          yu[:, YOFF[tl]:YOFF[tl] + 130])
                if dbg:
                    nc.sync.dma_start(
                        out=dbgstg_d[:, (hp * NT + tt) * 130:
                                     (hp * NT + tt + 1) * 130],
                        in_=stg[:, :])
                yn = npool.tile([128, 128], bf16, tag="yn",
                                name=f"yn_{hp}_{tt}")
                for lh in range(2):
                    nc.gpsimd.normalize_recip(
                        yn[:, lh * 64:(lh + 1) * 64],
                        stg[:, lh * 65: lh * 65 + 64],
                        stg[:, lh * 65 + 64: lh * 65 + 65],
                    )
                if dbg:
                    nc.sync.dma_start(
                        out=dbgyn_d[:, hp * T + tt * 128:
                                    hp * T + (tt + 1) * 128],
                        in_=yn[:, :])

                def transp(hp=hp, tt=tt, yn=yn):
                    nc.sync.dma_start(
                        out=yT[:, hp * T + tt * 128: hp * T + (tt + 1) * 128],
                        in_=yn[:, :], transpose=True)
                tail = last_group and tl == 3
                if last_group:
                    # tail tiles: PE transpose (~0.6us) instead of the XBAR
                    # DMA path (~1.9us of DGE+sem latency); the filler ring
                    # is quiet by now, the score ring after the last score
                    pool_, tag_ = (psc, "sc") if tail else (pqkv, "qkv")
                    pt = pool_.tile([128, 128], bf16, tag=tag_,
                                    name=f"pt_{hp}_{tt}")
                    nc.tensor.transpose(pt[:, :], yn[:, :], eye[:, :])
                    nc.vector.tensor_copy(
                        yT[:, hp * T + tt * 128: hp * T + (tt + 1) * 128],
                        pt[:, :])
                    if tail:
                        proj_tile(tt, True)
                    else:
                        pending.append((min(step[0] + 1, NSTEPS - 1),
                                        lambda tt=tt: proj_tile(tt, True)))
                else:
                    pending.append((min(step[0] + 2, NSTEPS - 2), transp))
                    if hp == 1:
                        pending.append((min(step[0] + 5, NSTEPS - 1),
                                        lambda tt=tt: proj_tile(tt, False)))

            yu_tiles = {}

            def make_step(hp, j, i, last_group):
                """Returns (sc_thunk, yu_thunk) for one i step."""
                ni = 4 * j + 4
                fb = hp * T
                d0 = max(128 * (i - 4 * j), 0)
                box = {}

                def sc_emit():
                    if i == 0:
                        yu_tiles[(hp, j)] = pyu.tile(
                            [128, 642], f32, tag="yu", name=f"yu_{hp}_{j}")
                    sc = psc.tile([128, 1024], f32, tag="sc",
                                  name=f"sc_{hp}_{j}_{i}")
                    for half in (0, 1):
                        po = 64 * half
                        nc.tensor.matmul(
                            sc[:, half * 512 + d0:(half + 1) * 512],
                            kT[po:po + 64, fb + i * 128: fb + (i + 1) * 128],
                            qT[po:po + 64,
                               fb + j * 512 + d0: fb + (j + 1) * 512],
                            start=True, stop=True,
                        )
                    et = epool.tile([128, 1024], bf16, tag="exp",
                                    name=f"et_{hp}_{j}_{i}")
                    nc.scalar.activation(
                        et[:, :].rearrange("p (g q) -> p g q", g=2)[:, :, d0:512],
                        sc[:, :].rearrange("p (g q) -> p g q", g=2)[:, :, d0:512],
                        Exp, scale=float(SCALE / (WSC * WSC)),
                    )
                    if i >= 4 * j:
                        for half in (0, 1):
                            sl = slice(half * 512 + d0, half * 512 + d0 + 128)
                            nc.vector.tensor_mul(et[:, sl], et[:, sl],
                                                 mlo[:, :])
                    box["et"] = et

                def yu_emit():
                    # start=True clears has_written for the WHOLE PSUM bank,
                    # so only the first matmul into each bank of the yu tile
                    # may carry it; later subtiles first-write on cleared
                    # bits (overwrite) with start=False.
                    yu, et = yu_tiles[(hp, j)], box["et"]
                    for tl in range(4):
                        if 128 * tl < d0:
                            continue
                        for half in (0, 1):
                            h = 2 * hp + half
                            nc.tensor.matmul(
                                yu[:, YOFF[tl] + 65 * half:
                                   YOFF[tl] + 65 * half + 65],
                                et[:, half * 512 + tl * 128:
                                   half * 512 + (tl + 1) * 128],
                                V[:, i * VSTR + 65 * h:
                                  i * VSTR + 65 * h + 65],
                                start=(i == 0 and half == 0 and tl in (0, 3)),
                                stop=(i == 4 * j + tl),
                                skip_group_check=True,
                            )
                    if i >= 4 * j:
                        finish_tile(hp, j, i - 4 * j, yu, last_group)

                return sc_emit, yu_emit

            steps = []
            for hp in range(2):
                for j in range(NJ):
                    for i in range(4 * j + 4):
                        steps.append(make_step(hp, j, i,
                                               hp == 1 and j == NJ - 1))
            # flat software pipeline: sc(i+1) is emitted before yu(i), incl.
            # across group boundaries, so the PE never heads-of-line on exp
            from collections import deque
            inflight = deque()
            for sc_emit, yu_emit in steps:
                sc_emit()
                if len(inflight) == 2:
                    inflight.popleft()()
                inflight.append(yu_emit)
                tick()
            while inflight:
                inflight.popleft()()
                tick()
            while filler:
                filler.pop(0)[1]()
            while pending:
                pending.pop(0)[1]()
            if dbg:
                for n, sb in (("qT", qT), ("kT", kT), ("yT", yT)):
                    nc.sync.dma_start(out=dbg_d[n][:, :], in_=sb[:, :])
                nc.sync.dma_start(out=dbgv_d[:, :], in_=V[:, :])

    nc.compile()
    return nc


def get_program():
    if "nc" not in _CACHE:
        _CACHE["nc"] = _build_program()
    return _CACHE["nc"]


def _pack_cmajor(a):
    """[C_rows, N] -> [128, (C_rows/128)*N] with chunk c at [:, c*N:(c+1)*N]."""
    rows, n = a.shape
    return np.ascontiguousarray(
        a.reshape(rows // 128, 128, n).transpose(1, 0, 2).reshape(128, -1))


def make_in_maps(x, W_attn, b_attn, W_proj):
    """Host-side sharding: per-core input dict."""
    x = np.asarray(x, np.float32)
    W_attn = np.asarray(W_attn, np.float32) * WSC
    b_attn = np.asarray(b_attn, np.float32) * WSC
    W_proj = np.asarray(W_proj, np.float32) / WSC

    mlo = (np.arange(128)[None, :] >= np.arange(128)[:, None]).astype(BF16)

    x8_b, xr_b = [], []
    for b in range(B):
        xt = x[b].T.astype(np.float32)                    # [C, T]
        x8 = xt.astype(F8)
        xres = (xt - x8.astype(np.float32)).astype(F8)
        x8_b.append(_pack_cmajor(x8))
        xr_b.append(_pack_cmajor(xres))

    in_maps = []
    for g in range(N_CORES):
        b, hg = divmod(g, 4)
        cs = slice(hg * DL, (hg + 1) * DL)
        m = {"x8p": x8_b[b], "xrp": xr_b[b], "mlo": mlo,
             "eye": np.eye(128).astype(BF16)}
        for ki, key in enumerate("qkv"):
            Wk = W_attn[:, ki * C:(ki + 1) * C][:, cs]
            W8 = Wk.astype(F8)
            Wr = (Wk - W8.astype(np.float32)).astype(F8)
            m[f"w8{key}"] = _pack_cmajor(W8)
            m[f"wr{key}"] = _pack_cmajor(Wr)
        m["wpp"] = _pack_cmajor(W_proj[cs, :].astype(BF16))
        m["bq"] = np.ascontiguousarray(
            b_attn[0 * C:1 * C][cs].reshape(2, 128).T).astype(np.float32)
        m["bk"] = np.ascontiguousarray(
            b_attn[1 * C:2 * C][cs].reshape(2, 128).T).astype(np.float32)
        m["bvr"] = np.ascontiguousarray(
            np.tile(b_attn[2 * C:3 * C][cs][None, :], (128, 1))).astype(np.float32)
        in_maps.append(m)
    return in_maps


def assemble_output(results, b_proj):
    """results: per-core dicts with 'out' [T, C] bf16 partials."""
    b_proj = np.asarray(b_proj, np.float32)
    out = np.zeros((B, T, C), np.float32)
    for g in range(N_CORES):
        out[g // 4] += np.asarray(results[g]["out"], np.float32)
    out += b_proj[None, None, :]
    return out


def kernel(x, W_attn, b_attn, W_proj, b_proj):
    from concourse.bass_utils import run_bass_kernel_spmd

    nc = get_program()
    in_maps = make_in_maps(x, W_attn, b_attn, W_proj)
    res = run_bass_kernel_spmd(nc, in_maps, list(range(N_CORES)))
    return assemble_output(res.results, b_proj)
